# revision 15
# baseline (speedup 1.0000x reference)
"""Multi-head causal attention (B=1, T=4096, D=768, H=12) on 8 trn2 cores.

Sharding: per core, slot A = one full head (heads 0-7 across the 8 cores);
slot B = half of a split head (heads 8-11, each split across 2 cores by
token PARITY: core 2k gets even tokens of head 8+k, core 2k+1 odd tokens).
Parity-splitting keeps the causal key extents identical across cores, so
every core runs the IDENTICAL program (SPMD); cores differ only in data
(weights, masks, parity).  Slot B's queries are packed (parity-strided
projection); its head outputs are written back to natural token columns
with stride-2 DVE writes, so one merged out-projection covers both slots
and the host just sums the 8 partial [768, 4096] outputs.

Per-core work: slot A = 144 key-chunk units, slot B = 80 units (vs 288 for
the old 2-full-slot scheme).  V-bias is folded into a host-side constant
(P@(V+1 bv^T) = P@V + denom bv^T, exact through softmax normalization).

On-device layout (per core):
  xT    [768, 4096] bf16   x transposed (host supplies)
  K_sb  [128, 4096] bf16   rows 0:64 head-A K dims, 64:128 head-B K dims
  Q_sb  [128, 4096] bf16   rows 0:64 head-A Q (natural); rows 64:128 cols
                           0:2048 head-B Q (parity-packed)
  V2    [128, 32*208] bf16 per key-chunk: [V_A 0:64 |1@64| 0 |1@97| 0 |
                           V_B 129:193] -> one matmul per slot yields AV
                           rows + a denominator row (A: row 64, B: row 32)
  scores chunks [128 keys, 512 q] in PSUM, exp'd on ACT -> PT bf16
  outT = wo2^T @ ho per 512-query tile, DMA'd from PSUM as f32 partials
"""

import math
import numpy as np
import ml_dtypes
from contextlib import ExitStack

import concourse.bass as bass
import concourse.bacc as bacc
import concourse.mybir as mybir
import concourse.tile as tile
from concourse.bass_utils import run_bass_kernel_spmd

BF16 = mybir.dt.bfloat16
F32 = mybir.dt.float32
AF = mybir.ActivationFunctionType

T = 4096
D_MODEL = 768
HEAD_DIM = 64
N_HEADS = 12
N_CORES = 8
QT = 512                  # query tile width (A natural / B packed)
KC = 128                  # key chunk (psum partition dim)
GRP = 2                   # score chunks per exp group (psum banks)
NQT = T // QT             # 8 A-tiles
NPB = 4                   # B packed tiles (each covers 1024 natural tokens)
CCH = D_MODEL // 128      # 6 contraction chunks
VST = 208                 # V2 stride per key chunk
NMASK = 13                # 4 A diag patterns + 8 B patterns + parity col mask

_PROGRAM_CACHE = {}


def build_program():
    nc = bacc.Bacc(None)

    xT_d = nc.declare_dram_parameter("xT", [D_MODEL, T], BF16, isOutput=False)
    # x columns of this core's parity, packed: x[:, parity::2].T
    xTB_d = nc.declare_dram_parameter("xTB", [D_MODEL, T // 2], BF16, isOutput=False)
    # w cols: 0:64 wq_A | 64:192 wk_AB | 192:320 wv_AB | 320:384 wq_B
    w_d = nc.declare_dram_parameter("wproj", [D_MODEL, 384], BF16, isOutput=False)
    b_d = nc.declare_dram_parameter("bqk", [128, 3], F32, isOutput=False)
    wo_d = nc.declare_dram_parameter("wo2", [128, D_MODEL], BF16, isOutput=False)
    mk_d = nc.declare_dram_parameter("masks", [NMASK, 128, QT], BF16, isOutput=False)
    outT_d = nc.declare_dram_parameter("outT", [D_MODEL, T], BF16, isOutput=True)

    with tile.TileContext(nc) as tc, ExitStack() as ctx:
        consts = ctx.enter_context(tc.tile_pool(name="consts", bufs=1))
        big = ctx.enter_context(tc.tile_pool(name="big", bufs=1))
        ptp = ctx.enter_context(tc.tile_pool(name="ptp", bufs=3))
        osb = ctx.enter_context(tc.tile_pool(name="osb", bufs=3))
        rp = ctx.enter_context(tc.tile_pool(name="rp", bufs=2))
        dramp = ctx.enter_context(tc.tile_pool(name="dramp", bufs=2, space="DRAM"))
        # PSUM: scp 2 tiles x 2 banks + avp 2 x 1 + opp 2 x 1 = 8 banks
        scp = ctx.enter_context(tc.tile_pool(name="scp", bufs=2, space="PSUM"))
        avp = ctx.enter_context(tc.tile_pool(name="avp", bufs=2, space="PSUM"))
        opp = ctx.enter_context(tc.tile_pool(name="opp", bufs=2, space="PSUM"))

        # ---- inputs to SBUF ----
        xT_sb = []
        for j in range(CCH):
            t = big.tile([128, T], BF16, tag=f"xT{j}")
            nc.sync.dma_start(out=t[:], in_=xT_d[j * 128:(j + 1) * 128, :])
            xT_sb.append(t)
        xTB_sb = []
        for j in range(CCH):
            t = big.tile([128, T // 2], BF16, tag=f"xTB{j}")
            nc.sync.dma_start(out=t[:], in_=xTB_d[j * 128:(j + 1) * 128, :])
            xTB_sb.append(t)
        w_sb = consts.tile([128, CCH * 384], BF16, tag="w")
        for j in range(CCH):
            nc.sync.dma_start(
                out=w_sb[:, j * 384:(j + 1) * 384],
                in_=w_d[j * 128:(j + 1) * 128, :],
            )
        b_sb = consts.tile([128, 3], F32, tag="b")
        nc.sync.dma_start(out=b_sb[:], in_=b_d[:, :])
        wo_sb = consts.tile([128, D_MODEL], BF16, tag="wo")
        nc.sync.dma_start(out=wo_sb[:], in_=wo_d[:, :])
        mask_sb = consts.tile([128, NMASK * QT], BF16, tag="mask")
        for p in range(NMASK):
            nc.sync.dma_start(out=mask_sb[:, p * QT:(p + 1) * QT], in_=mk_d[p, :, :])

        # ---- persistent tensors ----
        K_sb = big.tile([128, T], BF16, tag="K")
        Q_sb = big.tile([128, T], BF16, tag="Q")
        V2 = big.tile([128, (T // KC) * VST], BF16, tag="V2")
        ho = big.tile([128, T], BF16, tag="ho")
        nc.vector.memset(ho[:], 0.0)
        nc.vector.memset(V2[:], 0.0)
        v3 = V2[:].rearrange("p (t c) -> p t c", c=VST)
        nc.vector.memset(v3[:, :, 64:65], 1.0)    # ones row for denom_A
        nc.vector.memset(v3[:, :, 97:98], 1.0)    # ones row for denom_B

        def emit_proj(tt):
            ts = slice(tt * QT, (tt + 1) * QT)
            odd = tt % 2 == 1
            pb = (tt - 1) // 2
            # K (both slots, M=128) + Q_A (M=64) + packed Q_B (M=64, odd tt)
            pk = scp.tile([128, 2 * QT], F32, tag="sc")
            for j in range(CCH):
                rhs = xT_sb[j][:, ts]
                st, sp = j == 0, j == CCH - 1
                nc.tensor.matmul(
                    pk[:, 0:QT], w_sb[:, j * 384 + 64:j * 384 + 192], rhs,
                    start=st, stop=sp, tile_position=(0, 0),
                )
                nc.tensor.matmul(
                    pk[0:64, QT:2 * QT], w_sb[:, j * 384:j * 384 + 64], rhs,
                    start=st, stop=sp, tile_position=(0, 0),
                )
                if odd:
                    nc.tensor.matmul(
                        pk[64:128, QT:2 * QT],
                        w_sb[:, j * 384 + 320:j * 384 + 384],
                        xTB_sb[j][:, pb * QT:(pb + 1) * QT],
                        start=st, stop=sp, tile_position=(0, 64),
                    )
            nc.vector.tensor_scalar_add(K_sb[:, ts], pk[:, 0:QT], b_sb[:, 1:2])
            nc.vector.tensor_scalar_add(
                Q_sb[0:64, ts], pk[0:64, QT:2 * QT], b_sb[0:64, 0:1])
            if odd:
                nc.vector.tensor_scalar_add(
                    Q_sb[64:128, pb * QT:(pb + 1) * QT],
                    pk[64:128, QT:2 * QT], b_sb[64:128, 2:3])
            # V direct [token, dim]: psum [128 tok, 128 dims] per sub-tile
            pv = scp.tile([128, 2 * QT], F32, tag="sc")
            for sub in range(4):
                kc = tt * 4 + sub
                for j in range(CCH):
                    nc.tensor.matmul(
                        pv[:, sub * 128:(sub + 1) * 128],
                        xT_sb[j][:, kc * KC:(kc + 1) * KC],
                        w_sb[:, j * 384 + 192:j * 384 + 320],
                        start=(j == 0), stop=(j == CCH - 1), tile_position=(0, 0),
                    )
            for sub in range(4):
                kc = tt * 4 + sub
                blk = V2[:, kc * VST:kc * VST + 193]
                out_ap = bass.AP(tensor=blk.tensor, offset=blk.offset,
                                 ap=[list(blk.ap[0]), [129, 2], [1, 64]])
                nc.vector.tensor_copy(out_ap, pv[:, sub * 128:(sub + 1) * 128])

        def emit_job(slot, i):
            """slot 'A': full head, query tile i (natural); slot 'B': split
            head, packed tile i."""
            if slot == "A":
                nst, band0 = 4 * (i + 1), 4 * i
                qap = Q_sb[0:64, i * QT:(i + 1) * QT]
                krow, tp = 0, (0, 0)
            else:
                nst, band0 = 8 * (i + 1), 8 * i
                qap = Q_sb[64:128, i * QT:(i + 1) * QT]
                krow, tp = 64, (64, 0)
            av = avp.tile([128, QT], F32, tag="av")
            for g0 in range(0, nst, GRP):
                sc = scp.tile([128, GRP * QT], F32, tag="sc")
                for gi in range(GRP):
                    kc = g0 + gi
                    nc.tensor.matmul(
                        sc[:, gi * QT:(gi + 1) * QT],
                        K_sb[krow:krow + 64, kc * KC:(kc + 1) * KC],
                        qap, start=True, stop=True, tile_position=tp,
                    )
                pt = ptp.tile([128, GRP * QT], BF16, tag="pt")
                nc.scalar.activation(
                    pt[:], sc[:], AF.Exp, scale=1.0 / math.sqrt(HEAD_DIM))
                for gi in range(GRP):
                    kc = g0 + gi
                    ptj = pt[:, gi * QT:(gi + 1) * QT]
                    if kc >= band0:
                        mi = (kc - band0) if slot == "A" else 4 + (kc - band0)
                        nc.vector.tensor_mul(
                            ptj, ptj, mask_sb[:, mi * QT:(mi + 1) * QT])
                    st, sp = kc == 0, kc == nst - 1
                    if slot == "A":
                        nc.tensor.matmul(
                            av[0:65, :], V2[:, kc * VST:kc * VST + 65], ptj,
                            start=st, stop=sp, tile_position=(0, 0),
                        )
                    else:
                        nc.tensor.matmul(
                            av[0:128, :], V2[:, kc * VST + 65:kc * VST + 193],
                            ptj, start=st, stop=sp, tile_position=(0, 0),
                        )
            # normalize: reciprocal of denom, partition-broadcast via DRAM
            drow = 64 if slot == "A" else 32
            r = rp.tile([128, QT], F32, tag="r")
            nc.vector.reciprocal(r[drow:drow + 1, :], av[drow:drow + 1, :])
            rd = dramp.tile([1, QT], F32, tag="rd")
            nc.sync.dma_start(out=rd[0:1, :], in_=r[drow:drow + 1, :])
            rbc = rp.tile([128, QT], F32, tag="rbc")
            rows = slice(0, 64) if slot == "A" else slice(64, 128)
            rdap = rd[0:1, :]
            nc.gpsimd.dma_start(
                out=rbc[rows, :],
                in_=bass.AP(tensor=rdap.tensor, offset=rdap.offset,
                            ap=[[0, 64]] + [list(d) for d in rdap.ap[1:]]))
            if slot == "A":
                nc.vector.tensor_mul(
                    ho[0:64, i * QT:(i + 1) * QT], av[0:64, :], rbc[0:64, :])
            else:
                # write packed value j to BOTH natural columns 2j and 2j+1;
                # the per-core parity column mask (data) zeroes the wrong one
                # right before the out-projection.
                hob = ho[64:128, 1024 * i:1024 * (i + 1)].rearrange(
                    "p (n two) -> p two n", two=2)
                nc.vector.tensor_mul(hob[:, 0:1, :], av[64:128, :], rbc[64:128, :])
                nc.vector.tensor_mul(hob[:, 1:2, :], av[64:128, :], rbc[64:128, :])

        def emit_outproj(qt):
            qs = slice(qt * QT, (qt + 1) * QT)
            nc.vector.tensor_mul(
                ho[64:128, qs], ho[64:128, qs], mask_sb[64:128, 12 * QT:13 * QT])
            for dch in range(CCH):
                op = opp.tile([128, QT], F32, tag="op")
                nc.tensor.matmul(
                    op[:], wo_sb[:, dch * 128:(dch + 1) * 128],
                    ho[:, qt * QT:(qt + 1) * QT], start=True, stop=True,
                    tile_position=(0, 0),
                )
                ot = osb.tile([128, QT], BF16, tag="ot")
                nc.vector.tensor_copy(ot[:], op[:])
                nc.sync.dma_start(
                    out=outT_d[dch * 128:(dch + 1) * 128, qt * QT:(qt + 1) * QT],
                    in_=ot[:])

        # ---- wavefront emission ----
        for tt in range(NQT):
            emit_proj(tt)
            if tt % 2 == 1:
                emit_job("B", (tt - 1) // 2)
            emit_job("A", tt)
            if tt % 2 == 1:
                emit_outproj(tt - 1)
                emit_outproj(tt)
    nc.finalize()
    return nc


def _host_inputs(x, wq, bq, wk, bk, wv, bv, wo):
    """Per-core input maps. Slot A of core c = head c; slot B = split head
    8 + c//2 with token parity c%2."""
    bf16 = ml_dtypes.bfloat16
    xT = np.ascontiguousarray(x[0].T).astype(bf16)
    xTB_by_par = [np.ascontiguousarray(x[0][p::2].T).astype(bf16) for p in (0, 1)]

    in_maps = []
    for c in range(N_CORES):
        ha, hb, par = c, 8 + c // 2, c % 2
        w = np.zeros((D_MODEL, 384), np.float32)
        w[:, 0:64] = wq[ha]
        w[:, 64:128] = wk[ha]
        w[:, 128:192] = wk[hb]
        w[:, 192:256] = wv[ha]
        w[:, 256:320] = wv[hb]
        w[:, 320:384] = wq[hb]
        b = np.zeros((128, 3), np.float32)
        b[0:64, 0] = bq[ha]
        b[0:64, 1] = bk[ha]
        b[64:128, 1] = bk[hb]
        b[64:128, 2] = bq[hb]
        wo2 = np.zeros((128, D_MODEL), np.float32)
        wo2[0:64] = wo[ha * 64:(ha + 1) * 64]
        wo2[64:128] = wo[hb * 64:(hb + 1) * 64]
        kl = np.arange(128)[:, None]
        qq = np.arange(QT)[None, :]
        masks = np.zeros((NMASK, 128, QT), np.float32)
        for pat in range(4):
            masks[pat] = (128 * pat + kl) <= qq
        for pat in range(8):
            masks[4 + pat] = (128 * pat + kl) <= (2 * qq + par)
        masks[12, :, :] = (qq % 2 == par)
        in_maps.append({
            "xT": xT,
            "xTB": xTB_by_par[par],
            "wproj": w.astype(bf16),
            "bqk": b.astype(np.float32),
            "wo2": wo2.astype(bf16),
            "masks": masks.astype(bf16),
        })
    return in_maps


def kernel(_trace=False, _tmpdir=None, **inputs):
    x = np.asarray(inputs["x"], np.float32)
    wq = np.asarray(inputs["wq"], np.float32)
    bq = np.asarray(inputs["bq"], np.float32)
    wk = np.asarray(inputs["wk"], np.float32)
    bk = np.asarray(inputs["bk"], np.float32)
    wv = np.asarray(inputs["wv"], np.float32)
    bv = np.asarray(inputs["bv"], np.float32)
    wo = np.asarray(inputs["wo"], np.float32)
    bo = np.asarray(inputs["bo"], np.float32)

    if "nc" not in _PROGRAM_CACHE:
        _PROGRAM_CACHE["nc"] = build_program()
    nc = _PROGRAM_CACHE["nc"]

    in_maps = _host_inputs(x, wq, bq, wk, bk, wv, bv, wo)
    res = run_bass_kernel_spmd(
        nc, in_maps, list(range(N_CORES)), trace=_trace, tmpdir=_tmpdir,
    )
    acc = np.zeros((D_MODEL, T), np.float32)
    for c in range(N_CORES):
        acc += res.results[c]["outT"]
    # V-bias folds to a constant through softmax: + bv_cat @ wo (+ bo)
    const = bv.reshape(-1) @ wo + bo
    out = acc.T + const[None, :]
    if _trace:
        return out[None].astype(np.float32), res
    return out[None].astype(np.float32)


# revision 59
# speedup vs baseline: 1.1718x; 1.1718x over previous
"""Multi-head causal attention (B=1, T=4096, D=768, H=12) on 8 trn2 cores.

Sharding: per core, slot A = one full head (heads 0-7 across the 8 cores);
slot B = half of a split head (heads 8-11, each split across 2 cores by
token PARITY: core 2k gets even tokens of head 8+k, core 2k+1 odd tokens).
Parity-splitting keeps the causal key extents identical across cores, so
every core runs the IDENTICAL program (SPMD); cores differ only in data
(weights, masks, parity).  Slot B's queries are packed (parity-strided
projection); its head outputs are written back to natural token columns
with stride-2 DVE writes, so one merged out-projection covers both slots
and the host just sums the 8 partial [768, 4096] outputs.

Per-core work: slot A = 144 key-chunk units, slot B = 80 units (vs 288 for
the old 2-full-slot scheme).  V-bias is folded into a host-side constant
(P@(V+1 bv^T) = P@V + denom bv^T, exact through softmax normalization).

On-device layout (per core):
  xT    [768, 4096] bf16   x transposed (host supplies); xTB = parity cols
  K_sb  [128, 4096] bf16   rows 0:64 head-A K dims, 64:128 head-B K dims
  Q_sb  [128, 4096] bf16   rows 0:64 head-A Q (natural); rows 64:128 cols
                           0:2048 head-B Q (parity-packed)
  V2    [128, 32*208] bf16 per key-chunk: [V_A 0:64 |1@64| 0 |1@97| 0 |
                           V_B 129:193] -> one matmul per slot yields AV
                           rows + a denominator row (A: row 64, B: row 32)
  scores chunks [128 keys, 512 q] in PSUM, exp'd on ACT -> PT bf16

Scheduling: emission is software-pipelined — projection of token tile
tt+1 and deferred xT loads are woven between the attention score groups
of stage tt, and each group's mask+AV matmuls are delayed 8 groups behind
its exp so PE never stalls on the exp latency.  Softmax normalization
broadcasts 1/denom across partitions with a K=1 ones-matmul on PE (no
DRAM bounce).  Out-projection per 512-query tile -> bf16 partials.
"""

import math
import numpy as np
import ml_dtypes
from contextlib import ExitStack

import concourse.bass as bass
import concourse.bacc as bacc
import concourse.mybir as mybir
import concourse.tile as tile
from concourse.bass_utils import run_bass_kernel_spmd

BF16 = mybir.dt.bfloat16
F32 = mybir.dt.float32
F8 = mybir.dt.float8e4
DR = mybir.MatmulPerfMode.DoubleRow
AF = mybir.ActivationFunctionType

T = 4096
D_MODEL = 768
HEAD_DIM = 64
N_HEADS = 12
N_CORES = 8
QT = 512                  # query tile width (A natural / B packed)
KC = 128                  # key chunk (psum partition dim)
GRP = 2                   # score chunks per exp group (psum banks)
NQT = T // QT             # 8 A-tiles
NPB = 4                   # B packed tiles (each covers 1024 natural tokens)
CCH = D_MODEL // 128      # 6 contraction chunks
VST = 208                 # V2 stride per key chunk
NMASK = 13                # 4 A diag patterns + 8 B patterns + parity col mask

_PROGRAM_CACHE = {}


def build_program():
    nc = bacc.Bacc(None)

    xT_d = nc.declare_dram_parameter("xT", [D_MODEL, T], BF16, isOutput=False)
    # x columns of this core's parity, packed: x[:, parity::2].T
    xTB_d = nc.declare_dram_parameter("xTB", [D_MODEL, T // 2], BF16, isOutput=False)
    # w cols: 0:64 wq_A | 64:192 wk_AB | 192:320 wv_AB | 320:384 wq_B
    w_d = nc.declare_dram_parameter("wproj", [D_MODEL, 384], BF16, isOutput=False)
    b_d = nc.declare_dram_parameter("bqk", [128, 3], F32, isOutput=False)
    wo_d = nc.declare_dram_parameter("wo2", [128, D_MODEL], BF16, isOutput=False)
    mk_d = nc.declare_dram_parameter("masks", [NMASK, 128, QT], BF16, isOutput=False)
    outT_d = nc.declare_dram_parameter("outT", [D_MODEL, T], BF16, isOutput=True)

    with tile.TileContext(nc) as tc, ExitStack() as ctx:
        consts = ctx.enter_context(tc.tile_pool(name="consts", bufs=1))
        big = ctx.enter_context(tc.tile_pool(name="big", bufs=1))
        ptp = ctx.enter_context(tc.tile_pool(name="ptp", bufs=int(__import__("os").environ.get("KPTP", "9"))))
        osb = ctx.enter_context(tc.tile_pool(name="osb", bufs=3))
        rp = ctx.enter_context(tc.tile_pool(name="rp", bufs=2))
        dramp = ctx.enter_context(tc.tile_pool(name="dramp", bufs=2, space="DRAM"))
        # PSUM budget is 8 banks total; knobs for the split
        import os as _os
        fp8 = _os.environ.get("KFP8", "0") == "1"
        grp = int(_os.environ.get("KGRP", str(GRP)))
        _scb = int(_os.environ.get("KSCB", "2"))
        _avp = int(_os.environ.get("KAVP", "3"))
        _opp = int(_os.environ.get("KOPP", "1"))
        scp = ctx.enter_context(tc.tile_pool(name="scp", bufs=_scb, space="PSUM"))
        avp = ctx.enter_context(tc.tile_pool(name="avp", bufs=_avp, space="PSUM"))
        if _opp > 0:
            opp = ctx.enter_context(
                tc.tile_pool(name="opp", bufs=_opp, space="PSUM"))
        else:
            opp = avp

        # ---- inputs to SBUF: small consts first, then xT streamed in
        # token-tile slices so stage-0 projection starts within ~4us ----
        w_sb = consts.tile([128, CCH * 384], BF16, tag="w")
        _wap = w_d[:, :]
        nc.sync.dma_start(
            out=w_sb[:],
            in_=bass.AP(tensor=_wap.tensor, offset=_wap.offset,
                        ap=[[384, 128], [128 * 384, CCH], [1, 384]]))
        b_sb = consts.tile([128, 3], F32, tag="b")
        nc.sync.dma_start(out=b_sb[:], in_=b_d[:, :])
        wo_sb = consts.tile([128, D_MODEL], BF16, tag="wo")
        nc.sync.dma_start(out=wo_sb[:], in_=wo_d[:, :])
        mask_sb = consts.tile([128, NMASK * QT], BF16, tag="mask")
        _map = mk_d[:, :, :]
        nc.sync.dma_start(
            out=mask_sb[:],
            in_=bass.AP(tensor=_map.tensor, offset=_map.offset,
                        ap=[[QT, 128], [128 * QT, NMASK], [1, QT]]))
        # xT sliced per token-tile pair so stage-0/1 projection starts early;
        # first xTB slice interleaved (stage 1 needs it)
        xT_sb = [big.tile([128, T], BF16, tag=f"xT{j}", name=f"xT{j}")
                 for j in range(CCH)]
        xTB_sb = [big.tile([128, T // 2], BF16, tag=f"xTB{j}", name=f"xTB{j}")
                  for j in range(CCH)]
        def load_xt(tp):
            cs = slice(tp * 2 * QT, (tp + 1) * 2 * QT)
            for j in range(CCH):
                nc.sync.dma_start(out=xT_sb[j][:, cs],
                                  in_=xT_d[j * 128:(j + 1) * 128, cs])

        def load_xtb(pb):
            cs = slice(pb * QT, (pb + 1) * QT)
            for j in range(CCH):
                nc.sync.dma_start(out=xTB_sb[j][:, cs],
                                  in_=xTB_d[j * 128:(j + 1) * 128, cs])

        # only the slices stage 0/1 need; the rest are woven in as filler
        load_xt(0)
        load_xtb(0)

        # ---- persistent tensors ----
        QKDT = F8 if fp8 else BF16
        K_sb = big.tile([128, T], QKDT, tag="K")
        Q_sb = big.tile([128, T], QKDT, tag="Q")
        if fp8:
            # plane layout for DoubleRow: rows 0:32 slot A (head-dim planes
            # 0:32 / 32:64 at byte offsets 0 / T), rows 32:64 slot B
            K8p = big.tile([128, 2 * T], F8, tag="K8p")
            Q8p = big.tile([128, 2 * T], F8, tag="Q8p")
            K8p3 = K8p[:].rearrange("p (two n) -> p two n", two=2)
            Q8p3 = Q8p[:].rearrange("p (two n) -> p two n", two=2)
        V2 = big.tile([128, (T // KC) * VST], BF16, tag="V2")
        ho = big.tile([128, T], BF16, tag="ho")
        nc.gpsimd.memset(ho[:], 0.0)
        nc.gpsimd.memset(V2[:], 0.0)
        v3 = V2[:].rearrange("p (t c) -> p t c", c=VST)
        nc.gpsimd.memset(v3[:, :, 64:65], 1.0)    # ones row for denom_A
        nc.gpsimd.memset(v3[:, :, 97:98], 1.0)    # ones row for denom_B
        ones64 = consts.tile([128, 64], BF16, tag="ones64")
        nc.gpsimd.memset(ones64[:], 1.0)          # lhsT for recip broadcast

        def proj_units(tt):
            """List of thunks emitting projection for token tile tt."""
            ts = slice(tt * QT, (tt + 1) * QT)
            odd = tt % 2 == 1
            pb = (tt - 1) // 2
            st_ = {}
            units = []

            # NOTE: only ONE pending psum accumulation group per 2KB bank —
            # K (bank0) + Q_A (bank1) may interleave, but Q_B (also bank1)
            # and each V sub-group (all in pv bank0) must run after the
            # previous same-bank group has stopped.
            def u_kq(j):
                if j == 0:
                    st_["pk"] = scp.tile([128, grp * QT], F32, tag="sc",
                                         name="pk")
                pk = st_["pk"]
                rhs = xT_sb[j][:, ts]
                st, sp = j == 0, j == CCH - 1
                nc.tensor.matmul(
                    pk[:, 0:QT], w_sb[:, j * 384 + 64:j * 384 + 192], rhs,
                    start=st, stop=sp, tile_position=(0, 0),
                )
                nc.tensor.matmul(
                    pk[0:64, QT:2 * QT], w_sb[:, j * 384:j * 384 + 64], rhs,
                    start=st, stop=sp, tile_position=(0, 0),
                )

            def u_qb(j):
                pk = st_["pk"]
                nc.tensor.matmul(
                    pk[64:128, QT:2 * QT],
                    w_sb[:, j * 384 + 320:j * 384 + 384],
                    xTB_sb[j][:, pb * QT:(pb + 1) * QT],
                    start=(j == 0), stop=(j == CCH - 1), tile_position=(0, 64),
                )

            def u_cast_kqa():
                pk = st_["pk"]
                nc.vector.tensor_scalar_add(K_sb[:, ts], pk[:, 0:QT], b_sb[:, 1:2])
                nc.vector.tensor_scalar_add(
                    Q_sb[0:64, ts], pk[0:64, QT:2 * QT], b_sb[0:64, 0:1])

            def u_regroup_kqa():
                # fp8 plane regroup: flat rows (4 groups of 32) -> plane
                # layout rows 0:32 (A) / 32:64 (B), byte offset 0 / T.
                # SWDGE (gpsimd) path keeps these off the busy HWDGE.
                for src0, dst0, pl in ((0, 0, 0), (32, 0, 1),
                                       (64, 32, 0), (96, 32, 1)):
                    nc.gpsimd.dma_start(
                        out=K8p3[dst0:dst0 + 32, pl:pl + 1, ts],
                        in_=K_sb[src0:src0 + 32, ts])
                for src0, pl in ((0, 0), (32, 1)):
                    nc.gpsimd.dma_start(
                        out=Q8p3[0:32, pl:pl + 1, ts],
                        in_=Q_sb[src0:src0 + 32, ts])

            def u_cast_qb():
                pk = st_["pk"]
                nc.vector.tensor_scalar_add(
                    Q_sb[64:128, pb * QT:(pb + 1) * QT],
                    pk[64:128, QT:2 * QT], b_sb[64:128, 2:3])

            def u_regroup_qb():
                pbs = slice(pb * QT, (pb + 1) * QT)
                for src0, pl in ((64, 0), (96, 1)):
                    nc.gpsimd.dma_start(
                        out=Q8p3[32:64, pl:pl + 1, pbs],
                        in_=Q_sb[src0:src0 + 32, pbs])

            def u_v(sub):
                if sub == 0:
                    st_["pv"] = scp.tile([128, grp * QT], F32, tag="sc",
                                         name="pv")
                pv = st_["pv"]
                kc = tt * 4 + sub
                for j in range(CCH):
                    nc.tensor.matmul(
                        pv[:, sub * 128:(sub + 1) * 128],
                        xT_sb[j][:, kc * KC:(kc + 1) * KC],
                        w_sb[:, j * 384 + 192:j * 384 + 320],
                        start=(j == 0), stop=(j == CCH - 1), tile_position=(0, 0),
                    )

            def u_vcopy():
                pv = st_["pv"]
                for sub in range(4):
                    kc = tt * 4 + sub
                    blk = V2[:, kc * VST:kc * VST + 193]
                    out_ap = bass.AP(tensor=blk.tensor, offset=blk.offset,
                                     ap=[list(blk.ap[0]), [129, 2], [1, 64]])
                    nc.vector.tensor_copy(out_ap, pv[:, sub * 128:(sub + 1) * 128])

            for j in range(CCH):
                units.append(lambda j=j: u_kq(j))
            units.append(u_cast_kqa)
            if fp8:
                units.append(u_regroup_kqa)
            if odd:
                for j in range(CCH):
                    units.append(lambda j=j: u_qb(j))
                units.append(u_cast_qb)
                if fp8:
                    units.append(u_regroup_qb)
            for sub in range(4):
                units.append(lambda sub=sub: u_v(sub))
            units.append(u_vcopy)
            return units

        def job_units(slot, i):
            """slot 'A': full head, query tile i (natural); slot 'B': split
            head, packed tile i.  Returns list of thunks (one per score
            group + a normalize tail)."""
            if slot == "A":
                nst, band0 = 4 * (i + 1), 4 * i
                krow, tp = 0, (0, 0)
            else:
                nst, band0 = 8 * (i + 1), 8 * i
                krow, tp = 64, (64, 0)
            qrow = slice(krow, krow + 64)
            st_ = {}

            def flush_av(force=False):
                # masks + AV for a group exp'd earlier; the delay keeps PE
                # from stalling on the exp latency
                import os as _os2
                depth = int(_os2.environ.get("KAVD", "8"))
                pend = st_.setdefault("pendq", [])
                if not pend or (not force and len(pend) <= depth - 1):
                    return
                pt, g0, w = pend.pop(0)
                av = st_["av"]
                for gi in range(w):
                    kc = g0 + gi
                    ptj = pt[:, gi * QT:(gi + 1) * QT]
                    if kc >= band0:
                        mi = (kc - band0) if slot == "A" else 4 + (kc - band0)
                        nc.vector.tensor_mul(
                            ptj, ptj, mask_sb[:, mi * QT:(mi + 1) * QT])
                    st, sp = kc == 0, kc == nst - 1
                    if slot == "A":
                        nc.tensor.matmul(
                            av[0:65, :], V2[:, kc * VST:kc * VST + 65], ptj,
                            start=st, stop=sp, tile_position=(0, 0),
                        )
                    else:
                        nc.tensor.matmul(
                            av[0:128, :], V2[:, kc * VST + 65:kc * VST + 193],
                            ptj, start=st, stop=sp, tile_position=(0, 0),
                        )

            def u_group(g0):
                if g0 == 0:
                    st_["av"] = avp.tile([128, QT], F32, tag="av", name="av")
                w = min(grp, nst - g0)
                sc = scp.tile([128, grp * QT], F32, tag="sc", name="sc")
                if fp8:
                    prow = 0 if slot == "A" else 32
                    q8 = Q8p3[prow:prow + 32, :, i * QT:(i + 1) * QT]
                    for gi in range(w):
                        kc = g0 + gi
                        nc.tensor.matmul(
                            sc[:, gi * QT:(gi + 1) * QT],
                            K8p3[prow:prow + 32, :, kc * KC:(kc + 1) * KC],
                            q8, start=True, stop=True, perf_mode=DR,
                            tile_position=(prow, 0),
                        )
                else:
                    qap = Q_sb[qrow, i * QT:(i + 1) * QT]
                    for gi in range(w):
                        kc = g0 + gi
                        nc.tensor.matmul(
                            sc[:, gi * QT:(gi + 1) * QT],
                            K_sb[krow:krow + 64, kc * KC:(kc + 1) * KC],
                            qap, start=True, stop=True, tile_position=tp,
                        )
                pt = ptp.tile([128, grp * QT], BF16, tag="pt", name="pt")
                nc.scalar.activation(
                    pt[:, 0:w * QT], sc[:, 0:w * QT], AF.Exp,
                    scale=1.0 / math.sqrt(HEAD_DIM))
                st_.setdefault("pendq", []).append((pt, g0, w))
                flush_av()

            def u_norm():
                while st_.get("pendq"):
                    flush_av(force=True)
                # normalize: bf16 reciprocal of the denom row, broadcast
                # across 64 partitions via a K=1 ones-matmul on PE (no DMA)
                av = st_["av"]
                drow = 64 if slot == "A" else 32
                rows = slice(0, 64) if slot == "A" else slice(64, 128)
                r = rp.tile([128, QT], BF16, tag="r", name="r")
                with nc.allow_low_precision(reason="softmax denom recip bf16"):
                    nc.vector.reciprocal(r[drow:drow + 1, :], av[drow:drow + 1, :])
                rbc_ps = avp.tile([128, QT], F32, tag="av", name="rbc_ps")
                nc.tensor.matmul(
                    rbc_ps[rows, :], ones64[drow:drow + 1, :], r[drow:drow + 1, :],
                    start=True, stop=True, tile_position=(drow, rows.start),
                )
                rbc = rp.tile([128, QT], F32, tag="rbc", name="rbc")
                nc.vector.tensor_copy(rbc[rows, :], rbc_ps[rows, :])
                if slot == "A":
                    nc.vector.tensor_mul(
                        ho[0:64, i * QT:(i + 1) * QT], av[0:64, :], rbc[0:64, :])
                else:
                    # write packed value j to BOTH natural columns 2j, 2j+1;
                    # the per-core parity column mask (data) zeroes the
                    # wrong one right before the out-projection.
                    hob = ho[64:128, 1024 * i:1024 * (i + 1)].rearrange(
                        "p (n two) -> p two n", two=2)
                    nc.vector.tensor_mul(hob[:, 0:1, :], av[64:128, :],
                                         rbc[64:128, :])
                    nc.vector.tensor_mul(hob[:, 1:2, :], av[64:128, :],
                                         rbc[64:128, :])

            units = [lambda g0=g0: u_group(g0) for g0 in range(0, nst, grp)]
            units.append(u_norm)
            return units

        def outproj_units(qt):
            qs = slice(qt * QT, (qt + 1) * QT)

            def u_pmask():
                nc.vector.tensor_mul(
                    ho[64:128, qs], ho[64:128, qs],
                    mask_sb[64:128, 12 * QT:13 * QT])

            def u_op(dch):
                op = opp.tile([128, QT], F32, tag="av" if opp is avp else "op", name="op")
                nc.tensor.matmul(
                    op[:], wo_sb[:, dch * 128:(dch + 1) * 128],
                    ho[:, qs], start=True, stop=True, tile_position=(0, 0),
                )
                ot = osb.tile([128, QT], BF16, tag="ot", name="ot")
                if _os.environ.get("KOPC", "0") == "1" and dch % 2 == 1:
                    nc.scalar.copy(ot[:], op[:])
                else:
                    nc.vector.tensor_copy(ot[:], op[:])
                nc.sync.dma_start(
                    out=outT_d[dch * 128:(dch + 1) * 128, qs], in_=ot[:])

            return [u_pmask] + [lambda d=d: u_op(d) for d in range(CCH)]

        def weave(main, filler):
            """Emit `main` units with `filler` units distributed evenly."""
            if not main:
                for f in filler:
                    f()
                return
            nf, nm = len(filler), len(main)
            fi = 0
            for k, u in enumerate(main):
                u()
                while fi * nm < (k + 1) * nf:
                    filler[fi]()
                    fi += 1
            while fi < nf:
                filler[fi]()
                fi += 1

        # ---- software-pipelined emission: proj(tt+1) + deferred input
        # loads woven into jobs(tt) --
        for u in proj_units(0):
            u()
        for tt in range(NQT):
            stream = []
            if tt % 2 == 1:
                stream += job_units("B", (tt - 1) // 2)
                stream += outproj_units(tt - 1)
            stream += job_units("A", tt)
            if tt % 2 == 1:
                stream += outproj_units(tt)
            filler = []
            if tt in (0, 2, 4):
                k = tt // 2 + 1
                filler.append(lambda k=k: load_xt(k))
                filler.append(lambda k=k: load_xtb(k))
            filler += proj_units(tt + 1) if tt + 1 < NQT else []
            weave(stream, filler)
    nc.finalize()
    return nc


def _host_inputs(x, wq, bq, wk, bk, wv, bv, wo):
    """Per-core input maps. Slot A of core c = head c; slot B = split head
    8 + c//2 with token parity c%2."""
    bf16 = ml_dtypes.bfloat16
    xT = np.ascontiguousarray(x[0].T).astype(bf16)
    xTB_by_par = [np.ascontiguousarray(x[0][p::2].T).astype(bf16) for p in (0, 1)]

    in_maps = []
    for c in range(N_CORES):
        ha, hb, par = c, 8 + c // 2, c % 2
        w = np.zeros((D_MODEL, 384), np.float32)
        w[:, 0:64] = wq[ha]
        w[:, 64:128] = wk[ha]
        w[:, 128:192] = wk[hb]
        w[:, 192:256] = wv[ha]
        w[:, 256:320] = wv[hb]
        w[:, 320:384] = wq[hb]
        b = np.zeros((128, 3), np.float32)
        b[0:64, 0] = bq[ha]
        b[0:64, 1] = bk[ha]
        b[64:128, 1] = bk[hb]
        b[64:128, 2] = bq[hb]
        wo2 = np.zeros((128, D_MODEL), np.float32)
        wo2[0:64] = wo[ha * 64:(ha + 1) * 64]
        wo2[64:128] = wo[hb * 64:(hb + 1) * 64]
        kl = np.arange(128)[:, None]
        qq = np.arange(QT)[None, :]
        masks = np.zeros((NMASK, 128, QT), np.float32)
        for pat in range(4):
            masks[pat] = (128 * pat + kl) <= qq
        for pat in range(8):
            masks[4 + pat] = (128 * pat + kl) <= (2 * qq + par)
        masks[12, :, :] = (qq % 2 == par)
        in_maps.append({
            "xT": xT,
            "xTB": xTB_by_par[par],
            "wproj": w.astype(bf16),
            "bqk": b.astype(np.float32),
            "wo2": wo2.astype(bf16),
            "masks": masks.astype(bf16),
        })
    return in_maps


def kernel(_trace=False, _tmpdir=None, **inputs):
    x = np.asarray(inputs["x"], np.float32)
    wq = np.asarray(inputs["wq"], np.float32)
    bq = np.asarray(inputs["bq"], np.float32)
    wk = np.asarray(inputs["wk"], np.float32)
    bk = np.asarray(inputs["bk"], np.float32)
    wv = np.asarray(inputs["wv"], np.float32)
    bv = np.asarray(inputs["bv"], np.float32)
    wo = np.asarray(inputs["wo"], np.float32)
    bo = np.asarray(inputs["bo"], np.float32)

    if "nc" not in _PROGRAM_CACHE:
        _PROGRAM_CACHE["nc"] = build_program()
    nc = _PROGRAM_CACHE["nc"]

    in_maps = _host_inputs(x, wq, bq, wk, bk, wv, bv, wo)
    res = run_bass_kernel_spmd(
        nc, in_maps, list(range(N_CORES)), trace=_trace, tmpdir=_tmpdir,
    )
    acc = np.zeros((D_MODEL, T), np.float32)
    for c in range(N_CORES):
        acc += res.results[c]["outT"]
    # V-bias folds to a constant through softmax: + bv_cat @ wo (+ bo)
    const = bv.reshape(-1) @ wo + bo
    out = acc.T + const[None, :]
    if _trace:
        return out[None].astype(np.float32), res
    return out[None].astype(np.float32)


# revision 61
# speedup vs baseline: 1.1914x; 1.0167x over previous
"""Multi-head causal attention (B=1, T=4096, D=768, H=12) on 8 trn2 cores.

Sharding: per core, slot A = one full head (heads 0-7 across the 8 cores);
slot B = half of a split head (heads 8-11, each split across 2 cores by
token PARITY: core 2k gets even tokens of head 8+k, core 2k+1 odd tokens).
Parity-splitting keeps the causal key extents identical across cores, so
every core runs the IDENTICAL program (SPMD); cores differ only in data
(weights, masks, parity).  Slot B's queries are packed (parity-strided
projection); its head outputs are written back to natural token columns
with stride-2 DVE writes, so one merged out-projection covers both slots
and the host just sums the 8 partial [768, 4096] outputs.

Per-core work: slot A = 144 key-chunk units, slot B = 80 units (vs 288 for
the old 2-full-slot scheme).  V-bias is folded into a host-side constant
(P@(V+1 bv^T) = P@V + denom bv^T, exact through softmax normalization).

On-device layout (per core):
  xT    [768, 4096] bf16   x transposed (host supplies); xTB = parity cols
  K_sb  [128, 4096] bf16   rows 0:64 head-A K dims, 64:128 head-B K dims
  Q_sb  [128, 4096] bf16   rows 0:64 head-A Q (natural); rows 64:128 cols
                           0:2048 head-B Q (parity-packed)
  V2    [128, 32*208] bf16 per key-chunk: [V_A 0:64 |1@64| 0 |1@97| 0 |
                           V_B 129:193] -> one matmul per slot yields AV
                           rows + a denominator row (A: row 64, B: row 32)
  scores chunks [128 keys, 512 q] in PSUM, exp'd on ACT -> PT bf16

Scheduling: emission is software-pipelined — projection of token tile
tt+1 and deferred xT loads are woven between the attention score groups
of stage tt, and each group's mask+AV matmuls are delayed 8 groups behind
its exp so PE never stalls on the exp latency.  Softmax normalization
broadcasts 1/denom across partitions with a K=1 ones-matmul on PE (no
DRAM bounce).  Out-projection per 512-query tile -> bf16 partials.
"""

import math
import numpy as np
import ml_dtypes
from contextlib import ExitStack

import concourse.bass as bass
import concourse.bacc as bacc
import concourse.mybir as mybir
import concourse.tile as tile
from concourse.bass_utils import run_bass_kernel_spmd

BF16 = mybir.dt.bfloat16
F32 = mybir.dt.float32
F8 = mybir.dt.float8e4
DR = mybir.MatmulPerfMode.DoubleRow
AF = mybir.ActivationFunctionType

T = 4096
D_MODEL = 768
HEAD_DIM = 64
N_HEADS = 12
N_CORES = 8
QT = 512                  # query tile width (A natural / B packed)
KC = 128                  # key chunk (psum partition dim)
GRP = 2                   # score chunks per exp group (psum banks)
NQT = T // QT             # 8 A-tiles
NPB = 4                   # B packed tiles (each covers 1024 natural tokens)
CCH = D_MODEL // 128      # 6 contraction chunks
VST = 208                 # V2 stride per key chunk
NMASK = 13                # 4 A diag patterns + 8 B patterns + parity col mask

_PROGRAM_CACHE = {}


def build_program():
    nc = bacc.Bacc(None)

    xT_d = nc.declare_dram_parameter("xT", [D_MODEL, T], BF16, isOutput=False)
    # x columns of this core's parity, packed: x[:, parity::2].T
    xTB_d = nc.declare_dram_parameter("xTB", [D_MODEL, T // 2], BF16, isOutput=False)
    # w cols: 0:64 wq_A | 64:192 wk_AB | 192:320 wv_AB | 320:384 wq_B
    w_d = nc.declare_dram_parameter("wproj", [D_MODEL, 384], BF16, isOutput=False)
    b_d = nc.declare_dram_parameter("bqk", [128, 3], F32, isOutput=False)
    wo_d = nc.declare_dram_parameter("wo2", [128, D_MODEL], BF16, isOutput=False)
    mk_d = nc.declare_dram_parameter("masks", [NMASK, 128, QT], BF16, isOutput=False)
    outT_d = nc.declare_dram_parameter("outT", [D_MODEL, T], BF16, isOutput=True)

    with tile.TileContext(nc) as tc, ExitStack() as ctx:
        consts = ctx.enter_context(tc.tile_pool(name="consts", bufs=1))
        big = ctx.enter_context(tc.tile_pool(name="big", bufs=1))
        ptp = ctx.enter_context(tc.tile_pool(name="ptp", bufs=int(__import__("os").environ.get("KPTP", "9"))))
        osb = ctx.enter_context(tc.tile_pool(name="osb", bufs=3))
        rp = ctx.enter_context(tc.tile_pool(name="rp", bufs=2))
        dramp = ctx.enter_context(tc.tile_pool(name="dramp", bufs=2, space="DRAM"))
        # PSUM budget is 8 banks total; knobs for the split
        import os as _os
        fp8 = _os.environ.get("KFP8", "0") == "1"
        grp = int(_os.environ.get("KGRP", str(GRP)))
        _scb = int(_os.environ.get("KSCB", "2"))
        _avp = int(_os.environ.get("KAVP", "3"))
        _opp = int(_os.environ.get("KOPP", "1"))
        scp = ctx.enter_context(tc.tile_pool(name="scp", bufs=_scb, space="PSUM"))
        avp = ctx.enter_context(tc.tile_pool(name="avp", bufs=_avp, space="PSUM"))
        if _opp > 0:
            opp = ctx.enter_context(
                tc.tile_pool(name="opp", bufs=_opp, space="PSUM"))
        else:
            opp = avp

        # ---- inputs to SBUF: small consts first, then xT streamed in
        # token-tile slices so stage-0 projection starts within ~4us ----
        w_sb = consts.tile([128, CCH * 384], BF16, tag="w")
        _wap = w_d[:, :]
        nc.sync.dma_start(
            out=w_sb[:],
            in_=bass.AP(tensor=_wap.tensor, offset=_wap.offset,
                        ap=[[384, 128], [128 * 384, CCH], [1, 384]]))
        b_sb = consts.tile([128, 3], F32, tag="b")
        nc.sync.dma_start(out=b_sb[:], in_=b_d[:, :])
        wo_sb = consts.tile([128, D_MODEL], BF16, tag="wo")
        nc.sync.dma_start(out=wo_sb[:], in_=wo_d[:, :])
        mask_sb = consts.tile([128, NMASK * QT], BF16, tag="mask")
        # xT sliced per token-tile pair so stage-0/1 projection starts early;
        # first xTB slice interleaved (stage 1 needs it)
        xT_sb = [big.tile([128, T], BF16, tag=f"xT{j}", name=f"xT{j}")
                 for j in range(CCH)]
        xTB_sb = [big.tile([128, T // 2], BF16, tag=f"xTB{j}", name=f"xTB{j}")
                  for j in range(CCH)]
        def load_xt(tp):
            cs = slice(tp * 2 * QT, (tp + 1) * 2 * QT)
            for j in range(CCH):
                nc.sync.dma_start(out=xT_sb[j][:, cs],
                                  in_=xT_d[j * 128:(j + 1) * 128, cs])

        def load_xtb(pb):
            cs = slice(pb * QT, (pb + 1) * QT)
            for j in range(CCH):
                nc.sync.dma_start(out=xTB_sb[j][:, cs],
                                  in_=xTB_d[j * 128:(j + 1) * 128, cs])

        def load_xt1(tt):
            cs = slice(tt * QT, (tt + 1) * QT)
            for j in range(CCH):
                nc.sync.dma_start(out=xT_sb[j][:, cs],
                                  in_=xT_d[j * 128:(j + 1) * 128, cs])

        # only the slices stage 0/1 need right away; masks deferred behind
        # them; the rest are woven in as filler
        load_xt1(0)
        load_xt1(1)
        load_xtb(0)
        _map = mk_d[:, :, :]
        nc.sync.dma_start(
            out=mask_sb[:],
            in_=bass.AP(tensor=_map.tensor, offset=_map.offset,
                        ap=[[QT, 128], [128 * QT, NMASK], [1, QT]]))

        # ---- persistent tensors ----
        QKDT = F8 if fp8 else BF16
        K_sb = big.tile([128, T], QKDT, tag="K")
        Q_sb = big.tile([128, T], QKDT, tag="Q")
        if fp8:
            # plane layout for DoubleRow: rows 0:32 slot A (head-dim planes
            # 0:32 / 32:64 at byte offsets 0 / T), rows 32:64 slot B
            K8p = big.tile([128, 2 * T], F8, tag="K8p")
            Q8p = big.tile([128, 2 * T], F8, tag="Q8p")
            K8p3 = K8p[:].rearrange("p (two n) -> p two n", two=2)
            Q8p3 = Q8p[:].rearrange("p (two n) -> p two n", two=2)
        V2 = big.tile([128, (T // KC) * VST], BF16, tag="V2")
        ho = big.tile([128, T], BF16, tag="ho")
        nc.gpsimd.memset(ho[:], 0.0)
        nc.gpsimd.memset(V2[:], 0.0)
        v3 = V2[:].rearrange("p (t c) -> p t c", c=VST)
        nc.gpsimd.memset(v3[:, :, 64:65], 1.0)    # ones row for denom_A
        nc.gpsimd.memset(v3[:, :, 97:98], 1.0)    # ones row for denom_B
        ones64 = consts.tile([128, 64], BF16, tag="ones64")
        nc.gpsimd.memset(ones64[:], 1.0)          # lhsT for recip broadcast

        def proj_units(tt):
            """List of thunks emitting projection for token tile tt."""
            ts = slice(tt * QT, (tt + 1) * QT)
            odd = tt % 2 == 1
            pb = (tt - 1) // 2
            st_ = {}
            units = []

            # NOTE: only ONE pending psum accumulation group per 2KB bank —
            # K (bank0) + Q_A (bank1) may interleave, but Q_B (also bank1)
            # and each V sub-group (all in pv bank0) must run after the
            # previous same-bank group has stopped.
            def u_kq(j):
                if j == 0:
                    st_["pk"] = scp.tile([128, grp * QT], F32, tag="sc",
                                         name="pk")
                pk = st_["pk"]
                rhs = xT_sb[j][:, ts]
                st, sp = j == 0, j == CCH - 1
                nc.tensor.matmul(
                    pk[:, 0:QT], w_sb[:, j * 384 + 64:j * 384 + 192], rhs,
                    start=st, stop=sp, tile_position=(0, 0),
                )
                nc.tensor.matmul(
                    pk[0:64, QT:2 * QT], w_sb[:, j * 384:j * 384 + 64], rhs,
                    start=st, stop=sp, tile_position=(0, 0),
                )

            def u_qb(j):
                pk = st_["pk"]
                nc.tensor.matmul(
                    pk[64:128, QT:2 * QT],
                    w_sb[:, j * 384 + 320:j * 384 + 384],
                    xTB_sb[j][:, pb * QT:(pb + 1) * QT],
                    start=(j == 0), stop=(j == CCH - 1), tile_position=(0, 64),
                )

            def u_cast_kqa():
                pk = st_["pk"]
                nc.vector.tensor_scalar_add(K_sb[:, ts], pk[:, 0:QT], b_sb[:, 1:2])
                nc.vector.tensor_scalar_add(
                    Q_sb[0:64, ts], pk[0:64, QT:2 * QT], b_sb[0:64, 0:1])

            def u_regroup_kqa():
                # fp8 plane regroup: flat rows (4 groups of 32) -> plane
                # layout rows 0:32 (A) / 32:64 (B), byte offset 0 / T.
                # SWDGE (gpsimd) path keeps these off the busy HWDGE.
                for src0, dst0, pl in ((0, 0, 0), (32, 0, 1),
                                       (64, 32, 0), (96, 32, 1)):
                    nc.gpsimd.dma_start(
                        out=K8p3[dst0:dst0 + 32, pl:pl + 1, ts],
                        in_=K_sb[src0:src0 + 32, ts])
                for src0, pl in ((0, 0), (32, 1)):
                    nc.gpsimd.dma_start(
                        out=Q8p3[0:32, pl:pl + 1, ts],
                        in_=Q_sb[src0:src0 + 32, ts])

            def u_cast_qb():
                pk = st_["pk"]
                nc.vector.tensor_scalar_add(
                    Q_sb[64:128, pb * QT:(pb + 1) * QT],
                    pk[64:128, QT:2 * QT], b_sb[64:128, 2:3])

            def u_regroup_qb():
                pbs = slice(pb * QT, (pb + 1) * QT)
                for src0, pl in ((64, 0), (96, 1)):
                    nc.gpsimd.dma_start(
                        out=Q8p3[32:64, pl:pl + 1, pbs],
                        in_=Q_sb[src0:src0 + 32, pbs])

            def u_v(sub):
                if sub == 0:
                    st_["pv"] = scp.tile([128, grp * QT], F32, tag="sc",
                                         name="pv")
                pv = st_["pv"]
                kc = tt * 4 + sub
                for j in range(CCH):
                    nc.tensor.matmul(
                        pv[:, sub * 128:(sub + 1) * 128],
                        xT_sb[j][:, kc * KC:(kc + 1) * KC],
                        w_sb[:, j * 384 + 192:j * 384 + 320],
                        start=(j == 0), stop=(j == CCH - 1), tile_position=(0, 0),
                    )

            def u_vcopy():
                pv = st_["pv"]
                for sub in range(4):
                    kc = tt * 4 + sub
                    blk = V2[:, kc * VST:kc * VST + 193]
                    out_ap = bass.AP(tensor=blk.tensor, offset=blk.offset,
                                     ap=[list(blk.ap[0]), [129, 2], [1, 64]])
                    nc.vector.tensor_copy(out_ap, pv[:, sub * 128:(sub + 1) * 128])

            for j in range(CCH):
                units.append(lambda j=j: u_kq(j))
            units.append(u_cast_kqa)
            if fp8:
                units.append(u_regroup_kqa)
            if odd:
                for j in range(CCH):
                    units.append(lambda j=j: u_qb(j))
                units.append(u_cast_qb)
                if fp8:
                    units.append(u_regroup_qb)
            for sub in range(4):
                units.append(lambda sub=sub: u_v(sub))
            units.append(u_vcopy)
            return units

        def job_units(slot, i, avd=None):
            """slot 'A': full head, query tile i (natural); slot 'B': split
            head, packed tile i.  Returns list of thunks (one per score
            group + a normalize tail)."""
            if slot == "A":
                nst, band0 = 4 * (i + 1), 4 * i
                krow, tp = 0, (0, 0)
            else:
                nst, band0 = 8 * (i + 1), 8 * i
                krow, tp = 64, (64, 0)
            qrow = slice(krow, krow + 64)
            st_ = {}

            def flush_av(force=False):
                # masks + AV for a group exp'd earlier; the delay keeps PE
                # from stalling on the exp latency
                import os as _os2
                depth = avd if avd is not None else int(_os2.environ.get("KAVD", "8"))
                pend = st_.setdefault("pendq", [])
                if not pend or (not force and len(pend) <= depth - 1):
                    return
                pt, g0, w = pend.pop(0)
                av = st_["av"]
                for gi in range(w):
                    kc = g0 + gi
                    # valid-query truncation: for diagonal-band chunks,
                    # queries below qoff are entirely masked-out, so the
                    # mask mul and AV matmul (incl. denom row) skip them —
                    # exact, since those queries don't attend these keys.
                    qoff = 0
                    if kc >= band0:
                        pat = kc - band0
                        mi = pat if slot == "A" else 4 + pat
                        qoff = (128 if slot == "A" else 64) * pat
                        nc.vector.tensor_mul(
                            pt[:, gi * QT + qoff:(gi + 1) * QT],
                            pt[:, gi * QT + qoff:(gi + 1) * QT],
                            mask_sb[:, mi * QT + qoff:(mi + 1) * QT])
                    ptj = pt[:, gi * QT + qoff:(gi + 1) * QT]
                    st, sp = kc == 0, kc == nst - 1
                    if slot == "A":
                        nc.tensor.matmul(
                            av[0:65, qoff:QT], V2[:, kc * VST:kc * VST + 65],
                            ptj, start=st, stop=sp, tile_position=(0, 0),
                        )
                    else:
                        nc.tensor.matmul(
                            av[0:128, qoff:QT],
                            V2[:, kc * VST + 65:kc * VST + 193],
                            ptj, start=st, stop=sp, tile_position=(0, 0),
                        )

            def u_group(g0):
                if g0 == 0:
                    st_["av"] = avp.tile([128, QT], F32, tag="av", name="av")
                w = min(grp, nst - g0)
                sc = scp.tile([128, grp * QT], F32, tag="sc", name="sc")
                if fp8:
                    prow = 0 if slot == "A" else 32
                    q8 = Q8p3[prow:prow + 32, :, i * QT:(i + 1) * QT]
                    for gi in range(w):
                        kc = g0 + gi
                        nc.tensor.matmul(
                            sc[:, gi * QT:(gi + 1) * QT],
                            K8p3[prow:prow + 32, :, kc * KC:(kc + 1) * KC],
                            q8, start=True, stop=True, perf_mode=DR,
                            tile_position=(prow, 0),
                        )
                else:
                    qap = Q_sb[qrow, i * QT:(i + 1) * QT]
                    for gi in range(w):
                        kc = g0 + gi
                        nc.tensor.matmul(
                            sc[:, gi * QT:(gi + 1) * QT],
                            K_sb[krow:krow + 64, kc * KC:(kc + 1) * KC],
                            qap, start=True, stop=True, tile_position=tp,
                        )
                pt = ptp.tile([128, grp * QT], BF16, tag="pt", name="pt")
                nc.scalar.activation(
                    pt[:, 0:w * QT], sc[:, 0:w * QT], AF.Exp,
                    scale=1.0 / math.sqrt(HEAD_DIM))
                st_.setdefault("pendq", []).append((pt, g0, w))
                flush_av()

            def u_norm():
                while st_.get("pendq"):
                    flush_av(force=True)
                # normalize: bf16 reciprocal of the denom row, broadcast
                # across 64 partitions via a K=1 ones-matmul on PE (no DMA)
                av = st_["av"]
                drow = 64 if slot == "A" else 32
                rows = slice(0, 64) if slot == "A" else slice(64, 128)
                r = rp.tile([128, QT], BF16, tag="r", name="r")
                with nc.allow_low_precision(reason="softmax denom recip bf16"):
                    nc.vector.reciprocal(r[drow:drow + 1, :], av[drow:drow + 1, :])
                rbc_ps = avp.tile([128, QT], F32, tag="av", name="rbc_ps")
                nc.tensor.matmul(
                    rbc_ps[rows, :], ones64[drow:drow + 1, :], r[drow:drow + 1, :],
                    start=True, stop=True, tile_position=(drow, rows.start),
                )
                rbc = rp.tile([128, QT], F32, tag="rbc", name="rbc")
                nc.vector.tensor_copy(rbc[rows, :], rbc_ps[rows, :])
                if slot == "A":
                    nc.vector.tensor_mul(
                        ho[0:64, i * QT:(i + 1) * QT], av[0:64, :], rbc[0:64, :])
                else:
                    # write packed value j to BOTH natural columns 2j, 2j+1;
                    # the per-core parity column mask (data) zeroes the
                    # wrong one right before the out-projection.
                    hob = ho[64:128, 1024 * i:1024 * (i + 1)].rearrange(
                        "p (n two) -> p two n", two=2)
                    nc.vector.tensor_mul(hob[:, 0:1, :], av[64:128, :],
                                         rbc[64:128, :])
                    nc.vector.tensor_mul(hob[:, 1:2, :], av[64:128, :],
                                         rbc[64:128, :])

            units = [lambda g0=g0: u_group(g0) for g0 in range(0, nst, grp)]
            units.append(u_norm)
            return units

        def outproj_units(qt):
            qs = slice(qt * QT, (qt + 1) * QT)

            def u_pmask():
                nc.vector.tensor_mul(
                    ho[64:128, qs], ho[64:128, qs],
                    mask_sb[64:128, 12 * QT:13 * QT])

            def u_op(dch):
                op = opp.tile([128, QT], F32, tag="av" if opp is avp else "op", name="op")
                nc.tensor.matmul(
                    op[:], wo_sb[:, dch * 128:(dch + 1) * 128],
                    ho[:, qs], start=True, stop=True, tile_position=(0, 0),
                )
                ot = osb.tile([128, QT], BF16, tag="ot", name="ot")
                if qt >= 6 or dch % 2 == 1:
                    nc.scalar.copy(ot[:], op[:])
                else:
                    nc.vector.tensor_copy(ot[:], op[:])
                nc.sync.dma_start(
                    out=outT_d[dch * 128:(dch + 1) * 128, qs], in_=ot[:])

            return [u_pmask] + [lambda d=d: u_op(d) for d in range(CCH)]

        def weave(main, filler):
            """Emit `main` units with `filler` units distributed evenly."""
            if not main:
                for f in filler:
                    f()
                return
            nf, nm = len(filler), len(main)
            fi = 0
            for k, u in enumerate(main):
                u()
                while fi * nm < (k + 1) * nf:
                    filler[fi]()
                    fi += 1
            while fi < nf:
                filler[fi]()
                fi += 1

        # ---- software-pipelined emission: proj(tt+1) + deferred input
        # loads woven into jobs(tt) --
        for u in proj_units(0):
            u()
        for tt in range(NQT):
            stream = []
            if tt % 2 == 1:
                stream += job_units("B", (tt - 1) // 2)
                stream += outproj_units(tt - 1)
            stream += job_units("A", tt, avd=2 if tt == NQT - 1 else None)
            if tt % 2 == 1:
                stream += outproj_units(tt)
            filler = []
            if tt in (0, 2, 4):
                k = tt // 2 + 1
                filler.append(lambda k=k: load_xt(k))
                filler.append(lambda k=k: load_xtb(k))
            filler += proj_units(tt + 1) if tt + 1 < NQT else []
            weave(stream, filler)
    nc.finalize()
    return nc


def _host_inputs(x, wq, bq, wk, bk, wv, bv, wo):
    """Per-core input maps. Slot A of core c = head c; slot B = split head
    8 + c//2 with token parity c%2."""
    bf16 = ml_dtypes.bfloat16
    xT = np.ascontiguousarray(x[0].T).astype(bf16)
    xTB_by_par = [np.ascontiguousarray(x[0][p::2].T).astype(bf16) for p in (0, 1)]

    in_maps = []
    for c in range(N_CORES):
        ha, hb, par = c, 8 + c // 2, c % 2
        w = np.zeros((D_MODEL, 384), np.float32)
        w[:, 0:64] = wq[ha]
        w[:, 64:128] = wk[ha]
        w[:, 128:192] = wk[hb]
        w[:, 192:256] = wv[ha]
        w[:, 256:320] = wv[hb]
        w[:, 320:384] = wq[hb]
        b = np.zeros((128, 3), np.float32)
        b[0:64, 0] = bq[ha]
        b[0:64, 1] = bk[ha]
        b[64:128, 1] = bk[hb]
        b[64:128, 2] = bq[hb]
        wo2 = np.zeros((128, D_MODEL), np.float32)
        wo2[0:64] = wo[ha * 64:(ha + 1) * 64]
        wo2[64:128] = wo[hb * 64:(hb + 1) * 64]
        kl = np.arange(128)[:, None]
        qq = np.arange(QT)[None, :]
        masks = np.zeros((NMASK, 128, QT), np.float32)
        for pat in range(4):
            masks[pat] = (128 * pat + kl) <= qq
        for pat in range(8):
            masks[4 + pat] = (128 * pat + kl) <= (2 * qq + par)
        masks[12, :, :] = (qq % 2 == par)
        in_maps.append({
            "xT": xT,
            "xTB": xTB_by_par[par],
            "wproj": w.astype(bf16),
            "bqk": b.astype(np.float32),
            "wo2": wo2.astype(bf16),
            "masks": masks.astype(bf16),
        })
    return in_maps


def kernel(_trace=False, _tmpdir=None, **inputs):
    x = np.asarray(inputs["x"], np.float32)
    wq = np.asarray(inputs["wq"], np.float32)
    bq = np.asarray(inputs["bq"], np.float32)
    wk = np.asarray(inputs["wk"], np.float32)
    bk = np.asarray(inputs["bk"], np.float32)
    wv = np.asarray(inputs["wv"], np.float32)
    bv = np.asarray(inputs["bv"], np.float32)
    wo = np.asarray(inputs["wo"], np.float32)
    bo = np.asarray(inputs["bo"], np.float32)

    if "nc" not in _PROGRAM_CACHE:
        _PROGRAM_CACHE["nc"] = build_program()
    nc = _PROGRAM_CACHE["nc"]

    in_maps = _host_inputs(x, wq, bq, wk, bk, wv, bv, wo)
    res = run_bass_kernel_spmd(
        nc, in_maps, list(range(N_CORES)), trace=_trace, tmpdir=_tmpdir,
    )
    acc = np.zeros((D_MODEL, T), np.float32)
    for c in range(N_CORES):
        acc += res.results[c]["outT"]
    # V-bias folds to a constant through softmax: + bv_cat @ wo (+ bo)
    const = bv.reshape(-1) @ wo + bo
    out = acc.T + const[None, :]
    if _trace:
        return out[None].astype(np.float32), res
    return out[None].astype(np.float32)


# revision 62
# speedup vs baseline: 1.1982x; 1.0057x over previous
"""Multi-head causal attention (B=1, T=4096, D=768, H=12) on 8 trn2 cores.

Sharding: per core, slot A = one full head (heads 0-7 across the 8 cores);
slot B = half of a split head (heads 8-11, each split across 2 cores by
token PARITY: core 2k gets even tokens of head 8+k, core 2k+1 odd tokens).
Parity-splitting keeps the causal key extents identical across cores, so
every core runs the IDENTICAL program (SPMD); cores differ only in data
(weights, masks, parity).  Slot B's queries are packed (parity-strided
projection); its head outputs are written back to natural token columns
with stride-2 DVE writes, so one merged out-projection covers both slots
and the host just sums the 8 partial [768, 4096] outputs.

Per-core work: slot A = 144 key-chunk units, slot B = 80 units (vs 288 for
the old 2-full-slot scheme).  V-bias is folded into a host-side constant
(P@(V+1 bv^T) = P@V + denom bv^T, exact through softmax normalization).

On-device layout (per core):
  xT    [768, 4096] bf16   x transposed (host supplies); xTB = parity cols
  K_sb  [128, 4096] bf16   rows 0:64 head-A K dims, 64:128 head-B K dims
  Q_sb  [128, 4096] bf16   rows 0:64 head-A Q (natural); rows 64:128 cols
                           0:2048 head-B Q (parity-packed)
  V2    [128, 32*208] bf16 per key-chunk: [V_A 0:64 |1@64| 0 |1@97| 0 |
                           V_B 129:193] -> one matmul per slot yields AV
                           rows + a denominator row (A: row 64, B: row 32)
  scores chunks [128 keys, 512 q] in PSUM, exp'd on ACT -> PT bf16

Scheduling: emission is software-pipelined — projection of token tile
tt+1 and deferred xT loads are woven between the attention score groups
of stage tt, and each group's mask+AV matmuls are delayed 8 groups behind
its exp so PE never stalls on the exp latency.  Softmax normalization
broadcasts 1/denom across partitions with a K=1 ones-matmul on PE (no
DRAM bounce).  Out-projection per 512-query tile -> bf16 partials.
"""

import math
import numpy as np
import ml_dtypes
from contextlib import ExitStack

import concourse.bass as bass
import concourse.bacc as bacc
import concourse.mybir as mybir
import concourse.tile as tile
from concourse.bass_utils import run_bass_kernel_spmd

BF16 = mybir.dt.bfloat16
F32 = mybir.dt.float32
F8 = mybir.dt.float8e4
DR = mybir.MatmulPerfMode.DoubleRow
AF = mybir.ActivationFunctionType

T = 4096
D_MODEL = 768
HEAD_DIM = 64
N_HEADS = 12
N_CORES = 8
QT = 512                  # query tile width (A natural / B packed)
KC = 128                  # key chunk (psum partition dim)
GRP = 2                   # score chunks per exp group (psum banks)
NQT = T // QT             # 8 A-tiles
NPB = 4                   # B packed tiles (each covers 1024 natural tokens)
CCH = D_MODEL // 128      # 6 contraction chunks
VST = 208                 # V2 stride per key chunk
NMASK = 13                # 4 A diag patterns + 8 B patterns + parity col mask

_PROGRAM_CACHE = {}


def build_program():
    nc = bacc.Bacc(None)

    xT_d = nc.declare_dram_parameter("xT", [D_MODEL, T], BF16, isOutput=False)
    # x columns of this core's parity, packed: x[:, parity::2].T
    xTB_d = nc.declare_dram_parameter("xTB", [D_MODEL, T // 2], BF16, isOutput=False)
    # w cols: 0:64 wq_A | 64:192 wk_AB | 192:320 wv_AB | 320:384 wq_B
    w_d = nc.declare_dram_parameter("wproj", [D_MODEL, 384], BF16, isOutput=False)
    b_d = nc.declare_dram_parameter("bqk", [128, 3], F32, isOutput=False)
    wo_d = nc.declare_dram_parameter("wo2", [128, D_MODEL], BF16, isOutput=False)
    mk_d = nc.declare_dram_parameter("masks", [NMASK, 128, QT], BF16, isOutput=False)
    outT_d = nc.declare_dram_parameter("outT", [D_MODEL, T], BF16, isOutput=True)

    with tile.TileContext(nc) as tc, ExitStack() as ctx:
        consts = ctx.enter_context(tc.tile_pool(name="consts", bufs=1))
        big = ctx.enter_context(tc.tile_pool(name="big", bufs=1))
        ptp = ctx.enter_context(tc.tile_pool(name="ptp", bufs=int(__import__("os").environ.get("KPTP", "9"))))
        osb = ctx.enter_context(tc.tile_pool(name="osb", bufs=3))
        rp = ctx.enter_context(tc.tile_pool(name="rp", bufs=2))
        dramp = ctx.enter_context(tc.tile_pool(name="dramp", bufs=2, space="DRAM"))
        # PSUM budget is 8 banks total; knobs for the split
        import os as _os
        fp8 = _os.environ.get("KFP8", "0") == "1"
        grp = int(_os.environ.get("KGRP", str(GRP)))
        _scb = int(_os.environ.get("KSCB", "2"))
        _avp = int(_os.environ.get("KAVP", "2"))
        _opp = int(_os.environ.get("KOPP", "2"))
        scp = ctx.enter_context(tc.tile_pool(name="scp", bufs=_scb, space="PSUM"))
        avp = ctx.enter_context(tc.tile_pool(name="avp", bufs=_avp, space="PSUM"))
        if _opp > 0:
            opp = ctx.enter_context(
                tc.tile_pool(name="opp", bufs=_opp, space="PSUM"))
        else:
            opp = avp

        # ---- inputs to SBUF: small consts first, then xT streamed in
        # token-tile slices so stage-0 projection starts within ~4us ----
        w_sb = consts.tile([128, CCH * 384], BF16, tag="w")
        _wap = w_d[:, :]
        nc.sync.dma_start(
            out=w_sb[:],
            in_=bass.AP(tensor=_wap.tensor, offset=_wap.offset,
                        ap=[[384, 128], [128 * 384, CCH], [1, 384]]))
        b_sb = consts.tile([128, 3], F32, tag="b")
        nc.sync.dma_start(out=b_sb[:], in_=b_d[:, :])
        wo_sb = consts.tile([128, D_MODEL], BF16, tag="wo")
        nc.sync.dma_start(out=wo_sb[:], in_=wo_d[:, :])
        mask_sb = consts.tile([128, NMASK * QT], BF16, tag="mask")
        # xT sliced per token-tile pair so stage-0/1 projection starts early;
        # first xTB slice interleaved (stage 1 needs it)
        xT_sb = [big.tile([128, T], BF16, tag=f"xT{j}", name=f"xT{j}")
                 for j in range(CCH)]
        xTB_sb = [big.tile([128, T // 2], BF16, tag=f"xTB{j}", name=f"xTB{j}")
                  for j in range(CCH)]
        def load_xt(tp):
            cs = slice(tp * 2 * QT, (tp + 1) * 2 * QT)
            for j in range(CCH):
                nc.sync.dma_start(out=xT_sb[j][:, cs],
                                  in_=xT_d[j * 128:(j + 1) * 128, cs])

        def load_xtb(pb):
            cs = slice(pb * QT, (pb + 1) * QT)
            for j in range(CCH):
                nc.sync.dma_start(out=xTB_sb[j][:, cs],
                                  in_=xTB_d[j * 128:(j + 1) * 128, cs])

        def load_xt1(tt):
            cs = slice(tt * QT, (tt + 1) * QT)
            for j in range(CCH):
                nc.sync.dma_start(out=xT_sb[j][:, cs],
                                  in_=xT_d[j * 128:(j + 1) * 128, cs])

        # only the slices stage 0/1 need right away; masks deferred behind
        # them; the rest are woven in as filler
        load_xt1(0)
        load_xt1(1)
        load_xtb(0)
        _map = mk_d[:, :, :]
        nc.sync.dma_start(
            out=mask_sb[:],
            in_=bass.AP(tensor=_map.tensor, offset=_map.offset,
                        ap=[[QT, 128], [128 * QT, NMASK], [1, QT]]))

        # ---- persistent tensors ----
        QKDT = F8 if fp8 else BF16
        K_sb = big.tile([128, T], QKDT, tag="K")
        Q_sb = big.tile([128, T], QKDT, tag="Q")
        if fp8:
            # plane layout for DoubleRow: rows 0:32 slot A (head-dim planes
            # 0:32 / 32:64 at byte offsets 0 / T), rows 32:64 slot B
            K8p = big.tile([128, 2 * T], F8, tag="K8p")
            Q8p = big.tile([128, 2 * T], F8, tag="Q8p")
            K8p3 = K8p[:].rearrange("p (two n) -> p two n", two=2)
            Q8p3 = Q8p[:].rearrange("p (two n) -> p two n", two=2)
        V2 = big.tile([128, (T // KC) * VST], BF16, tag="V2")
        ho = big.tile([128, T], BF16, tag="ho")
        nc.gpsimd.memset(ho[:], 0.0)
        nc.gpsimd.memset(V2[:], 0.0)
        v3 = V2[:].rearrange("p (t c) -> p t c", c=VST)
        nc.gpsimd.memset(v3[:, :, 64:65], 1.0)    # ones row for denom_A
        nc.gpsimd.memset(v3[:, :, 97:98], 1.0)    # ones row for denom_B
        ones64 = consts.tile([128, 64], BF16, tag="ones64")
        nc.gpsimd.memset(ones64[:], 1.0)          # lhsT for recip broadcast

        def proj_units(tt):
            """List of thunks emitting projection for token tile tt."""
            ts = slice(tt * QT, (tt + 1) * QT)
            odd = tt % 2 == 1
            pb = (tt - 1) // 2
            st_ = {}
            units = []

            # NOTE: only ONE pending psum accumulation group per 2KB bank —
            # K (bank0) + Q_A (bank1) may interleave, but Q_B (also bank1)
            # and each V sub-group (all in pv bank0) must run after the
            # previous same-bank group has stopped.
            def u_kq(j):
                if j == 0:
                    st_["pk"] = scp.tile([128, grp * QT], F32, tag="sc",
                                         name="pk")
                pk = st_["pk"]
                rhs = xT_sb[j][:, ts]
                st, sp = j == 0, j == CCH - 1
                nc.tensor.matmul(
                    pk[:, 0:QT], w_sb[:, j * 384 + 64:j * 384 + 192], rhs,
                    start=st, stop=sp, tile_position=(0, 0),
                )
                nc.tensor.matmul(
                    pk[0:64, QT:2 * QT], w_sb[:, j * 384:j * 384 + 64], rhs,
                    start=st, stop=sp, tile_position=(0, 0),
                )

            def u_qb(j):
                pk = st_["pk"]
                nc.tensor.matmul(
                    pk[64:128, QT:2 * QT],
                    w_sb[:, j * 384 + 320:j * 384 + 384],
                    xTB_sb[j][:, pb * QT:(pb + 1) * QT],
                    start=(j == 0), stop=(j == CCH - 1), tile_position=(0, 64),
                )

            def u_cast_kqa():
                pk = st_["pk"]
                nc.vector.tensor_scalar_add(K_sb[:, ts], pk[:, 0:QT], b_sb[:, 1:2])
                nc.vector.tensor_scalar_add(
                    Q_sb[0:64, ts], pk[0:64, QT:2 * QT], b_sb[0:64, 0:1])

            def u_regroup_kqa():
                # fp8 plane regroup: flat rows (4 groups of 32) -> plane
                # layout rows 0:32 (A) / 32:64 (B), byte offset 0 / T.
                # SWDGE (gpsimd) path keeps these off the busy HWDGE.
                for src0, dst0, pl in ((0, 0, 0), (32, 0, 1),
                                       (64, 32, 0), (96, 32, 1)):
                    nc.gpsimd.dma_start(
                        out=K8p3[dst0:dst0 + 32, pl:pl + 1, ts],
                        in_=K_sb[src0:src0 + 32, ts])
                for src0, pl in ((0, 0), (32, 1)):
                    nc.gpsimd.dma_start(
                        out=Q8p3[0:32, pl:pl + 1, ts],
                        in_=Q_sb[src0:src0 + 32, ts])

            def u_cast_qb():
                pk = st_["pk"]
                nc.vector.tensor_scalar_add(
                    Q_sb[64:128, pb * QT:(pb + 1) * QT],
                    pk[64:128, QT:2 * QT], b_sb[64:128, 2:3])

            def u_regroup_qb():
                pbs = slice(pb * QT, (pb + 1) * QT)
                for src0, pl in ((64, 0), (96, 1)):
                    nc.gpsimd.dma_start(
                        out=Q8p3[32:64, pl:pl + 1, pbs],
                        in_=Q_sb[src0:src0 + 32, pbs])

            def u_v(sub):
                if sub == 0:
                    st_["pv"] = scp.tile([128, grp * QT], F32, tag="sc",
                                         name="pv")
                pv = st_["pv"]
                kc = tt * 4 + sub
                for j in range(CCH):
                    nc.tensor.matmul(
                        pv[:, sub * 128:(sub + 1) * 128],
                        xT_sb[j][:, kc * KC:(kc + 1) * KC],
                        w_sb[:, j * 384 + 192:j * 384 + 320],
                        start=(j == 0), stop=(j == CCH - 1), tile_position=(0, 0),
                    )

            def u_vcopy():
                pv = st_["pv"]
                for sub in range(4):
                    kc = tt * 4 + sub
                    blk = V2[:, kc * VST:kc * VST + 193]
                    out_ap = bass.AP(tensor=blk.tensor, offset=blk.offset,
                                     ap=[list(blk.ap[0]), [129, 2], [1, 64]])
                    nc.vector.tensor_copy(out_ap, pv[:, sub * 128:(sub + 1) * 128])

            for j in range(CCH):
                units.append(lambda j=j: u_kq(j))
            units.append(u_cast_kqa)
            if fp8:
                units.append(u_regroup_kqa)
            if odd:
                for j in range(CCH):
                    units.append(lambda j=j: u_qb(j))
                units.append(u_cast_qb)
                if fp8:
                    units.append(u_regroup_qb)
            for sub in range(4):
                units.append(lambda sub=sub: u_v(sub))
            units.append(u_vcopy)
            return units

        def job_units(slot, i, avd=None):
            """slot 'A': full head, query tile i (natural); slot 'B': split
            head, packed tile i.  Returns list of thunks (one per score
            group + a normalize tail)."""
            if slot == "A":
                nst, band0 = 4 * (i + 1), 4 * i
                krow, tp = 0, (0, 0)
            else:
                nst, band0 = 8 * (i + 1), 8 * i
                krow, tp = 64, (64, 0)
            qrow = slice(krow, krow + 64)
            st_ = {}

            def flush_av(force=False):
                # masks + AV for a group exp'd earlier; the delay keeps PE
                # from stalling on the exp latency
                import os as _os2
                depth = avd if avd is not None else int(_os2.environ.get("KAVD", "8"))
                pend = st_.setdefault("pendq", [])
                if not pend or (not force and len(pend) <= depth - 1):
                    return
                pt, g0, w = pend.pop(0)
                av = st_["av"]
                for gi in range(w):
                    kc = g0 + gi
                    # valid-query truncation: for diagonal-band chunks,
                    # queries below qoff are entirely masked-out, so the
                    # mask mul and AV matmul (incl. denom row) skip them —
                    # exact, since those queries don't attend these keys.
                    qoff = 0
                    if kc >= band0:
                        pat = kc - band0
                        mi = pat if slot == "A" else 4 + pat
                        qoff = (128 if slot == "A" else 64) * pat
                        nc.vector.tensor_mul(
                            pt[:, gi * QT + qoff:(gi + 1) * QT],
                            pt[:, gi * QT + qoff:(gi + 1) * QT],
                            mask_sb[:, mi * QT + qoff:(mi + 1) * QT])
                    ptj = pt[:, gi * QT + qoff:(gi + 1) * QT]
                    st, sp = kc == 0, kc == nst - 1
                    if slot == "A":
                        nc.tensor.matmul(
                            av[0:65, qoff:QT], V2[:, kc * VST:kc * VST + 65],
                            ptj, start=st, stop=sp, tile_position=(0, 0),
                        )
                    else:
                        nc.tensor.matmul(
                            av[0:128, qoff:QT],
                            V2[:, kc * VST + 65:kc * VST + 193],
                            ptj, start=st, stop=sp, tile_position=(0, 0),
                        )

            def u_group(g0):
                if g0 == 0:
                    st_["av"] = avp.tile([128, QT], F32, tag="av", name="av")
                w = min(grp, nst - g0)
                sc = scp.tile([128, grp * QT], F32, tag="sc", name="sc")
                if fp8:
                    prow = 0 if slot == "A" else 32
                    q8 = Q8p3[prow:prow + 32, :, i * QT:(i + 1) * QT]
                    for gi in range(w):
                        kc = g0 + gi
                        nc.tensor.matmul(
                            sc[:, gi * QT:(gi + 1) * QT],
                            K8p3[prow:prow + 32, :, kc * KC:(kc + 1) * KC],
                            q8, start=True, stop=True, perf_mode=DR,
                            tile_position=(prow, 0),
                        )
                else:
                    qap = Q_sb[qrow, i * QT:(i + 1) * QT]
                    for gi in range(w):
                        kc = g0 + gi
                        nc.tensor.matmul(
                            sc[:, gi * QT:(gi + 1) * QT],
                            K_sb[krow:krow + 64, kc * KC:(kc + 1) * KC],
                            qap, start=True, stop=True, tile_position=tp,
                        )
                pt = ptp.tile([128, grp * QT], BF16, tag="pt", name="pt")
                nc.scalar.activation(
                    pt[:, 0:w * QT], sc[:, 0:w * QT], AF.Exp,
                    scale=1.0 / math.sqrt(HEAD_DIM))
                st_.setdefault("pendq", []).append((pt, g0, w))
                flush_av()

            def u_norm():
                while st_.get("pendq"):
                    flush_av(force=True)
                # normalize: bf16 reciprocal of the denom row, broadcast
                # across 64 partitions via a K=1 ones-matmul on PE (no DMA)
                av = st_["av"]
                drow = 64 if slot == "A" else 32
                rows = slice(0, 64) if slot == "A" else slice(64, 128)
                r = rp.tile([128, QT], BF16, tag="r", name="r")
                with nc.allow_low_precision(reason="softmax denom recip bf16"):
                    nc.vector.reciprocal(r[drow:drow + 1, :], av[drow:drow + 1, :])
                rbc_ps = avp.tile([128, QT], F32, tag="av", name="rbc_ps")
                nc.tensor.matmul(
                    rbc_ps[rows, :], ones64[drow:drow + 1, :], r[drow:drow + 1, :],
                    start=True, stop=True, tile_position=(drow, rows.start),
                )
                rbc = rp.tile([128, QT], F32, tag="rbc", name="rbc")
                nc.vector.tensor_copy(rbc[rows, :], rbc_ps[rows, :])
                if slot == "A":
                    nc.vector.tensor_mul(
                        ho[0:64, i * QT:(i + 1) * QT], av[0:64, :], rbc[0:64, :])
                else:
                    # write packed value j to BOTH natural columns 2j, 2j+1;
                    # the per-core parity column mask (data) zeroes the
                    # wrong one right before the out-projection.
                    hob = ho[64:128, 1024 * i:1024 * (i + 1)].rearrange(
                        "p (n two) -> p two n", two=2)
                    nc.vector.tensor_mul(hob[:, 0:1, :], av[64:128, :],
                                         rbc[64:128, :])
                    nc.vector.tensor_mul(hob[:, 1:2, :], av[64:128, :],
                                         rbc[64:128, :])

            units = [lambda g0=g0: u_group(g0) for g0 in range(0, nst, grp)]
            units.append(u_norm)
            return units

        def outproj_units(qt):
            qs = slice(qt * QT, (qt + 1) * QT)

            def u_pmask():
                nc.vector.tensor_mul(
                    ho[64:128, qs], ho[64:128, qs],
                    mask_sb[64:128, 12 * QT:13 * QT])

            def u_op(dch):
                op = opp.tile([128, QT], F32, tag="av" if opp is avp else "op", name="op")
                nc.tensor.matmul(
                    op[:], wo_sb[:, dch * 128:(dch + 1) * 128],
                    ho[:, qs], start=True, stop=True, tile_position=(0, 0),
                )
                ot = osb.tile([128, QT], BF16, tag="ot", name="ot")
                if qt >= 6 or dch % 2 == 1:
                    nc.scalar.copy(ot[:], op[:])
                else:
                    nc.vector.tensor_copy(ot[:], op[:])
                nc.sync.dma_start(
                    out=outT_d[dch * 128:(dch + 1) * 128, qs], in_=ot[:])

            return [u_pmask] + [lambda d=d: u_op(d) for d in range(CCH)]

        def weave(main, filler):
            """Emit `main` units with `filler` units distributed evenly."""
            if not main:
                for f in filler:
                    f()
                return
            nf, nm = len(filler), len(main)
            fi = 0
            for k, u in enumerate(main):
                u()
                while fi * nm < (k + 1) * nf:
                    filler[fi]()
                    fi += 1
            while fi < nf:
                filler[fi]()
                fi += 1

        # ---- software-pipelined emission: proj(tt+1) + deferred input
        # loads woven into jobs(tt) --
        for u in proj_units(0):
            u()
        for tt in range(NQT):
            stream = []
            if tt % 2 == 1:
                stream += job_units("B", (tt - 1) // 2)
                stream += outproj_units(tt - 1)
            stream += job_units("A", tt, avd=2 if tt == NQT - 1 else None)
            if tt % 2 == 1:
                stream += outproj_units(tt)
            filler = []
            if tt in (0, 2, 4):
                k = tt // 2 + 1
                filler.append(lambda k=k: load_xt(k))
                filler.append(lambda k=k: load_xtb(k))
            filler += proj_units(tt + 1) if tt + 1 < NQT else []
            weave(stream, filler)
    nc.finalize()
    return nc


def _host_inputs(x, wq, bq, wk, bk, wv, bv, wo):
    """Per-core input maps. Slot A of core c = head c; slot B = split head
    8 + c//2 with token parity c%2."""
    bf16 = ml_dtypes.bfloat16
    xT = np.ascontiguousarray(x[0].T).astype(bf16)
    xTB_by_par = [np.ascontiguousarray(x[0][p::2].T).astype(bf16) for p in (0, 1)]

    in_maps = []
    for c in range(N_CORES):
        ha, hb, par = c, 8 + c // 2, c % 2
        w = np.zeros((D_MODEL, 384), np.float32)
        w[:, 0:64] = wq[ha]
        w[:, 64:128] = wk[ha]
        w[:, 128:192] = wk[hb]
        w[:, 192:256] = wv[ha]
        w[:, 256:320] = wv[hb]
        w[:, 320:384] = wq[hb]
        b = np.zeros((128, 3), np.float32)
        b[0:64, 0] = bq[ha]
        b[0:64, 1] = bk[ha]
        b[64:128, 1] = bk[hb]
        b[64:128, 2] = bq[hb]
        wo2 = np.zeros((128, D_MODEL), np.float32)
        wo2[0:64] = wo[ha * 64:(ha + 1) * 64]
        wo2[64:128] = wo[hb * 64:(hb + 1) * 64]
        kl = np.arange(128)[:, None]
        qq = np.arange(QT)[None, :]
        masks = np.zeros((NMASK, 128, QT), np.float32)
        for pat in range(4):
            masks[pat] = (128 * pat + kl) <= qq
        for pat in range(8):
            masks[4 + pat] = (128 * pat + kl) <= (2 * qq + par)
        masks[12, :, :] = (qq % 2 == par)
        in_maps.append({
            "xT": xT,
            "xTB": xTB_by_par[par],
            "wproj": w.astype(bf16),
            "bqk": b.astype(np.float32),
            "wo2": wo2.astype(bf16),
            "masks": masks.astype(bf16),
        })
    return in_maps


def kernel(_trace=False, _tmpdir=None, **inputs):
    x = np.asarray(inputs["x"], np.float32)
    wq = np.asarray(inputs["wq"], np.float32)
    bq = np.asarray(inputs["bq"], np.float32)
    wk = np.asarray(inputs["wk"], np.float32)
    bk = np.asarray(inputs["bk"], np.float32)
    wv = np.asarray(inputs["wv"], np.float32)
    bv = np.asarray(inputs["bv"], np.float32)
    wo = np.asarray(inputs["wo"], np.float32)
    bo = np.asarray(inputs["bo"], np.float32)

    if "nc" not in _PROGRAM_CACHE:
        _PROGRAM_CACHE["nc"] = build_program()
    nc = _PROGRAM_CACHE["nc"]

    in_maps = _host_inputs(x, wq, bq, wk, bk, wv, bv, wo)
    res = run_bass_kernel_spmd(
        nc, in_maps, list(range(N_CORES)), trace=_trace, tmpdir=_tmpdir,
    )
    acc = np.zeros((D_MODEL, T), np.float32)
    for c in range(N_CORES):
        acc += res.results[c]["outT"]
    # V-bias folds to a constant through softmax: + bv_cat @ wo (+ bo)
    const = bv.reshape(-1) @ wo + bo
    out = acc.T + const[None, :]
    if _trace:
        return out[None].astype(np.float32), res
    return out[None].astype(np.float32)


# revision 63
# speedup vs baseline: 1.2116x; 1.0112x over previous
"""Multi-head causal attention (B=1, T=4096, D=768, H=12) on 8 trn2 cores.

Sharding: per core, slot A = one full head (heads 0-7 across the 8 cores);
slot B = half of a split head (heads 8-11, each split across 2 cores by
token PARITY: core 2k gets even tokens of head 8+k, core 2k+1 odd tokens).
Parity-splitting keeps the causal key extents identical across cores, so
every core runs the IDENTICAL program (SPMD); cores differ only in data
(weights, masks, parity).  Slot B's queries are packed (parity-strided
projection); its head outputs are written back to natural token columns
with stride-2 DVE writes, so one merged out-projection covers both slots
and the host just sums the 8 partial [768, 4096] outputs.

Per-core work: slot A = 144 key-chunk units, slot B = 80 units (vs 288 for
the old 2-full-slot scheme).  V-bias is folded into a host-side constant
(P@(V+1 bv^T) = P@V + denom bv^T, exact through softmax normalization).

On-device layout (per core):
  xT    [768, 4096] bf16   x transposed (host supplies); xTB = parity cols
  K_sb  [128, 4096] bf16   rows 0:64 head-A K dims, 64:128 head-B K dims
  Q_sb  [128, 4096] bf16   rows 0:64 head-A Q (natural); rows 64:128 cols
                           0:2048 head-B Q (parity-packed)
  V2    [128, 32*208] bf16 per key-chunk: [V_A 0:64 |1@64| 0 |1@97| 0 |
                           V_B 129:193] -> one matmul per slot yields AV
                           rows + a denominator row (A: row 64, B: row 32)
  scores chunks [128 keys, 512 q] in PSUM, exp'd on ACT -> PT bf16

Scheduling: emission is software-pipelined — projection of token tile
tt+1 and deferred xT loads are woven between the attention score groups
of stage tt, and each group's mask+AV matmuls are delayed 8 groups behind
its exp so PE never stalls on the exp latency.  Softmax normalization
broadcasts 1/denom across partitions with a K=1 ones-matmul on PE (no
DRAM bounce).  Out-projection per 512-query tile -> bf16 partials.
"""

import math
import numpy as np
import ml_dtypes
from contextlib import ExitStack

import concourse.bass as bass
import concourse.bacc as bacc
import concourse.mybir as mybir
import concourse.tile as tile
from concourse.bass_utils import run_bass_kernel_spmd

BF16 = mybir.dt.bfloat16
F32 = mybir.dt.float32
F8 = mybir.dt.float8e4
DR = mybir.MatmulPerfMode.DoubleRow
AF = mybir.ActivationFunctionType

T = 4096
D_MODEL = 768
HEAD_DIM = 64
N_HEADS = 12
N_CORES = 8
QT = 512                  # query tile width (A natural / B packed)
KC = 128                  # key chunk (psum partition dim)
GRP = 2                   # score chunks per exp group (psum banks)
NQT = T // QT             # 8 A-tiles
NPB = 4                   # B packed tiles (each covers 1024 natural tokens)
CCH = D_MODEL // 128      # 6 contraction chunks
VST = 208                 # V2 stride per key chunk
NMASK = 13                # 4 A diag patterns + 8 B patterns + parity col mask

_PROGRAM_CACHE = {}


def build_program():
    nc = bacc.Bacc(None)

    xT_d = nc.declare_dram_parameter("xT", [D_MODEL, T], BF16, isOutput=False)
    # x columns of this core's parity, packed: x[:, parity::2].T
    xTB_d = nc.declare_dram_parameter("xTB", [D_MODEL, T // 2], BF16, isOutput=False)
    # w cols: 0:64 wq_A | 64:192 wk_AB | 192:320 wv_AB | 320:384 wq_B
    w_d = nc.declare_dram_parameter("wproj", [D_MODEL, 384], BF16, isOutput=False)
    b_d = nc.declare_dram_parameter("bqk", [128, 3], F32, isOutput=False)
    wo_d = nc.declare_dram_parameter("wo2", [128, D_MODEL], BF16, isOutput=False)
    mk_d = nc.declare_dram_parameter("masks", [NMASK, 128, QT], BF16, isOutput=False)
    outT_d = nc.declare_dram_parameter("outT", [D_MODEL, T], BF16, isOutput=True)

    with tile.TileContext(nc) as tc, ExitStack() as ctx:
        consts = ctx.enter_context(tc.tile_pool(name="consts", bufs=1))
        big = ctx.enter_context(tc.tile_pool(name="big", bufs=1))
        ptp = ctx.enter_context(tc.tile_pool(name="ptp", bufs=int(__import__("os").environ.get("KPTP", "9"))))
        osb = ctx.enter_context(tc.tile_pool(name="osb", bufs=3))
        rp = ctx.enter_context(tc.tile_pool(name="rp", bufs=2))
        dramp = ctx.enter_context(tc.tile_pool(name="dramp", bufs=2, space="DRAM"))
        # PSUM budget is 8 banks total; knobs for the split
        import os as _os
        fp8 = _os.environ.get("KFP8", "0") == "1"
        grp = int(_os.environ.get("KGRP", str(GRP)))
        _scb = int(_os.environ.get("KSCB", "2"))
        _avp = int(_os.environ.get("KAVP", "2"))
        _opp = int(_os.environ.get("KOPP", "2"))
        scp = ctx.enter_context(tc.tile_pool(name="scp", bufs=_scb, space="PSUM"))
        avp = ctx.enter_context(tc.tile_pool(name="avp", bufs=_avp, space="PSUM"))
        if _opp > 0:
            opp = ctx.enter_context(
                tc.tile_pool(name="opp", bufs=_opp, space="PSUM"))
        else:
            opp = avp

        # ---- inputs to SBUF: small consts first, then xT streamed in
        # token-tile slices so stage-0 projection starts within ~4us ----
        w_sb = consts.tile([128, CCH * 384], BF16, tag="w")
        _wap = w_d[:, :]
        nc.sync.dma_start(
            out=w_sb[:],
            in_=bass.AP(tensor=_wap.tensor, offset=_wap.offset,
                        ap=[[384, 128], [128 * 384, CCH], [1, 384]]))
        b_sb = consts.tile([128, 3], F32, tag="b")
        nc.sync.dma_start(out=b_sb[:], in_=b_d[:, :])
        wo_sb = consts.tile([128, D_MODEL], BF16, tag="wo")
        nc.sync.dma_start(out=wo_sb[:], in_=wo_d[:, :])
        mask_sb = consts.tile([128, NMASK * QT], BF16, tag="mask")
        # xT sliced per token-tile pair so stage-0/1 projection starts early;
        # first xTB slice interleaved (stage 1 needs it)
        xT_sb = [big.tile([128, T], BF16, tag=f"xT{j}", name=f"xT{j}")
                 for j in range(CCH)]
        xTB_sb = [big.tile([128, T // 2], BF16, tag=f"xTB{j}", name=f"xTB{j}")
                  for j in range(CCH)]
        def load_xt(tp):
            cs = slice(tp * 2 * QT, (tp + 1) * 2 * QT)
            for j in range(CCH):
                nc.sync.dma_start(out=xT_sb[j][:, cs],
                                  in_=xT_d[j * 128:(j + 1) * 128, cs])

        def load_xtb(pb):
            cs = slice(pb * QT, (pb + 1) * QT)
            for j in range(CCH):
                nc.sync.dma_start(out=xTB_sb[j][:, cs],
                                  in_=xTB_d[j * 128:(j + 1) * 128, cs])

        def load_xt1(tt):
            cs = slice(tt * QT, (tt + 1) * QT)
            for j in range(CCH):
                nc.sync.dma_start(out=xT_sb[j][:, cs],
                                  in_=xT_d[j * 128:(j + 1) * 128, cs])

        # only the slices stage 0/1 need right away; masks deferred behind
        # them; the rest are woven in as filler
        load_xt1(0)
        load_xt1(1)
        load_xtb(0)
        _map = mk_d[:, :, :]
        nc.sync.dma_start(
            out=mask_sb[:],
            in_=bass.AP(tensor=_map.tensor, offset=_map.offset,
                        ap=[[QT, 128], [128 * QT, NMASK], [1, QT]]))

        # ---- persistent tensors ----
        QKDT = F8 if fp8 else BF16
        K_sb = big.tile([128, T], QKDT, tag="K")
        Q_sb = big.tile([128, T], QKDT, tag="Q")
        if fp8:
            # plane layout for DoubleRow: rows 0:32 slot A (head-dim planes
            # 0:32 / 32:64 at byte offsets 0 / T), rows 32:64 slot B
            K8p = big.tile([128, 2 * T], F8, tag="K8p")
            Q8p = big.tile([128, 2 * T], F8, tag="Q8p")
            K8p3 = K8p[:].rearrange("p (two n) -> p two n", two=2)
            Q8p3 = Q8p[:].rearrange("p (two n) -> p two n", two=2)
        V2 = big.tile([128, (T // KC) * VST], BF16, tag="V2")
        ho = big.tile([128, T], BF16, tag="ho")
        nc.gpsimd.memset(ho[:], 0.0)
        nc.gpsimd.memset(V2[:], 0.0)
        v3 = V2[:].rearrange("p (t c) -> p t c", c=VST)
        nc.gpsimd.memset(v3[:, :, 64:65], 1.0)    # ones row for denom_A
        nc.gpsimd.memset(v3[:, :, 97:98], 1.0)    # ones row for denom_B
        ones64 = consts.tile([128, 64], BF16, tag="ones64")
        nc.gpsimd.memset(ones64[:], 1.0)          # lhsT for recip broadcast

        def proj_units(tt):
            """List of thunks emitting projection for token tile tt."""
            ts = slice(tt * QT, (tt + 1) * QT)
            odd = tt % 2 == 1
            pb = (tt - 1) // 2
            st_ = {}
            units = []

            # NOTE: only ONE pending psum accumulation group per 2KB bank —
            # K (bank0) + Q_A (bank1) may interleave, but Q_B (also bank1)
            # and each V sub-group (all in pv bank0) must run after the
            # previous same-bank group has stopped.
            def u_kq(j):
                if j == 0:
                    st_["pk"] = scp.tile([128, grp * QT], F32, tag="sc",
                                         name="pk")
                pk = st_["pk"]
                rhs = xT_sb[j][:, ts]
                st, sp = j == 0, j == CCH - 1
                nc.tensor.matmul(
                    pk[:, 0:QT], w_sb[:, j * 384 + 64:j * 384 + 192], rhs,
                    start=st, stop=sp, tile_position=(0, 0),
                )
                nc.tensor.matmul(
                    pk[0:64, QT:2 * QT], w_sb[:, j * 384:j * 384 + 64], rhs,
                    start=st, stop=sp, tile_position=(0, 0),
                )

            def u_qb(j):
                pk = st_["pk"]
                nc.tensor.matmul(
                    pk[64:128, QT:2 * QT],
                    w_sb[:, j * 384 + 320:j * 384 + 384],
                    xTB_sb[j][:, pb * QT:(pb + 1) * QT],
                    start=(j == 0), stop=(j == CCH - 1), tile_position=(0, 64),
                )

            def u_cast_kqa():
                pk = st_["pk"]
                nc.vector.tensor_scalar_add(K_sb[:, ts], pk[:, 0:QT], b_sb[:, 1:2])
                nc.vector.tensor_scalar_add(
                    Q_sb[0:64, ts], pk[0:64, QT:2 * QT], b_sb[0:64, 0:1])

            def u_regroup_kqa():
                # fp8 plane regroup: flat rows (4 groups of 32) -> plane
                # layout rows 0:32 (A) / 32:64 (B), byte offset 0 / T.
                # SWDGE (gpsimd) path keeps these off the busy HWDGE.
                for src0, dst0, pl in ((0, 0, 0), (32, 0, 1),
                                       (64, 32, 0), (96, 32, 1)):
                    nc.gpsimd.dma_start(
                        out=K8p3[dst0:dst0 + 32, pl:pl + 1, ts],
                        in_=K_sb[src0:src0 + 32, ts])
                for src0, pl in ((0, 0), (32, 1)):
                    nc.gpsimd.dma_start(
                        out=Q8p3[0:32, pl:pl + 1, ts],
                        in_=Q_sb[src0:src0 + 32, ts])

            def u_cast_qb():
                pk = st_["pk"]
                nc.vector.tensor_scalar_add(
                    Q_sb[64:128, pb * QT:(pb + 1) * QT],
                    pk[64:128, QT:2 * QT], b_sb[64:128, 2:3])

            def u_regroup_qb():
                pbs = slice(pb * QT, (pb + 1) * QT)
                for src0, pl in ((64, 0), (96, 1)):
                    nc.gpsimd.dma_start(
                        out=Q8p3[32:64, pl:pl + 1, pbs],
                        in_=Q_sb[src0:src0 + 32, pbs])

            def u_v(sub):
                if sub == 0:
                    st_["pv"] = scp.tile([128, grp * QT], F32, tag="sc",
                                         name="pv")
                pv = st_["pv"]
                kc = tt * 4 + sub
                for j in range(CCH):
                    nc.tensor.matmul(
                        pv[:, sub * 128:(sub + 1) * 128],
                        xT_sb[j][:, kc * KC:(kc + 1) * KC],
                        w_sb[:, j * 384 + 192:j * 384 + 320],
                        start=(j == 0), stop=(j == CCH - 1), tile_position=(0, 0),
                    )

            def u_vcopy():
                pv = st_["pv"]
                for sub in range(4):
                    kc = tt * 4 + sub
                    blk = V2[:, kc * VST:kc * VST + 193]
                    out_ap = bass.AP(tensor=blk.tensor, offset=blk.offset,
                                     ap=[list(blk.ap[0]), [129, 2], [1, 64]])
                    nc.vector.tensor_copy(out_ap, pv[:, sub * 128:(sub + 1) * 128])

            for j in range(CCH):
                units.append(lambda j=j: u_kq(j))
            units.append(u_cast_kqa)
            if fp8:
                units.append(u_regroup_kqa)
            if odd:
                for j in range(CCH):
                    units.append(lambda j=j: u_qb(j))
                units.append(u_cast_qb)
                if fp8:
                    units.append(u_regroup_qb)
            for sub in range(4):
                units.append(lambda sub=sub: u_v(sub))
            units.append(u_vcopy)
            return units

        def job_units(slot, i, avd=None):
            """slot 'A': full head, query tile i (natural); slot 'B': split
            head, packed tile i.  Returns list of thunks (one per score
            group + a normalize tail)."""
            if slot == "A":
                nst, band0 = 4 * (i + 1), 4 * i
                krow, tp = 0, (0, 0)
            else:
                nst, band0 = 8 * (i + 1), 8 * i
                krow, tp = 64, (64, 0)
            qrow = slice(krow, krow + 64)
            st_ = {}

            def flush_av(force=False):
                # masks + AV for a group exp'd earlier; the delay keeps PE
                # from stalling on the exp latency
                import os as _os2
                depth = avd if avd is not None else int(_os2.environ.get("KAVD", "8"))
                pend = st_.setdefault("pendq", [])
                if not pend or (not force and len(pend) <= depth - 1):
                    return
                pt, g0, w = pend.pop(0)
                av = st_["av"]
                for gi in range(w):
                    kc = g0 + gi
                    # valid-query truncation: for diagonal-band chunks,
                    # queries below qoff are entirely masked-out, so the
                    # mask mul and AV matmul (incl. denom row) skip them —
                    # exact, since those queries don't attend these keys.
                    qoff = 0
                    if kc >= band0:
                        pat = kc - band0
                        mi = pat if slot == "A" else 4 + pat
                        qoff = (128 if slot == "A" else 64) * pat
                        nc.vector.tensor_mul(
                            pt[:, gi * QT + qoff:(gi + 1) * QT],
                            pt[:, gi * QT + qoff:(gi + 1) * QT],
                            mask_sb[:, mi * QT + qoff:(mi + 1) * QT])
                    ptj = pt[:, gi * QT + qoff:(gi + 1) * QT]
                    st, sp = kc == 0, kc == nst - 1
                    if slot == "A":
                        nc.tensor.matmul(
                            av[0:65, qoff:QT], V2[:, kc * VST:kc * VST + 65],
                            ptj, start=st, stop=sp, tile_position=(0, 0),
                        )
                    else:
                        nc.tensor.matmul(
                            av[0:128, qoff:QT],
                            V2[:, kc * VST + 65:kc * VST + 193],
                            ptj, start=st, stop=sp, tile_position=(0, 0),
                        )

            def _qoff(kc):
                # valid-query start for diagonal-band chunks: queries below
                # this are strictly non-causal for every key in the chunk
                if kc < band0:
                    return 0
                return (128 if slot == "A" else 64) * (kc - band0)

            def u_group(g0):
                if g0 == 0:
                    st_["av"] = avp.tile([128, QT], F32, tag="av", name="av")
                w = min(grp, nst - g0)
                sc = scp.tile([128, grp * QT], F32, tag="sc", name="sc")
                qoffs = [_qoff(g0 + gi) for gi in range(w)]
                for gi in range(w):
                    kc = g0 + gi
                    qo = qoffs[gi]
                    if fp8:
                        prow = 0 if slot == "A" else 32
                        nc.tensor.matmul(
                            sc[:, gi * QT + qo:(gi + 1) * QT],
                            K8p3[prow:prow + 32, :, kc * KC:(kc + 1) * KC],
                            Q8p3[prow:prow + 32, :, i * QT + qo:(i + 1) * QT],
                            start=True, stop=True, perf_mode=DR,
                            tile_position=(prow, 0),
                        )
                    else:
                        nc.tensor.matmul(
                            sc[:, gi * QT + qo:(gi + 1) * QT],
                            K_sb[krow:krow + 64, kc * KC:(kc + 1) * KC],
                            Q_sb[qrow, i * QT + qo:(i + 1) * QT],
                            start=True, stop=True, tile_position=tp,
                        )
                pt = ptp.tile([128, grp * QT], BF16, tag="pt", name="pt")
                if any(qoffs):
                    # ragged group: exp per chunk over its valid sub-range
                    # (reads only psum the truncated score matmul wrote)
                    for gi in range(w):
                        qo = qoffs[gi]
                        nc.scalar.activation(
                            pt[:, gi * QT + qo:(gi + 1) * QT],
                            sc[:, gi * QT + qo:(gi + 1) * QT], AF.Exp,
                            scale=1.0 / math.sqrt(HEAD_DIM))
                else:
                    nc.scalar.activation(
                        pt[:, 0:w * QT], sc[:, 0:w * QT], AF.Exp,
                        scale=1.0 / math.sqrt(HEAD_DIM))
                st_.setdefault("pendq", []).append((pt, g0, w))
                flush_av()

            def u_norm():
                while st_.get("pendq"):
                    flush_av(force=True)
                # normalize: bf16 reciprocal of the denom row, broadcast
                # across 64 partitions via a K=1 ones-matmul on PE (no DMA)
                av = st_["av"]
                drow = 64 if slot == "A" else 32
                rows = slice(0, 64) if slot == "A" else slice(64, 128)
                r = rp.tile([128, QT], BF16, tag="r", name="r")
                with nc.allow_low_precision(reason="softmax denom recip bf16"):
                    nc.vector.reciprocal(r[drow:drow + 1, :], av[drow:drow + 1, :])
                rbc_ps = avp.tile([128, QT], F32, tag="av", name="rbc_ps")
                nc.tensor.matmul(
                    rbc_ps[rows, :], ones64[drow:drow + 1, :], r[drow:drow + 1, :],
                    start=True, stop=True, tile_position=(drow, rows.start),
                )
                rbc = rp.tile([128, QT], F32, tag="rbc", name="rbc")
                nc.vector.tensor_copy(rbc[rows, :], rbc_ps[rows, :])
                if slot == "A":
                    nc.vector.tensor_mul(
                        ho[0:64, i * QT:(i + 1) * QT], av[0:64, :], rbc[0:64, :])
                else:
                    # write packed value j to BOTH natural columns 2j, 2j+1;
                    # the per-core parity column mask (data) zeroes the
                    # wrong one right before the out-projection.
                    hob = ho[64:128, 1024 * i:1024 * (i + 1)].rearrange(
                        "p (n two) -> p two n", two=2)
                    nc.vector.tensor_mul(hob[:, 0:1, :], av[64:128, :],
                                         rbc[64:128, :])
                    nc.vector.tensor_mul(hob[:, 1:2, :], av[64:128, :],
                                         rbc[64:128, :])

            units = [lambda g0=g0: u_group(g0) for g0 in range(0, nst, grp)]
            units.append(u_norm)
            return units

        def outproj_units(qt):
            qs = slice(qt * QT, (qt + 1) * QT)

            def u_pmask():
                nc.vector.tensor_mul(
                    ho[64:128, qs], ho[64:128, qs],
                    mask_sb[64:128, 12 * QT:13 * QT])

            def u_op(dch):
                op = opp.tile([128, QT], F32, tag="av" if opp is avp else "op", name="op")
                nc.tensor.matmul(
                    op[:], wo_sb[:, dch * 128:(dch + 1) * 128],
                    ho[:, qs], start=True, stop=True, tile_position=(0, 0),
                )
                ot = osb.tile([128, QT], BF16, tag="ot", name="ot")
                if qt >= 6 or dch % 2 == 1:
                    nc.scalar.copy(ot[:], op[:])
                else:
                    nc.vector.tensor_copy(ot[:], op[:])
                nc.sync.dma_start(
                    out=outT_d[dch * 128:(dch + 1) * 128, qs], in_=ot[:])

            return [u_pmask] + [lambda d=d: u_op(d) for d in range(CCH)]

        def weave(main, filler):
            """Emit `main` units with `filler` units distributed evenly."""
            if not main:
                for f in filler:
                    f()
                return
            nf, nm = len(filler), len(main)
            fi = 0
            for k, u in enumerate(main):
                u()
                while fi * nm < (k + 1) * nf:
                    filler[fi]()
                    fi += 1
            while fi < nf:
                filler[fi]()
                fi += 1

        # ---- software-pipelined emission: proj(tt+1) + deferred input
        # loads woven into jobs(tt) --
        for u in proj_units(0):
            u()
        for tt in range(NQT):
            stream = []
            if tt % 2 == 1:
                stream += job_units("B", (tt - 1) // 2)
                stream += outproj_units(tt - 1)
            stream += job_units("A", tt, avd=2 if tt == NQT - 1 else None)
            if tt % 2 == 1:
                stream += outproj_units(tt)
            filler = []
            if tt in (0, 2, 4):
                k = tt // 2 + 1
                filler.append(lambda k=k: load_xt(k))
                filler.append(lambda k=k: load_xtb(k))
            filler += proj_units(tt + 1) if tt + 1 < NQT else []
            weave(stream, filler)
    nc.finalize()
    return nc


def _host_inputs(x, wq, bq, wk, bk, wv, bv, wo):
    """Per-core input maps. Slot A of core c = head c; slot B = split head
    8 + c//2 with token parity c%2."""
    bf16 = ml_dtypes.bfloat16
    xT = np.ascontiguousarray(x[0].T).astype(bf16)
    xTB_by_par = [np.ascontiguousarray(x[0][p::2].T).astype(bf16) for p in (0, 1)]

    in_maps = []
    for c in range(N_CORES):
        ha, hb, par = c, 8 + c // 2, c % 2
        w = np.zeros((D_MODEL, 384), np.float32)
        w[:, 0:64] = wq[ha]
        w[:, 64:128] = wk[ha]
        w[:, 128:192] = wk[hb]
        w[:, 192:256] = wv[ha]
        w[:, 256:320] = wv[hb]
        w[:, 320:384] = wq[hb]
        b = np.zeros((128, 3), np.float32)
        b[0:64, 0] = bq[ha]
        b[0:64, 1] = bk[ha]
        b[64:128, 1] = bk[hb]
        b[64:128, 2] = bq[hb]
        wo2 = np.zeros((128, D_MODEL), np.float32)
        wo2[0:64] = wo[ha * 64:(ha + 1) * 64]
        wo2[64:128] = wo[hb * 64:(hb + 1) * 64]
        kl = np.arange(128)[:, None]
        qq = np.arange(QT)[None, :]
        masks = np.zeros((NMASK, 128, QT), np.float32)
        for pat in range(4):
            masks[pat] = (128 * pat + kl) <= qq
        for pat in range(8):
            masks[4 + pat] = (128 * pat + kl) <= (2 * qq + par)
        masks[12, :, :] = (qq % 2 == par)
        in_maps.append({
            "xT": xT,
            "xTB": xTB_by_par[par],
            "wproj": w.astype(bf16),
            "bqk": b.astype(np.float32),
            "wo2": wo2.astype(bf16),
            "masks": masks.astype(bf16),
        })
    return in_maps


def kernel(_trace=False, _tmpdir=None, **inputs):
    x = np.asarray(inputs["x"], np.float32)
    wq = np.asarray(inputs["wq"], np.float32)
    bq = np.asarray(inputs["bq"], np.float32)
    wk = np.asarray(inputs["wk"], np.float32)
    bk = np.asarray(inputs["bk"], np.float32)
    wv = np.asarray(inputs["wv"], np.float32)
    bv = np.asarray(inputs["bv"], np.float32)
    wo = np.asarray(inputs["wo"], np.float32)
    bo = np.asarray(inputs["bo"], np.float32)

    if "nc" not in _PROGRAM_CACHE:
        _PROGRAM_CACHE["nc"] = build_program()
    nc = _PROGRAM_CACHE["nc"]

    in_maps = _host_inputs(x, wq, bq, wk, bk, wv, bv, wo)
    res = run_bass_kernel_spmd(
        nc, in_maps, list(range(N_CORES)), trace=_trace, tmpdir=_tmpdir,
    )
    acc = np.zeros((D_MODEL, T), np.float32)
    for c in range(N_CORES):
        acc += res.results[c]["outT"]
    # V-bias folds to a constant through softmax: + bv_cat @ wo (+ bo)
    const = bv.reshape(-1) @ wo + bo
    out = acc.T + const[None, :]
    if _trace:
        return out[None].astype(np.float32), res
    return out[None].astype(np.float32)


# revision 66
# speedup vs baseline: 1.2210x; 1.0077x over previous
"""Multi-head causal attention (B=1, T=4096, D=768, H=12) on 8 trn2 cores.

Sharding: per core, slot A = one full head (heads 0-7 across the 8 cores);
slot B = half of a split head (heads 8-11, each split across 2 cores by
token PARITY: core 2k gets even tokens of head 8+k, core 2k+1 odd tokens).
Parity-splitting keeps the causal key extents identical across cores, so
every core runs the IDENTICAL program (SPMD); cores differ only in data
(weights, masks, parity).  Slot B's queries are packed (parity-strided
projection); its head outputs are written back to natural token columns
with stride-2 DVE writes, so one merged out-projection covers both slots
and the host just sums the 8 partial [768, 4096] outputs.

Per-core work: slot A = 144 key-chunk units, slot B = 80 units (vs 288 for
the old 2-full-slot scheme).  V-bias is folded into a host-side constant
(P@(V+1 bv^T) = P@V + denom bv^T, exact through softmax normalization).

On-device layout (per core):
  xT    [768, 4096] bf16   x transposed (host supplies); xTB = parity cols
  K_sb  [128, 4096] bf16   rows 0:64 head-A K dims, 64:128 head-B K dims
  Q_sb  [128, 4096] bf16   rows 0:64 head-A Q (natural); rows 64:128 cols
                           0:2048 head-B Q (parity-packed)
  V2    [128, 32*208] bf16 per key-chunk: [V_A 0:64 |1@64| 0 |1@97| 0 |
                           V_B 129:193] -> one matmul per slot yields AV
                           rows + a denominator row (A: row 64, B: row 32)
  scores chunks [128 keys, 512 q] in PSUM, exp'd on ACT -> PT bf16

Scheduling: emission is software-pipelined — projection of token tile
tt+1 and deferred xT loads are woven between the attention score groups
of stage tt, and each group's mask+AV matmuls are delayed 8 groups behind
its exp so PE never stalls on the exp latency.  Softmax normalization
broadcasts 1/denom across partitions with a K=1 ones-matmul on PE (no
DRAM bounce).  Out-projection per 512-query tile -> bf16 partials.
"""

import math
import numpy as np
import ml_dtypes
from contextlib import ExitStack

import concourse.bass as bass
import concourse.bacc as bacc
import concourse.mybir as mybir
import concourse.tile as tile
from concourse.bass_utils import run_bass_kernel_spmd

BF16 = mybir.dt.bfloat16
F32 = mybir.dt.float32
F8 = mybir.dt.float8e4
DR = mybir.MatmulPerfMode.DoubleRow
AF = mybir.ActivationFunctionType

T = 4096
D_MODEL = 768
HEAD_DIM = 64
N_HEADS = 12
N_CORES = 8
QT = 512                  # query tile width (A natural / B packed)
KC = 128                  # key chunk (psum partition dim)
GRP = 2                   # score chunks per exp group (psum banks)
NQT = T // QT             # 8 A-tiles
NPB = 4                   # B packed tiles (each covers 1024 natural tokens)
CCH = D_MODEL // 128      # 6 contraction chunks
VST = 208                 # V2 stride per key chunk
NMASK = 13                # 4 A diag patterns + 8 B patterns + parity col mask

_PROGRAM_CACHE = {}


def build_program():
    nc = bacc.Bacc(None)

    xT_d = nc.declare_dram_parameter("xT", [D_MODEL, T], BF16, isOutput=False)
    # x columns of this core's parity, packed: x[:, parity::2].T
    xTB_d = nc.declare_dram_parameter("xTB", [D_MODEL, T // 2], BF16, isOutput=False)
    # w cols: 0:64 wq_A | 64:192 wk_AB | 192:320 wv_AB | 320:384 wq_B
    w_d = nc.declare_dram_parameter("wproj", [D_MODEL, 384], BF16, isOutput=False)
    b_d = nc.declare_dram_parameter("bqk", [128, 3], F32, isOutput=False)
    wo_d = nc.declare_dram_parameter("wo2", [128, D_MODEL], BF16, isOutput=False)
    mk_d = nc.declare_dram_parameter("masks", [NMASK, 128, QT], BF16, isOutput=False)
    outT_d = nc.declare_dram_parameter("outT", [D_MODEL, T], BF16, isOutput=True)

    with tile.TileContext(nc) as tc, ExitStack() as ctx:
        consts = ctx.enter_context(tc.tile_pool(name="consts", bufs=1))
        big = ctx.enter_context(tc.tile_pool(name="big", bufs=1))
        ptp = ctx.enter_context(tc.tile_pool(name="ptp", bufs=int(__import__("os").environ.get("KPTP", "9"))))
        osb = ctx.enter_context(tc.tile_pool(name="osb", bufs=3))
        rp = ctx.enter_context(tc.tile_pool(name="rp", bufs=2))
        dramp = ctx.enter_context(tc.tile_pool(name="dramp", bufs=2, space="DRAM"))
        # PSUM budget is 8 banks total; knobs for the split
        import os as _os
        fp8 = _os.environ.get("KFP8", "0") == "1"
        grp = int(_os.environ.get("KGRP", str(GRP)))
        _scb = int(_os.environ.get("KSCB", "2"))
        _avp = int(_os.environ.get("KAVP", "2"))
        _opp = int(_os.environ.get("KOPP", "2"))
        scp = ctx.enter_context(tc.tile_pool(name="scp", bufs=_scb, space="PSUM"))
        avp = ctx.enter_context(tc.tile_pool(name="avp", bufs=_avp, space="PSUM"))
        if _opp > 0:
            opp = ctx.enter_context(
                tc.tile_pool(name="opp", bufs=_opp, space="PSUM"))
        else:
            opp = avp

        # ---- inputs to SBUF: small consts first, then xT streamed in
        # token-tile slices so stage-0 projection starts within ~4us ----
        w_sb = consts.tile([128, CCH * 384], BF16, tag="w")
        _wap = w_d[:, :]
        nc.sync.dma_start(
            out=w_sb[:],
            in_=bass.AP(tensor=_wap.tensor, offset=_wap.offset,
                        ap=[[384, 128], [128 * 384, CCH], [1, 384]]))
        b_sb = consts.tile([128, 3], F32, tag="b")
        nc.sync.dma_start(out=b_sb[:], in_=b_d[:, :])
        wo_sb = consts.tile([128, D_MODEL], BF16, tag="wo")
        nc.sync.dma_start(out=wo_sb[:], in_=wo_d[:, :])
        mask_sb = consts.tile([128, NMASK * QT], BF16, tag="mask")
        # xT sliced per token-tile pair so stage-0/1 projection starts early;
        # first xTB slice interleaved (stage 1 needs it)
        xT_sb = [big.tile([128, T], BF16, tag=f"xT{j}", name=f"xT{j}")
                 for j in range(CCH)]
        xTB_sb = [big.tile([128, T // 2], BF16, tag=f"xTB{j}", name=f"xTB{j}")
                  for j in range(CCH)]

        def xt(j, cs):
            return xT_sb[j][:, cs]

        def xtb(j, cs):
            return xTB_sb[j][:, cs]

        def load_xt1(tt):
            cs = slice(tt * QT, (tt + 1) * QT)
            for j in range(CCH):
                nc.sync.dma_start(out=xT_sb[j][:, cs],
                                  in_=xT_d[j * 128:(j + 1) * 128, cs])

        def load_xt(tp):
            cs = slice(tp * 2 * QT, (tp + 1) * 2 * QT)
            for j in range(CCH):
                nc.sync.dma_start(out=xT_sb[j][:, cs],
                                  in_=xT_d[j * 128:(j + 1) * 128, cs])

        def load_xtb(pb):
            cs = slice(pb * QT, (pb + 1) * QT)
            for j in range(CCH):
                nc.sync.dma_start(out=xTB_sb[j][:, cs],
                                  in_=xTB_d[j * 128:(j + 1) * 128, cs])

        # only the slices stage 0/1 need right away; masks deferred behind
        # them; the rest are woven in as filler
        load_xt1(0)
        load_xt1(1)
        load_xtb(0)
        _map = mk_d[:, :, :]
        nc.sync.dma_start(
            out=mask_sb[:],
            in_=bass.AP(tensor=_map.tensor, offset=_map.offset,
                        ap=[[QT, 128], [128 * QT, NMASK], [1, QT]]))

        # ---- persistent tensors ----
        QKDT = F8 if fp8 else BF16
        K_sb = big.tile([128, T], QKDT, tag="K")
        Q_sb = big.tile([128, T], QKDT, tag="Q")
        if fp8:
            # plane layout for DoubleRow: rows 0:32 slot A (head-dim planes
            # 0:32 / 32:64 at byte offsets 0 / T), rows 32:64 slot B
            K8p = big.tile([128, 2 * T], F8, tag="K8p")
            Q8p = big.tile([128, 2 * T], F8, tag="Q8p")
            K8p3 = K8p[:].rearrange("p (two n) -> p two n", two=2)
            Q8p3 = Q8p[:].rearrange("p (two n) -> p two n", two=2)
        V2 = big.tile([128, (T // KC) * VST], BF16, tag="V2")
        ho = big.tile([128, T], BF16, tag="ho")
        nc.gpsimd.memset(ho[:], 0.0)
        nc.gpsimd.memset(V2[:], 0.0)
        v3 = V2[:].rearrange("p (t c) -> p t c", c=VST)
        nc.gpsimd.memset(v3[:, :, 64:65], 1.0)    # ones row for denom_A
        nc.gpsimd.memset(v3[:, :, 97:98], 1.0)    # ones row for denom_B
        ones64 = consts.tile([128, 64], BF16, tag="ones64")
        nc.gpsimd.memset(ones64[:], 1.0)          # lhsT for recip broadcast

        def proj_units(tt):
            """List of thunks emitting projection for token tile tt."""
            ts = slice(tt * QT, (tt + 1) * QT)
            odd = tt % 2 == 1
            pb = (tt - 1) // 2
            st_ = {}
            units = []

            # NOTE: only ONE pending psum accumulation group per 2KB bank —
            # K (bank0) + Q_A (bank1) may interleave, but Q_B (also bank1)
            # and each V sub-group (all in pv bank0) must run after the
            # previous same-bank group has stopped.
            def u_kq(j):
                if j == 0:
                    st_["pk"] = scp.tile([128, grp * QT], F32, tag="sc",
                                         name="pk")
                pk = st_["pk"]
                rhs = xt(j, ts)
                st, sp = j == 0, j == CCH - 1
                nc.tensor.matmul(
                    pk[:, 0:QT], w_sb[:, j * 384 + 64:j * 384 + 192], rhs,
                    start=st, stop=sp, tile_position=(0, 0),
                )
                nc.tensor.matmul(
                    pk[0:64, QT:2 * QT], w_sb[:, j * 384:j * 384 + 64], rhs,
                    start=st, stop=sp, tile_position=(0, 0),
                )

            def u_qb(j):
                pk = st_["pk"]
                nc.tensor.matmul(
                    pk[64:128, QT:2 * QT],
                    w_sb[:, j * 384 + 320:j * 384 + 384],
                    xtb(j, slice(pb * QT, (pb + 1) * QT)),
                    start=(j == 0), stop=(j == CCH - 1), tile_position=(0, 64),
                )

            def u_cast_kqa():
                pk = st_["pk"]
                nc.vector.tensor_scalar_add(K_sb[:, ts], pk[:, 0:QT], b_sb[:, 1:2])
                nc.vector.tensor_scalar_add(
                    Q_sb[0:64, ts], pk[0:64, QT:2 * QT], b_sb[0:64, 0:1])

            def u_regroup_kqa():
                # fp8 plane regroup: flat rows (4 groups of 32) -> plane
                # layout rows 0:32 (A) / 32:64 (B), byte offset 0 / T.
                # SWDGE (gpsimd) path keeps these off the busy HWDGE.
                for src0, dst0, pl in ((0, 0, 0), (32, 0, 1),
                                       (64, 32, 0), (96, 32, 1)):
                    nc.gpsimd.dma_start(
                        out=K8p3[dst0:dst0 + 32, pl:pl + 1, ts],
                        in_=K_sb[src0:src0 + 32, ts])
                for src0, pl in ((0, 0), (32, 1)):
                    nc.gpsimd.dma_start(
                        out=Q8p3[0:32, pl:pl + 1, ts],
                        in_=Q_sb[src0:src0 + 32, ts])

            def u_cast_qb():
                pk = st_["pk"]
                nc.vector.tensor_scalar_add(
                    Q_sb[64:128, pb * QT:(pb + 1) * QT],
                    pk[64:128, QT:2 * QT], b_sb[64:128, 2:3])

            def u_regroup_qb():
                pbs = slice(pb * QT, (pb + 1) * QT)
                for src0, pl in ((64, 0), (96, 1)):
                    nc.gpsimd.dma_start(
                        out=Q8p3[32:64, pl:pl + 1, pbs],
                        in_=Q_sb[src0:src0 + 32, pbs])

            def u_v(sub):
                if sub == 0:
                    st_["pv"] = scp.tile([128, grp * QT], F32, tag="sc",
                                         name="pv")
                pv = st_["pv"]
                kc = tt * 4 + sub
                for j in range(CCH):
                    nc.tensor.matmul(
                        pv[:, sub * 128:(sub + 1) * 128],
                        xt(j, slice(kc * KC, (kc + 1) * KC)),
                        w_sb[:, j * 384 + 192:j * 384 + 320],
                        start=(j == 0), stop=(j == CCH - 1), tile_position=(0, 0),
                    )

            def u_vcopy():
                pv = st_["pv"]
                for sub in range(4):
                    kc = tt * 4 + sub
                    blk = V2[:, kc * VST:kc * VST + 193]
                    out_ap = bass.AP(tensor=blk.tensor, offset=blk.offset,
                                     ap=[list(blk.ap[0]), [129, 2], [1, 64]])
                    nc.vector.tensor_copy(out_ap, pv[:, sub * 128:(sub + 1) * 128])

            for j in range(CCH):
                units.append(lambda j=j: u_kq(j))
            units.append(u_cast_kqa)
            if fp8:
                units.append(u_regroup_kqa)
            if odd:
                for j in range(CCH):
                    units.append(lambda j=j: u_qb(j))
                units.append(u_cast_qb)
                if fp8:
                    units.append(u_regroup_qb)
            for sub in range(4):
                units.append(lambda sub=sub: u_v(sub))
            units.append(u_vcopy)
            return units

        def job_units(slot, i, avd=None):
            """slot 'A': full head, query tile i (natural); slot 'B': split
            head, packed tile i.  Returns list of thunks (one per score
            group + a normalize tail)."""
            if slot == "A":
                nst, band0 = 4 * (i + 1), 4 * i
                krow, tp = 0, (0, 0)
            else:
                nst, band0 = 8 * (i + 1), 8 * i
                krow, tp = 64, (64, 0)
            qrow = slice(krow, krow + 64)
            st_ = {}

            def flush_av(force=False):
                # masks + AV for a group exp'd earlier; the delay keeps PE
                # from stalling on the exp latency
                import os as _os2
                depth = avd if avd is not None else int(_os2.environ.get("KAVD", "8"))
                pend = st_.setdefault("pendq", [])
                if not pend or (not force and len(pend) <= depth - 1):
                    return
                pt, g0, w = pend.pop(0)
                av = st_["av"]
                for gi in range(w):
                    kc = g0 + gi
                    # valid-query truncation: for diagonal-band chunks,
                    # queries below qoff are entirely masked-out, so the
                    # mask mul and AV matmul (incl. denom row) skip them —
                    # exact, since those queries don't attend these keys.
                    qoff = 0
                    if kc >= band0:
                        pat = kc - band0
                        mi = pat if slot == "A" else 4 + pat
                        qoff = (128 if slot == "A" else 64) * pat
                        nc.vector.tensor_mul(
                            pt[:, gi * QT + qoff:(gi + 1) * QT],
                            pt[:, gi * QT + qoff:(gi + 1) * QT],
                            mask_sb[:, mi * QT + qoff:(mi + 1) * QT])
                    ptj = pt[:, gi * QT + qoff:(gi + 1) * QT]
                    st, sp = kc == 0, kc == nst - 1
                    if slot == "A":
                        nc.tensor.matmul(
                            av[0:65, qoff:QT], V2[:, kc * VST:kc * VST + 65],
                            ptj, start=st, stop=sp, tile_position=(0, 0),
                        )
                    else:
                        nc.tensor.matmul(
                            av[0:128, qoff:QT],
                            V2[:, kc * VST + 65:kc * VST + 193],
                            ptj, start=st, stop=sp, tile_position=(0, 0),
                        )

            def _qoff(kc):
                # valid-query start for diagonal-band chunks: queries below
                # this are strictly non-causal for every key in the chunk
                if kc < band0:
                    return 0
                return (128 if slot == "A" else 64) * (kc - band0)

            def u_group(g0):
                if g0 == 0:
                    st_["av"] = avp.tile([128, QT], F32, tag="av", name="av")
                w = min(grp, nst - g0)
                sc = scp.tile([128, grp * QT], F32, tag="sc", name="sc")
                qoffs = [_qoff(g0 + gi) for gi in range(w)]
                for gi in range(w):
                    kc = g0 + gi
                    qo = qoffs[gi]
                    if fp8:
                        prow = 0 if slot == "A" else 32
                        nc.tensor.matmul(
                            sc[:, gi * QT + qo:(gi + 1) * QT],
                            K8p3[prow:prow + 32, :, kc * KC:(kc + 1) * KC],
                            Q8p3[prow:prow + 32, :, i * QT + qo:(i + 1) * QT],
                            start=True, stop=True, perf_mode=DR,
                            tile_position=(prow, 0),
                        )
                    else:
                        nc.tensor.matmul(
                            sc[:, gi * QT + qo:(gi + 1) * QT],
                            K_sb[krow:krow + 64, kc * KC:(kc + 1) * KC],
                            Q_sb[qrow, i * QT + qo:(i + 1) * QT],
                            start=True, stop=True, tile_position=tp,
                        )
                pt = ptp.tile([128, grp * QT], BF16, tag="pt", name="pt")
                if any(qoffs):
                    # ragged group: exp per chunk over its valid sub-range
                    # (reads only psum the truncated score matmul wrote)
                    for gi in range(w):
                        qo = qoffs[gi]
                        nc.scalar.activation(
                            pt[:, gi * QT + qo:(gi + 1) * QT],
                            sc[:, gi * QT + qo:(gi + 1) * QT], AF.Exp,
                            scale=1.0 / math.sqrt(HEAD_DIM))
                else:
                    nc.scalar.activation(
                        pt[:, 0:w * QT], sc[:, 0:w * QT], AF.Exp,
                        scale=1.0 / math.sqrt(HEAD_DIM))
                st_.setdefault("pendq", []).append((pt, g0, w))
                flush_av()

            def u_norm():
                while st_.get("pendq"):
                    flush_av(force=True)
                # normalize: bf16 reciprocal of the denom row, broadcast
                # across 64 partitions via a K=1 ones-matmul on PE (no DMA)
                av = st_["av"]
                drow = 64 if slot == "A" else 32
                rows = slice(0, 64) if slot == "A" else slice(64, 128)
                r = rp.tile([128, QT], BF16, tag="r", name="r")
                with nc.allow_low_precision(reason="softmax denom recip bf16"):
                    nc.vector.reciprocal(r[drow:drow + 1, :], av[drow:drow + 1, :])
                rbc_ps = avp.tile([128, QT], F32, tag="av", name="rbc_ps")
                nc.tensor.matmul(
                    rbc_ps[rows, :], ones64[drow:drow + 1, :], r[drow:drow + 1, :],
                    start=True, stop=True, tile_position=(drow, rows.start),
                )
                rbc = rp.tile([128, QT], F32, tag="rbc", name="rbc")
                nc.vector.tensor_copy(rbc[rows, :], rbc_ps[rows, :])
                if slot == "A":
                    nc.vector.tensor_mul(
                        ho[0:64, i * QT:(i + 1) * QT], av[0:64, :], rbc[0:64, :])
                else:
                    # write packed value j to BOTH natural columns 2j, 2j+1;
                    # the per-core parity column mask (data) zeroes the
                    # wrong one right before the out-projection.
                    hob = ho[64:128, 1024 * i:1024 * (i + 1)].rearrange(
                        "p (n two) -> p two n", two=2)
                    nc.vector.tensor_mul(hob[:, 0:1, :], av[64:128, :],
                                         rbc[64:128, :])
                    nc.vector.tensor_mul(hob[:, 1:2, :], av[64:128, :],
                                         rbc[64:128, :])

            units = [lambda g0=g0: u_group(g0) for g0 in range(0, nst, grp)]
            units.append(u_norm)
            return units

        def outproj_units(qt):
            qs = slice(qt * QT, (qt + 1) * QT)

            def u_pmask():
                nc.vector.tensor_mul(
                    ho[64:128, qs], ho[64:128, qs],
                    mask_sb[64:128, 12 * QT:13 * QT])

            def u_op(dch):
                op = opp.tile([128, QT], F32, tag="av" if opp is avp else "op", name="op")
                nc.tensor.matmul(
                    op[:], wo_sb[:, dch * 128:(dch + 1) * 128],
                    ho[:, qs], start=True, stop=True, tile_position=(0, 0),
                )
                ot = osb.tile([128, QT], BF16, tag="ot", name="ot")
                if qt >= 6 or dch % 2 == 1:
                    nc.scalar.copy(ot[:], op[:])
                else:
                    nc.vector.tensor_copy(ot[:], op[:])
                nc.sync.dma_start(
                    out=outT_d[dch * 128:(dch + 1) * 128, qs], in_=ot[:])

            return [u_pmask] + [lambda d=d: u_op(d) for d in range(CCH)]

        def weave(main, filler):
            """Emit `main` units with `filler` units distributed evenly."""
            if not main:
                for f in filler:
                    f()
                return
            nf, nm = len(filler), len(main)
            fi = 0
            for k, u in enumerate(main):
                u()
                while fi * nm < (k + 1) * nf:
                    filler[fi]()
                    fi += 1
            while fi < nf:
                filler[fi]()
                fi += 1

        # ---- software-pipelined emission: proj(tt+1) + deferred input
        # loads woven into jobs(tt) --
        for u in proj_units(0):
            u()
        for tt in range(NQT):
            # A_tt woven with proj(tt+1); then (even tt) B_{tt//2} — safe
            # only after ALL proj(tt+1) units, since its diagonal-band
            # chunks read K/V of tile tt+1
            stream = job_units("A", tt, avd=2 if tt == NQT - 1 else None)
            filler = []
            if tt in (0, 2, 4):
                k = tt // 2 + 1
                filler.append(lambda k=k: load_xt(k))
                filler.append(lambda k=k: load_xtb(k))
            filler += proj_units(tt + 1) if tt + 1 < NQT else []
            weave(stream, filler)
            if tt % 2 == 0:
                for u in job_units("B", tt // 2,
                                   avd=2 if tt == NQT - 2 else None):
                    u()
                for u in outproj_units(tt):
                    u()
            else:
                for u in outproj_units(tt):
                    u()
    nc.finalize()
    return nc


def _host_inputs(x, wq, bq, wk, bk, wv, bv, wo):
    """Per-core input maps. Slot A of core c = head c; slot B = split head
    8 + c//2 with token parity c%2."""
    bf16 = ml_dtypes.bfloat16
    xT = np.ascontiguousarray(x[0].T).astype(bf16)
    xTB_by_par = [np.ascontiguousarray(x[0][p::2].T).astype(bf16) for p in (0, 1)]

    in_maps = []
    for c in range(N_CORES):
        ha, hb, par = c, 8 + c // 2, c % 2
        w = np.zeros((D_MODEL, 384), np.float32)
        w[:, 0:64] = wq[ha]
        w[:, 64:128] = wk[ha]
        w[:, 128:192] = wk[hb]
        w[:, 192:256] = wv[ha]
        w[:, 256:320] = wv[hb]
        w[:, 320:384] = wq[hb]
        b = np.zeros((128, 3), np.float32)
        b[0:64, 0] = bq[ha]
        b[0:64, 1] = bk[ha]
        b[64:128, 1] = bk[hb]
        b[64:128, 2] = bq[hb]
        wo2 = np.zeros((128, D_MODEL), np.float32)
        wo2[0:64] = wo[ha * 64:(ha + 1) * 64]
        wo2[64:128] = wo[hb * 64:(hb + 1) * 64]
        kl = np.arange(128)[:, None]
        qq = np.arange(QT)[None, :]
        masks = np.zeros((NMASK, 128, QT), np.float32)
        for pat in range(4):
            masks[pat] = (128 * pat + kl) <= qq
        for pat in range(8):
            masks[4 + pat] = (128 * pat + kl) <= (2 * qq + par)
        masks[12, :, :] = (qq % 2 == par)
        in_maps.append({
            "xT": xT,
            "xTB": xTB_by_par[par],
            "wproj": w.astype(bf16),
            "bqk": b.astype(np.float32),
            "wo2": wo2.astype(bf16),
            "masks": masks.astype(bf16),
        })
    return in_maps


def kernel(_trace=False, _tmpdir=None, **inputs):
    x = np.asarray(inputs["x"], np.float32)
    wq = np.asarray(inputs["wq"], np.float32)
    bq = np.asarray(inputs["bq"], np.float32)
    wk = np.asarray(inputs["wk"], np.float32)
    bk = np.asarray(inputs["bk"], np.float32)
    wv = np.asarray(inputs["wv"], np.float32)
    bv = np.asarray(inputs["bv"], np.float32)
    wo = np.asarray(inputs["wo"], np.float32)
    bo = np.asarray(inputs["bo"], np.float32)

    if "nc" not in _PROGRAM_CACHE:
        _PROGRAM_CACHE["nc"] = build_program()
    nc = _PROGRAM_CACHE["nc"]

    in_maps = _host_inputs(x, wq, bq, wk, bk, wv, bv, wo)
    res = run_bass_kernel_spmd(
        nc, in_maps, list(range(N_CORES)), trace=_trace, tmpdir=_tmpdir,
    )
    acc = np.zeros((D_MODEL, T), np.float32)
    for c in range(N_CORES):
        acc += res.results[c]["outT"]
    # V-bias folds to a constant through softmax: + bv_cat @ wo (+ bo)
    const = bv.reshape(-1) @ wo + bo
    out = acc.T + const[None, :]
    if _trace:
        return out[None].astype(np.float32), res
    return out[None].astype(np.float32)


# revision 69
# speedup vs baseline: 1.2304x; 1.0077x over previous
"""Multi-head causal attention (B=1, T=4096, D=768, H=12) on 8 trn2 cores.

Sharding: per core, slot A = one full head (heads 0-7 across the 8 cores);
slot B = half of a split head (heads 8-11, each split across 2 cores by
token PARITY: core 2k gets even tokens of head 8+k, core 2k+1 odd tokens).
Parity-splitting keeps the causal key extents identical across cores, so
every core runs the IDENTICAL program (SPMD); cores differ only in data
(weights, masks, parity).  Slot B's queries are packed (parity-strided
projection); its head outputs are written back to natural token columns
with stride-2 DVE writes, so one merged out-projection covers both slots
and the host just sums the 8 partial [768, 4096] outputs.

Per-core work: slot A = 144 key-chunk units, slot B = 80 units (vs 288 for
the old 2-full-slot scheme).  V-bias is folded into a host-side constant
(P@(V+1 bv^T) = P@V + denom bv^T, exact through softmax normalization).

On-device layout (per core):
  xT    [768, 4096] bf16   x transposed (host supplies); xTB = parity cols
  K_sb  [128, 4096] bf16   rows 0:64 head-A K dims, 64:128 head-B K dims
  Q_sb  [128, 4096] bf16   rows 0:64 head-A Q (natural); rows 64:128 cols
                           0:2048 head-B Q (parity-packed)
  V2    [128, 32*208] bf16 per key-chunk: [V_A 0:64 |1@64| 0 |1@97| 0 |
                           V_B 129:193] -> one matmul per slot yields AV
                           rows + a denominator row (A: row 64, B: row 32)
  scores chunks [128 keys, 512 q] in PSUM, exp'd on ACT -> PT bf16

Scheduling: emission is software-pipelined — projection of token tile
tt+1 and deferred xT loads are woven between the attention score groups
of stage tt, and each group's mask+AV matmuls are delayed 8 groups behind
its exp so PE never stalls on the exp latency.  Softmax normalization
broadcasts 1/denom across partitions with a K=1 ones-matmul on PE (no
DRAM bounce).  Out-projection per 512-query tile -> bf16 partials.
"""

import math
import numpy as np
import ml_dtypes
from contextlib import ExitStack

import concourse.bass as bass
import concourse.bacc as bacc
import concourse.mybir as mybir
import concourse.tile as tile
from concourse.bass_utils import run_bass_kernel_spmd

BF16 = mybir.dt.bfloat16
F32 = mybir.dt.float32
F8 = mybir.dt.float8e4
DR = mybir.MatmulPerfMode.DoubleRow
AF = mybir.ActivationFunctionType

T = 4096
D_MODEL = 768
HEAD_DIM = 64
N_HEADS = 12
N_CORES = 8
QT = 512                  # query tile width (A natural / B packed)
KC = 128                  # key chunk (psum partition dim)
GRP = 2                   # score chunks per exp group (psum banks)
NQT = T // QT             # 8 A-tiles
NPB = 4                   # B packed tiles (each covers 1024 natural tokens)
CCH = D_MODEL // 128      # 6 contraction chunks
VST = 208                 # V2 stride per key chunk
NMASK = 13                # 4 A diag patterns + 8 B patterns + parity col mask

_PROGRAM_CACHE = {}


def build_program():
    nc = bacc.Bacc(None)

    xT_d = nc.declare_dram_parameter("xT", [D_MODEL, T], BF16, isOutput=False)
    # x columns of this core's parity, packed: x[:, parity::2].T
    xTB_d = nc.declare_dram_parameter("xTB", [D_MODEL, T // 2], BF16, isOutput=False)
    # w cols: 0:64 wq_A | 64:192 wk_AB | 192:320 wv_AB | 320:384 wq_B
    w_d = nc.declare_dram_parameter("wproj", [D_MODEL, 384], BF16, isOutput=False)
    b_d = nc.declare_dram_parameter("bqk", [128, 3], F32, isOutput=False)
    wo_d = nc.declare_dram_parameter("wo2", [128, D_MODEL], BF16, isOutput=False)
    mk_d = nc.declare_dram_parameter("masks", [NMASK, 128, QT], BF16, isOutput=False)
    outT_d = nc.declare_dram_parameter("outT", [D_MODEL, T], BF16, isOutput=True)

    with tile.TileContext(nc) as tc, ExitStack() as ctx:
        consts = ctx.enter_context(tc.tile_pool(name="consts", bufs=1))
        big = ctx.enter_context(tc.tile_pool(name="big", bufs=1))
        ptp = ctx.enter_context(tc.tile_pool(name="ptp", bufs=int(__import__("os").environ.get("KPTP", "13"))))
        osb = ctx.enter_context(tc.tile_pool(name="osb", bufs=3))
        rp = ctx.enter_context(tc.tile_pool(name="rp", bufs=2))
        dramp = ctx.enter_context(tc.tile_pool(name="dramp", bufs=2, space="DRAM"))
        # PSUM budget is 8 banks total; knobs for the split
        import os as _os
        fp8 = _os.environ.get("KFP8", "0") == "1"
        grp = int(_os.environ.get("KGRP", str(GRP)))
        _scb = int(_os.environ.get("KSCB", "2"))
        _avp = int(_os.environ.get("KAVP", "2"))
        _opp = int(_os.environ.get("KOPP", "2"))
        scp = ctx.enter_context(tc.tile_pool(name="scp", bufs=_scb, space="PSUM"))
        avp = ctx.enter_context(tc.tile_pool(name="avp", bufs=_avp, space="PSUM"))
        if _opp > 0:
            opp = ctx.enter_context(
                tc.tile_pool(name="opp", bufs=_opp, space="PSUM"))
        else:
            opp = avp

        # ---- inputs to SBUF: small consts first, then xT streamed in
        # token-tile slices so stage-0 projection starts within ~4us ----
        w_sb = consts.tile([128, CCH * 384], BF16, tag="w")
        _wap = w_d[:, :]
        nc.sync.dma_start(
            out=w_sb[:],
            in_=bass.AP(tensor=_wap.tensor, offset=_wap.offset,
                        ap=[[384, 128], [128 * 384, CCH], [1, 384]]))
        b_sb = consts.tile([128, 3], F32, tag="b")
        nc.sync.dma_start(out=b_sb[:], in_=b_d[:, :])
        wo_sb = consts.tile([128, D_MODEL], BF16, tag="wo")
        mask_sb = consts.tile([128, NMASK * QT], BF16, tag="mask")
        # xT sliced per token-tile pair so stage-0/1 projection starts early;
        # first xTB slice interleaved (stage 1 needs it)
        xT_sb = [big.tile([128, T], BF16, tag=f"xT{j}", name=f"xT{j}")
                 for j in range(CCH)]
        xTB_sb = [big.tile([128, T // 2], BF16, tag=f"xTB{j}", name=f"xTB{j}")
                  for j in range(CCH)]

        def xt(j, cs):
            return xT_sb[j][:, cs]

        def xtb(j, cs):
            return xTB_sb[j][:, cs]

        def load_xt1(tt):
            cs = slice(tt * QT, (tt + 1) * QT)
            for j in range(CCH):
                nc.sync.dma_start(out=xT_sb[j][:, cs],
                                  in_=xT_d[j * 128:(j + 1) * 128, cs])

        def load_xt(tp):
            cs = slice(tp * 2 * QT, (tp + 1) * 2 * QT)
            for j in range(CCH):
                nc.sync.dma_start(out=xT_sb[j][:, cs],
                                  in_=xT_d[j * 128:(j + 1) * 128, cs])

        def load_xtb(pb):
            cs = slice(pb * QT, (pb + 1) * QT)
            for j in range(CCH):
                nc.sync.dma_start(out=xTB_sb[j][:, cs],
                                  in_=xTB_d[j * 128:(j + 1) * 128, cs])

        # only the slices stage 0/1 need right away; masks deferred behind
        # them; the rest are woven in as filler
        load_xt1(0)
        load_xt1(1)
        load_xtb(0)
        nc.sync.dma_start(out=wo_sb[:], in_=wo_d[:, :])
        _map = mk_d[:, :, :]
        nc.sync.dma_start(
            out=mask_sb[:],
            in_=bass.AP(tensor=_map.tensor, offset=_map.offset,
                        ap=[[QT, 128], [128 * QT, NMASK], [1, QT]]))

        # ---- persistent tensors ----
        QKDT = F8 if fp8 else BF16
        K_sb = big.tile([128, T], QKDT, tag="K")
        Q_sb = big.tile([128, T], QKDT, tag="Q")
        if fp8:
            # plane layout for DoubleRow: rows 0:32 slot A (head-dim planes
            # 0:32 / 32:64 at byte offsets 0 / T), rows 32:64 slot B
            K8p = big.tile([128, 2 * T], F8, tag="K8p")
            Q8p = big.tile([128, 2 * T], F8, tag="Q8p")
            K8p3 = K8p[:].rearrange("p (two n) -> p two n", two=2)
            Q8p3 = Q8p[:].rearrange("p (two n) -> p two n", two=2)
        V2 = big.tile([128, (T // KC) * VST], BF16, tag="V2")
        ho = big.tile([128, T], BF16, tag="ho")
        nc.gpsimd.memset(ho[:], 0.0)
        nc.gpsimd.memset(V2[:], 0.0)
        v3 = V2[:].rearrange("p (t c) -> p t c", c=VST)
        nc.gpsimd.memset(v3[:, :, 64:65], 1.0)    # ones row for denom_A
        nc.gpsimd.memset(v3[:, :, 97:98], 1.0)    # ones row for denom_B
        ones64 = consts.tile([128, 64], BF16, tag="ones64")
        nc.gpsimd.memset(ones64[:], 1.0)          # lhsT for recip broadcast

        def proj_units(tt):
            """List of thunks emitting projection for token tile tt."""
            ts = slice(tt * QT, (tt + 1) * QT)
            odd = tt % 2 == 1
            pb = (tt - 1) // 2
            st_ = {}
            units = []

            # NOTE: only ONE pending psum accumulation group per 2KB bank —
            # K (bank0) + Q_A (bank1) may interleave, but Q_B (also bank1)
            # and each V sub-group (all in pv bank0) must run after the
            # previous same-bank group has stopped.
            def u_kq(j):
                if j == 0:
                    st_["pk"] = scp.tile([128, grp * QT], F32, tag="sc",
                                         name="pk")
                pk = st_["pk"]
                rhs = xt(j, ts)
                st, sp = j == 0, j == CCH - 1
                nc.tensor.matmul(
                    pk[:, 0:QT], w_sb[:, j * 384 + 64:j * 384 + 192], rhs,
                    start=st, stop=sp, tile_position=(0, 0),
                )
                nc.tensor.matmul(
                    pk[0:64, QT:2 * QT], w_sb[:, j * 384:j * 384 + 64], rhs,
                    start=st, stop=sp, tile_position=(0, 0),
                )

            def u_qb(j):
                pk = st_["pk"]
                nc.tensor.matmul(
                    pk[64:128, QT:2 * QT],
                    w_sb[:, j * 384 + 320:j * 384 + 384],
                    xtb(j, slice(pb * QT, (pb + 1) * QT)),
                    start=(j == 0), stop=(j == CCH - 1), tile_position=(0, 64),
                )

            def u_cast_kqa():
                pk = st_["pk"]
                nc.vector.tensor_scalar_add(K_sb[:, ts], pk[:, 0:QT], b_sb[:, 1:2])
                nc.vector.tensor_scalar_add(
                    Q_sb[0:64, ts], pk[0:64, QT:2 * QT], b_sb[0:64, 0:1])

            def u_regroup_kqa():
                # fp8 plane regroup: flat rows (4 groups of 32) -> plane
                # layout rows 0:32 (A) / 32:64 (B), byte offset 0 / T.
                # SWDGE (gpsimd) path keeps these off the busy HWDGE.
                for src0, dst0, pl in ((0, 0, 0), (32, 0, 1),
                                       (64, 32, 0), (96, 32, 1)):
                    nc.gpsimd.dma_start(
                        out=K8p3[dst0:dst0 + 32, pl:pl + 1, ts],
                        in_=K_sb[src0:src0 + 32, ts])
                for src0, pl in ((0, 0), (32, 1)):
                    nc.gpsimd.dma_start(
                        out=Q8p3[0:32, pl:pl + 1, ts],
                        in_=Q_sb[src0:src0 + 32, ts])

            def u_cast_qb():
                pk = st_["pk"]
                nc.vector.tensor_scalar_add(
                    Q_sb[64:128, pb * QT:(pb + 1) * QT],
                    pk[64:128, QT:2 * QT], b_sb[64:128, 2:3])

            def u_regroup_qb():
                pbs = slice(pb * QT, (pb + 1) * QT)
                for src0, pl in ((64, 0), (96, 1)):
                    nc.gpsimd.dma_start(
                        out=Q8p3[32:64, pl:pl + 1, pbs],
                        in_=Q_sb[src0:src0 + 32, pbs])

            def u_v(sub):
                if sub == 0:
                    st_["pv"] = scp.tile([128, grp * QT], F32, tag="sc",
                                         name="pv")
                pv = st_["pv"]
                kc = tt * 4 + sub
                for j in range(CCH):
                    nc.tensor.matmul(
                        pv[:, sub * 128:(sub + 1) * 128],
                        xt(j, slice(kc * KC, (kc + 1) * KC)),
                        w_sb[:, j * 384 + 192:j * 384 + 320],
                        start=(j == 0), stop=(j == CCH - 1), tile_position=(0, 0),
                    )

            def u_vcopy():
                pv = st_["pv"]
                for sub in range(4):
                    kc = tt * 4 + sub
                    blk = V2[:, kc * VST:kc * VST + 193]
                    out_ap = bass.AP(tensor=blk.tensor, offset=blk.offset,
                                     ap=[list(blk.ap[0]), [129, 2], [1, 64]])
                    nc.vector.tensor_copy(out_ap, pv[:, sub * 128:(sub + 1) * 128])

            for j in range(CCH):
                units.append(lambda j=j: u_kq(j))
            units.append(u_cast_kqa)
            if fp8:
                units.append(u_regroup_kqa)
            if odd:
                for j in range(CCH):
                    units.append(lambda j=j: u_qb(j))
                units.append(u_cast_qb)
                if fp8:
                    units.append(u_regroup_qb)
            for sub in range(4):
                units.append(lambda sub=sub: u_v(sub))
            units.append(u_vcopy)
            return units

        def job_units(slot, i, avd=None):
            """slot 'A': full head, query tile i (natural); slot 'B': split
            head, packed tile i.  Returns list of thunks (one per score
            group + a normalize tail)."""
            if slot == "A":
                nst, band0 = 4 * (i + 1), 4 * i
                krow, tp = 0, (0, 0)
            else:
                nst, band0 = 8 * (i + 1), 8 * i
                krow, tp = 64, (64, 0)
            qrow = slice(krow, krow + 64)
            st_ = {}

            def flush_av(force=False):
                # masks + AV for a group exp'd earlier; the delay keeps PE
                # from stalling on the exp latency
                import os as _os2
                depth = avd if avd is not None else int(_os2.environ.get("KAVD", "12"))
                pend = st_.setdefault("pendq", [])
                if not pend or (not force and len(pend) <= depth - 1):
                    return
                pt, g0, w = pend.pop(0)
                av = st_["av"]
                for gi in range(w):
                    kc = g0 + gi
                    # valid-query truncation: for diagonal-band chunks,
                    # queries below qoff are entirely masked-out, so the
                    # mask mul and AV matmul (incl. denom row) skip them —
                    # exact, since those queries don't attend these keys.
                    qoff = 0
                    if kc >= band0:
                        pat = kc - band0
                        mi = pat if slot == "A" else 4 + pat
                        qoff = (128 if slot == "A" else 64) * pat
                        nc.vector.tensor_mul(
                            pt[:, gi * QT + qoff:(gi + 1) * QT],
                            pt[:, gi * QT + qoff:(gi + 1) * QT],
                            mask_sb[:, mi * QT + qoff:(mi + 1) * QT])
                    ptj = pt[:, gi * QT + qoff:(gi + 1) * QT]
                    st, sp = kc == 0, kc == nst - 1
                    if slot == "A":
                        nc.tensor.matmul(
                            av[0:65, qoff:QT], V2[:, kc * VST:kc * VST + 65],
                            ptj, start=st, stop=sp, tile_position=(0, 0),
                        )
                    else:
                        nc.tensor.matmul(
                            av[0:128, qoff:QT],
                            V2[:, kc * VST + 65:kc * VST + 193],
                            ptj, start=st, stop=sp, tile_position=(0, 0),
                        )

            def _qoff(kc):
                # valid-query start for diagonal-band chunks: queries below
                # this are strictly non-causal for every key in the chunk
                if kc < band0:
                    return 0
                return (128 if slot == "A" else 64) * (kc - band0)

            def u_group(g0):
                if g0 == 0:
                    st_["av"] = avp.tile([128, QT], F32, tag="av", name="av")
                w = min(grp, nst - g0)
                sc = scp.tile([128, grp * QT], F32, tag="sc", name="sc")
                qoffs = [_qoff(g0 + gi) for gi in range(w)]
                for gi in range(w):
                    kc = g0 + gi
                    qo = qoffs[gi]
                    if fp8:
                        prow = 0 if slot == "A" else 32
                        nc.tensor.matmul(
                            sc[:, gi * QT + qo:(gi + 1) * QT],
                            K8p3[prow:prow + 32, :, kc * KC:(kc + 1) * KC],
                            Q8p3[prow:prow + 32, :, i * QT + qo:(i + 1) * QT],
                            start=True, stop=True, perf_mode=DR,
                            tile_position=(prow, 0),
                        )
                    else:
                        nc.tensor.matmul(
                            sc[:, gi * QT + qo:(gi + 1) * QT],
                            K_sb[krow:krow + 64, kc * KC:(kc + 1) * KC],
                            Q_sb[qrow, i * QT + qo:(i + 1) * QT],
                            start=True, stop=True, tile_position=tp,
                        )
                pt = ptp.tile([128, grp * QT], BF16, tag="pt", name="pt")
                if any(qoffs):
                    # ragged group: exp per chunk over its valid sub-range
                    # (reads only psum the truncated score matmul wrote)
                    for gi in range(w):
                        qo = qoffs[gi]
                        nc.scalar.activation(
                            pt[:, gi * QT + qo:(gi + 1) * QT],
                            sc[:, gi * QT + qo:(gi + 1) * QT], AF.Exp,
                            scale=1.0 / math.sqrt(HEAD_DIM))
                else:
                    nc.scalar.activation(
                        pt[:, 0:w * QT], sc[:, 0:w * QT], AF.Exp,
                        scale=1.0 / math.sqrt(HEAD_DIM))
                st_.setdefault("pendq", []).append((pt, g0, w))
                flush_av()

            def u_norm():
                while st_.get("pendq"):
                    flush_av(force=True)
                # normalize: bf16 reciprocal of the denom row, broadcast
                # across 64 partitions via a K=1 ones-matmul on PE (no DMA)
                av = st_["av"]
                drow = 64 if slot == "A" else 32
                rows = slice(0, 64) if slot == "A" else slice(64, 128)
                r = rp.tile([128, QT], BF16, tag="r", name="r")
                with nc.allow_low_precision(reason="softmax denom recip bf16"):
                    nc.vector.reciprocal(r[drow:drow + 1, :], av[drow:drow + 1, :])
                rbc_ps = avp.tile([128, QT], F32, tag="av", name="rbc_ps")
                nc.tensor.matmul(
                    rbc_ps[rows, :], ones64[drow:drow + 1, :], r[drow:drow + 1, :],
                    start=True, stop=True, tile_position=(drow, rows.start),
                )
                rbc = rp.tile([128, QT], F32, tag="rbc", name="rbc")
                nc.vector.tensor_copy(rbc[rows, :], rbc_ps[rows, :])
                if slot == "A":
                    nc.vector.tensor_mul(
                        ho[0:64, i * QT:(i + 1) * QT], av[0:64, :], rbc[0:64, :])
                else:
                    # write packed value j to BOTH natural columns 2j, 2j+1;
                    # the per-core parity column mask (data) zeroes the
                    # wrong one right before the out-projection.
                    hob = ho[64:128, 1024 * i:1024 * (i + 1)].rearrange(
                        "p (n two) -> p two n", two=2)
                    nc.vector.tensor_mul(hob[:, 0:1, :], av[64:128, :],
                                         rbc[64:128, :])
                    nc.vector.tensor_mul(hob[:, 1:2, :], av[64:128, :],
                                         rbc[64:128, :])

            units = [lambda g0=g0: u_group(g0) for g0 in range(0, nst, grp)]
            units.append(u_norm)
            return units

        def outproj_units(qt):
            qs = slice(qt * QT, (qt + 1) * QT)

            def u_pmask():
                nc.vector.tensor_mul(
                    ho[64:128, qs], ho[64:128, qs],
                    mask_sb[64:128, 12 * QT:13 * QT])

            def u_op(dch):
                op = opp.tile([128, QT], F32, tag="av" if opp is avp else "op", name="op")
                nc.tensor.matmul(
                    op[:], wo_sb[:, dch * 128:(dch + 1) * 128],
                    ho[:, qs], start=True, stop=True, tile_position=(0, 0),
                )
                ot = osb.tile([128, QT], BF16, tag="ot", name="ot")
                if qt >= 6 or dch % 2 == 1:
                    nc.scalar.copy(ot[:], op[:])
                else:
                    nc.vector.tensor_copy(ot[:], op[:])
                nc.sync.dma_start(
                    out=outT_d[dch * 128:(dch + 1) * 128, qs], in_=ot[:])

            return [u_pmask] + [lambda d=d: u_op(d) for d in range(CCH)]

        def weave(main, filler):
            """Emit `main` units with `filler` units distributed evenly."""
            if not main:
                for f in filler:
                    f()
                return
            nf, nm = len(filler), len(main)
            fi = 0
            for k, u in enumerate(main):
                u()
                while fi * nm < (k + 1) * nf:
                    filler[fi]()
                    fi += 1
            while fi < nf:
                filler[fi]()
                fi += 1

        # ---- software-pipelined emission: proj(tt+1) + deferred input
        # loads woven into jobs(tt) --
        for u in proj_units(0):
            u()
        for tt in range(NQT):
            # A_tt woven with proj(tt+1); then (even tt) B_{tt//2} — safe
            # only after ALL proj(tt+1) units, since its diagonal-band
            # chunks read K/V of tile tt+1
            stream = job_units("A", tt, avd=2 if tt == NQT - 1 else None)
            filler = []
            if tt in (0, 2, 4):
                k = tt // 2 + 1
                filler.append(lambda k=k: load_xt(k))
                filler.append(lambda k=k: load_xtb(k))
            filler += proj_units(tt + 1) if tt + 1 < NQT else []
            weave(stream, filler)
            if tt % 2 == 0:
                for u in job_units("B", tt // 2,
                                   avd=2 if tt == NQT - 2 else None):
                    u()
                for u in outproj_units(tt):
                    u()
            else:
                for u in outproj_units(tt):
                    u()
    nc.finalize()
    return nc


def _host_inputs(x, wq, bq, wk, bk, wv, bv, wo):
    """Per-core input maps. Slot A of core c = head c; slot B = split head
    8 + c//2 with token parity c%2."""
    bf16 = ml_dtypes.bfloat16
    xT = np.ascontiguousarray(x[0].T).astype(bf16)
    xTB_by_par = [np.ascontiguousarray(x[0][p::2].T).astype(bf16) for p in (0, 1)]

    in_maps = []
    for c in range(N_CORES):
        ha, hb, par = c, 8 + c // 2, c % 2
        w = np.zeros((D_MODEL, 384), np.float32)
        w[:, 0:64] = wq[ha]
        w[:, 64:128] = wk[ha]
        w[:, 128:192] = wk[hb]
        w[:, 192:256] = wv[ha]
        w[:, 256:320] = wv[hb]
        w[:, 320:384] = wq[hb]
        b = np.zeros((128, 3), np.float32)
        b[0:64, 0] = bq[ha]
        b[0:64, 1] = bk[ha]
        b[64:128, 1] = bk[hb]
        b[64:128, 2] = bq[hb]
        wo2 = np.zeros((128, D_MODEL), np.float32)
        wo2[0:64] = wo[ha * 64:(ha + 1) * 64]
        wo2[64:128] = wo[hb * 64:(hb + 1) * 64]
        kl = np.arange(128)[:, None]
        qq = np.arange(QT)[None, :]
        masks = np.zeros((NMASK, 128, QT), np.float32)
        for pat in range(4):
            masks[pat] = (128 * pat + kl) <= qq
        for pat in range(8):
            masks[4 + pat] = (128 * pat + kl) <= (2 * qq + par)
        masks[12, :, :] = (qq % 2 == par)
        in_maps.append({
            "xT": xT,
            "xTB": xTB_by_par[par],
            "wproj": w.astype(bf16),
            "bqk": b.astype(np.float32),
            "wo2": wo2.astype(bf16),
            "masks": masks.astype(bf16),
        })
    return in_maps


def kernel(_trace=False, _tmpdir=None, **inputs):
    x = np.asarray(inputs["x"], np.float32)
    wq = np.asarray(inputs["wq"], np.float32)
    bq = np.asarray(inputs["bq"], np.float32)
    wk = np.asarray(inputs["wk"], np.float32)
    bk = np.asarray(inputs["bk"], np.float32)
    wv = np.asarray(inputs["wv"], np.float32)
    bv = np.asarray(inputs["bv"], np.float32)
    wo = np.asarray(inputs["wo"], np.float32)
    bo = np.asarray(inputs["bo"], np.float32)

    if "nc" not in _PROGRAM_CACHE:
        _PROGRAM_CACHE["nc"] = build_program()
    nc = _PROGRAM_CACHE["nc"]

    in_maps = _host_inputs(x, wq, bq, wk, bk, wv, bv, wo)
    res = run_bass_kernel_spmd(
        nc, in_maps, list(range(N_CORES)), trace=_trace, tmpdir=_tmpdir,
    )
    acc = np.zeros((D_MODEL, T), np.float32)
    for c in range(N_CORES):
        acc += res.results[c]["outT"]
    # V-bias folds to a constant through softmax: + bv_cat @ wo (+ bo)
    const = bv.reshape(-1) @ wo + bo
    out = acc.T + const[None, :]
    if _trace:
        return out[None].astype(np.float32), res
    return out[None].astype(np.float32)


# revision 71
# speedup vs baseline: 1.2504x; 1.0163x over previous
"""Multi-head causal attention (B=1, T=4096, D=768, H=12) on 8 trn2 cores.

Sharding: per core, slot A = one full head (heads 0-7 across the 8 cores);
slot B = half of a split head (heads 8-11, each split across 2 cores by
token PARITY: core 2k gets even tokens of head 8+k, core 2k+1 odd tokens).
Parity-splitting keeps the causal key extents identical across cores, so
every core runs the IDENTICAL program (SPMD); cores differ only in data
(weights, masks, parity).  Slot B's queries are packed (parity-strided
projection); its head outputs are written back to natural token columns
with stride-2 DVE writes, so one merged out-projection covers both slots
and the host just sums the 8 partial [768, 4096] outputs.

Per-core work: slot A = 144 key-chunk units, slot B = 80 units (vs 288 for
the old 2-full-slot scheme).  V-bias is folded into a host-side constant
(P@(V+1 bv^T) = P@V + denom bv^T, exact through softmax normalization).

On-device layout (per core):
  xT    [768, 4096] bf16   x transposed (host supplies); xTB = parity cols
  K_sb  [128, 4096] bf16   rows 0:64 head-A K dims, 64:128 head-B K dims
  Q_sb  [128, 4096] bf16   rows 0:64 head-A Q (natural); rows 64:128 cols
                           0:2048 head-B Q (parity-packed)
  V2    [128, 32*208] bf16 per key-chunk: [V_A 0:64 |1@64| 0 |1@97| 0 |
                           V_B 129:193] -> one matmul per slot yields AV
                           rows + a denominator row (A: row 64, B: row 32)
  scores chunks [128 keys, 512 q] in PSUM, exp'd on ACT -> PT bf16

Scheduling: emission is software-pipelined — projection of token tile
tt+1 and deferred xT loads are woven between the attention score groups
of stage tt, and each group's mask+AV matmuls are delayed 8 groups behind
its exp so PE never stalls on the exp latency.  Softmax normalization
broadcasts 1/denom across partitions with a K=1 ones-matmul on PE (no
DRAM bounce).  Out-projection per 512-query tile -> bf16 partials.
"""

import math
import numpy as np
import ml_dtypes
from contextlib import ExitStack

import concourse.bass as bass
import concourse.bacc as bacc
import concourse.mybir as mybir
import concourse.tile as tile
from concourse.bass_utils import run_bass_kernel_spmd

BF16 = mybir.dt.bfloat16
F32 = mybir.dt.float32
F8 = mybir.dt.float8e4
DR = mybir.MatmulPerfMode.DoubleRow
AF = mybir.ActivationFunctionType

T = 4096
D_MODEL = 768
HEAD_DIM = 64
N_HEADS = 12
N_CORES = 8
QT = 512                  # query tile width (A natural / B packed)
KC = 128                  # key chunk (psum partition dim)
GRP = 2                   # score chunks per exp group (psum banks)
NQT = T // QT             # 8 A-tiles
NPB = 4                   # B packed tiles (each covers 1024 natural tokens)
CCH = D_MODEL // 128      # 6 contraction chunks
VST = 208                 # V2 stride per key chunk
NMASK = 13                # 4 A diag patterns + 8 B patterns + parity col mask

_PROGRAM_CACHE = {}


def build_program():
    nc = bacc.Bacc(None)

    xT_d = nc.declare_dram_parameter("xT", [D_MODEL, T], BF16, isOutput=False)
    # x columns of this core's parity, packed: x[:, parity::2].T
    xTB_d = nc.declare_dram_parameter("xTB", [D_MODEL, T // 2], BF16, isOutput=False)
    # w cols: 0:64 wq_A | 64:192 wk_AB | 192:320 wv_AB | 320:384 wq_B
    w_d = nc.declare_dram_parameter("wproj", [D_MODEL, 384], BF16, isOutput=False)
    b_d = nc.declare_dram_parameter("bqk", [128, 3], F32, isOutput=False)
    wo_d = nc.declare_dram_parameter("wo2", [128, D_MODEL], BF16, isOutput=False)
    mk_d = nc.declare_dram_parameter("masks", [NMASK, 128, QT], BF16, isOutput=False)
    outT_d = nc.declare_dram_parameter("outT", [D_MODEL, T], BF16, isOutput=True)

    with tile.TileContext(nc) as tc, ExitStack() as ctx:
        consts = ctx.enter_context(tc.tile_pool(name="consts", bufs=1))
        big = ctx.enter_context(tc.tile_pool(name="big", bufs=1))
        ptp = ctx.enter_context(tc.tile_pool(name="ptp", bufs=int(__import__("os").environ.get("KPTP", "13"))))
        osb = ctx.enter_context(tc.tile_pool(name="osb", bufs=int(__import__("os").environ.get("KOSB", "6"))))
        rp = ctx.enter_context(tc.tile_pool(name="rp", bufs=int(__import__("os").environ.get("KRP", "2"))))
        dramp = ctx.enter_context(tc.tile_pool(name="dramp", bufs=2, space="DRAM"))
        # PSUM budget is 8 banks total; knobs for the split
        import os as _os
        fp8 = _os.environ.get("KFP8", "0") == "1"
        grp = int(_os.environ.get("KGRP", str(GRP)))
        _scb = int(_os.environ.get("KSCB", "2"))
        _avp = int(_os.environ.get("KAVP", "2"))
        _opp = int(_os.environ.get("KOPP", "2"))
        scp = ctx.enter_context(tc.tile_pool(name="scp", bufs=_scb, space="PSUM"))
        avp = ctx.enter_context(tc.tile_pool(name="avp", bufs=_avp, space="PSUM"))
        if _opp > 0:
            opp = ctx.enter_context(
                tc.tile_pool(name="opp", bufs=_opp, space="PSUM"))
        else:
            opp = avp

        # ---- inputs to SBUF: small consts first, then xT streamed in
        # token-tile slices so stage-0 projection starts within ~4us ----
        w_sb = consts.tile([128, CCH * 384], BF16, tag="w")
        _wap = w_d[:, :]
        nc.sync.dma_start(
            out=w_sb[:],
            in_=bass.AP(tensor=_wap.tensor, offset=_wap.offset,
                        ap=[[384, 128], [128 * 384, CCH], [1, 384]]))
        b_sb = consts.tile([128, 3], F32, tag="b")
        nc.sync.dma_start(out=b_sb[:], in_=b_d[:, :])
        wo_sb = consts.tile([128, D_MODEL], BF16, tag="wo")
        mask_sb = consts.tile([128, NMASK * QT], BF16, tag="mask")
        # xT sliced per token-tile pair so stage-0/1 projection starts early;
        # first xTB slice interleaved (stage 1 needs it)
        xT_sb = [big.tile([128, T], BF16, tag=f"xT{j}", name=f"xT{j}")
                 for j in range(CCH)]
        xTB_sb = [big.tile([128, T // 2], BF16, tag=f"xTB{j}", name=f"xTB{j}")
                  for j in range(CCH)]

        def xt(j, cs):
            return xT_sb[j][:, cs]

        def xtb(j, cs):
            return xTB_sb[j][:, cs]

        def load_xt1(tt):
            cs = slice(tt * QT, (tt + 1) * QT)
            for j in range(CCH):
                nc.sync.dma_start(out=xT_sb[j][:, cs],
                                  in_=xT_d[j * 128:(j + 1) * 128, cs])

        def load_xt(tp):
            cs = slice(tp * 2 * QT, (tp + 1) * 2 * QT)
            for j in range(CCH):
                nc.sync.dma_start(out=xT_sb[j][:, cs],
                                  in_=xT_d[j * 128:(j + 1) * 128, cs])

        def load_xtb(pb):
            cs = slice(pb * QT, (pb + 1) * QT)
            for j in range(CCH):
                nc.sync.dma_start(out=xTB_sb[j][:, cs],
                                  in_=xTB_d[j * 128:(j + 1) * 128, cs])

        # only the slices stage 0/1 need right away; masks deferred behind
        # them; the rest are woven in as filler
        load_xt1(0)
        load_xt1(1)
        load_xtb(0)
        nc.sync.dma_start(out=wo_sb[:], in_=wo_d[:, :])
        _map = mk_d[:, :, :]
        nc.sync.dma_start(
            out=mask_sb[:],
            in_=bass.AP(tensor=_map.tensor, offset=_map.offset,
                        ap=[[QT, 128], [128 * QT, NMASK], [1, QT]]))

        # ---- persistent tensors ----
        QKDT = F8 if fp8 else BF16
        K_sb = big.tile([128, T], QKDT, tag="K")
        Q_sb = big.tile([128, T], QKDT, tag="Q")
        if fp8:
            # plane layout for DoubleRow: rows 0:32 slot A (head-dim planes
            # 0:32 / 32:64 at byte offsets 0 / T), rows 32:64 slot B
            K8p = big.tile([128, 2 * T], F8, tag="K8p")
            Q8p = big.tile([128, 2 * T], F8, tag="Q8p")
            K8p3 = K8p[:].rearrange("p (two n) -> p two n", two=2)
            Q8p3 = Q8p[:].rearrange("p (two n) -> p two n", two=2)
        V2 = big.tile([128, (T // KC) * VST], BF16, tag="V2")
        ho = big.tile([128, T], BF16, tag="ho")
        nc.gpsimd.memset(ho[:], 0.0)
        nc.gpsimd.memset(V2[:], 0.0)
        v3 = V2[:].rearrange("p (t c) -> p t c", c=VST)
        nc.gpsimd.memset(v3[:, :, 64:65], 1.0)    # ones row for denom_A
        nc.gpsimd.memset(v3[:, :, 97:98], 1.0)    # ones row for denom_B
        ones64 = consts.tile([128, 64], BF16, tag="ones64")
        nc.gpsimd.memset(ones64[:], 1.0)          # lhsT for recip broadcast

        def proj_units(tt):
            """List of thunks emitting projection for token tile tt."""
            ts = slice(tt * QT, (tt + 1) * QT)
            odd = tt % 2 == 1
            pb = (tt - 1) // 2
            st_ = {}
            units = []

            # NOTE: only ONE pending psum accumulation group per 2KB bank —
            # K (bank0) + Q_A (bank1) may interleave, but Q_B (also bank1)
            # and each V sub-group (all in pv bank0) must run after the
            # previous same-bank group has stopped.
            def u_kq(j):
                if j == 0:
                    st_["pk"] = scp.tile([128, grp * QT], F32, tag="sc",
                                         name="pk")
                pk = st_["pk"]
                rhs = xt(j, ts)
                st, sp = j == 0, j == CCH - 1
                nc.tensor.matmul(
                    pk[:, 0:QT], w_sb[:, j * 384 + 64:j * 384 + 192], rhs,
                    start=st, stop=sp, tile_position=(0, 0),
                )
                nc.tensor.matmul(
                    pk[0:64, QT:2 * QT], w_sb[:, j * 384:j * 384 + 64], rhs,
                    start=st, stop=sp, tile_position=(0, 0),
                )

            def u_qb(j):
                pk = st_["pk"]
                nc.tensor.matmul(
                    pk[64:128, QT:2 * QT],
                    w_sb[:, j * 384 + 320:j * 384 + 384],
                    xtb(j, slice(pb * QT, (pb + 1) * QT)),
                    start=(j == 0), stop=(j == CCH - 1), tile_position=(0, 64),
                )

            def u_cast_kqa():
                pk = st_["pk"]
                nc.vector.tensor_scalar_add(K_sb[:, ts], pk[:, 0:QT], b_sb[:, 1:2])
                nc.vector.tensor_scalar_add(
                    Q_sb[0:64, ts], pk[0:64, QT:2 * QT], b_sb[0:64, 0:1])

            def u_regroup_kqa():
                # fp8 plane regroup: flat rows (4 groups of 32) -> plane
                # layout rows 0:32 (A) / 32:64 (B), byte offset 0 / T.
                # SWDGE (gpsimd) path keeps these off the busy HWDGE.
                for src0, dst0, pl in ((0, 0, 0), (32, 0, 1),
                                       (64, 32, 0), (96, 32, 1)):
                    nc.gpsimd.dma_start(
                        out=K8p3[dst0:dst0 + 32, pl:pl + 1, ts],
                        in_=K_sb[src0:src0 + 32, ts])
                for src0, pl in ((0, 0), (32, 1)):
                    nc.gpsimd.dma_start(
                        out=Q8p3[0:32, pl:pl + 1, ts],
                        in_=Q_sb[src0:src0 + 32, ts])

            def u_cast_qb():
                pk = st_["pk"]
                nc.vector.tensor_scalar_add(
                    Q_sb[64:128, pb * QT:(pb + 1) * QT],
                    pk[64:128, QT:2 * QT], b_sb[64:128, 2:3])

            def u_regroup_qb():
                pbs = slice(pb * QT, (pb + 1) * QT)
                for src0, pl in ((64, 0), (96, 1)):
                    nc.gpsimd.dma_start(
                        out=Q8p3[32:64, pl:pl + 1, pbs],
                        in_=Q_sb[src0:src0 + 32, pbs])

            def u_v(sub):
                if sub == 0:
                    st_["pv"] = scp.tile([128, grp * QT], F32, tag="sc",
                                         name="pv")
                pv = st_["pv"]
                kc = tt * 4 + sub
                for j in range(CCH):
                    nc.tensor.matmul(
                        pv[:, sub * 128:(sub + 1) * 128],
                        xt(j, slice(kc * KC, (kc + 1) * KC)),
                        w_sb[:, j * 384 + 192:j * 384 + 320],
                        start=(j == 0), stop=(j == CCH - 1), tile_position=(0, 0),
                    )

            def u_vcopy():
                pv = st_["pv"]
                for sub in range(4):
                    kc = tt * 4 + sub
                    blk = V2[:, kc * VST:kc * VST + 193]
                    out_ap = bass.AP(tensor=blk.tensor, offset=blk.offset,
                                     ap=[list(blk.ap[0]), [129, 2], [1, 64]])
                    nc.vector.tensor_copy(out_ap, pv[:, sub * 128:(sub + 1) * 128])

            for j in range(CCH):
                units.append(lambda j=j: u_kq(j))
            units.append(u_cast_kqa)
            if fp8:
                units.append(u_regroup_kqa)
            if odd:
                for j in range(CCH):
                    units.append(lambda j=j: u_qb(j))
                units.append(u_cast_qb)
                if fp8:
                    units.append(u_regroup_qb)
            for sub in range(4):
                units.append(lambda sub=sub: u_v(sub))
            units.append(u_vcopy)
            return units

        def job_units(slot, i, avd=None):
            """slot 'A': full head, query tile i (natural); slot 'B': split
            head, packed tile i.  Returns list of thunks (one per score
            group + a normalize tail)."""
            if slot == "A":
                nst, band0 = 4 * (i + 1), 4 * i
                krow, tp = 0, (0, 0)
            else:
                nst, band0 = 8 * (i + 1), 8 * i
                krow, tp = 64, (64, 0)
            qrow = slice(krow, krow + 64)
            st_ = {}

            def flush_av(force=False):
                # masks + AV for a group exp'd earlier; the delay keeps PE
                # from stalling on the exp latency
                import os as _os2
                depth = avd if avd is not None else int(_os2.environ.get("KAVD", "12"))
                pend = st_.setdefault("pendq", [])
                if not pend or (not force and len(pend) <= depth - 1):
                    return
                pt, g0, w = pend.pop(0)
                av = st_["av"]
                for gi in range(w):
                    kc = g0 + gi
                    # valid-query truncation: for diagonal-band chunks,
                    # queries below qoff are entirely masked-out, so the
                    # mask mul and AV matmul (incl. denom row) skip them —
                    # exact, since those queries don't attend these keys.
                    qoff = 0
                    if kc >= band0:
                        pat = kc - band0
                        mi = pat if slot == "A" else 4 + pat
                        qoff = (128 if slot == "A" else 64) * pat
                        nc.vector.tensor_mul(
                            pt[:, gi * QT + qoff:(gi + 1) * QT],
                            pt[:, gi * QT + qoff:(gi + 1) * QT],
                            mask_sb[:, mi * QT + qoff:(mi + 1) * QT])
                    ptj = pt[:, gi * QT + qoff:(gi + 1) * QT]
                    st, sp = kc == 0, kc == nst - 1
                    if slot == "A":
                        nc.tensor.matmul(
                            av[0:65, qoff:QT], V2[:, kc * VST:kc * VST + 65],
                            ptj, start=st, stop=sp, tile_position=(0, 0),
                        )
                    else:
                        nc.tensor.matmul(
                            av[0:128, qoff:QT],
                            V2[:, kc * VST + 65:kc * VST + 193],
                            ptj, start=st, stop=sp, tile_position=(0, 0),
                        )

            def _qoff(kc):
                # valid-query start for diagonal-band chunks: queries below
                # this are strictly non-causal for every key in the chunk
                if kc < band0:
                    return 0
                return (128 if slot == "A" else 64) * (kc - band0)

            def u_group(g0):
                if g0 == 0:
                    st_["av"] = avp.tile([128, QT], F32, tag="av", name="av")
                w = min(grp, nst - g0)
                sc = scp.tile([128, grp * QT], F32, tag="sc", name="sc")
                qoffs = [_qoff(g0 + gi) for gi in range(w)]
                for gi in range(w):
                    kc = g0 + gi
                    qo = qoffs[gi]
                    if fp8:
                        prow = 0 if slot == "A" else 32
                        nc.tensor.matmul(
                            sc[:, gi * QT + qo:(gi + 1) * QT],
                            K8p3[prow:prow + 32, :, kc * KC:(kc + 1) * KC],
                            Q8p3[prow:prow + 32, :, i * QT + qo:(i + 1) * QT],
                            start=True, stop=True, perf_mode=DR,
                            tile_position=(prow, 0),
                        )
                    else:
                        nc.tensor.matmul(
                            sc[:, gi * QT + qo:(gi + 1) * QT],
                            K_sb[krow:krow + 64, kc * KC:(kc + 1) * KC],
                            Q_sb[qrow, i * QT + qo:(i + 1) * QT],
                            start=True, stop=True, tile_position=tp,
                        )
                pt = ptp.tile([128, grp * QT], BF16, tag="pt", name="pt")
                if any(qoffs):
                    # ragged group: exp per chunk over its valid sub-range
                    # (reads only psum the truncated score matmul wrote)
                    for gi in range(w):
                        qo = qoffs[gi]
                        nc.scalar.activation(
                            pt[:, gi * QT + qo:(gi + 1) * QT],
                            sc[:, gi * QT + qo:(gi + 1) * QT], AF.Exp,
                            scale=1.0 / math.sqrt(HEAD_DIM))
                else:
                    nc.scalar.activation(
                        pt[:, 0:w * QT], sc[:, 0:w * QT], AF.Exp,
                        scale=1.0 / math.sqrt(HEAD_DIM))
                st_.setdefault("pendq", []).append((pt, g0, w))
                flush_av()

            def u_norm():
                while st_.get("pendq"):
                    flush_av(force=True)
                # normalize: bf16 reciprocal of the denom row, broadcast
                # across 64 partitions via a K=1 ones-matmul on PE (no DMA)
                av = st_["av"]
                drow = 64 if slot == "A" else 32
                rows = slice(0, 64) if slot == "A" else slice(64, 128)
                r = rp.tile([128, QT], BF16, tag="r", name="r")
                with nc.allow_low_precision(reason="softmax denom recip bf16"):
                    nc.vector.reciprocal(r[drow:drow + 1, :], av[drow:drow + 1, :])
                rbc_ps = avp.tile([128, QT], F32, tag="av", name="rbc_ps")
                nc.tensor.matmul(
                    rbc_ps[rows, :], ones64[drow:drow + 1, :], r[drow:drow + 1, :],
                    start=True, stop=True, tile_position=(drow, rows.start),
                )
                rbc = rp.tile([128, QT], F32, tag="rbc", name="rbc")
                nc.vector.tensor_copy(rbc[rows, :], rbc_ps[rows, :])
                if slot == "A":
                    nc.vector.tensor_mul(
                        ho[0:64, i * QT:(i + 1) * QT], av[0:64, :], rbc[0:64, :])
                else:
                    # write packed value j to BOTH natural columns 2j, 2j+1;
                    # the per-core parity column mask (data) zeroes the
                    # wrong one right before the out-projection.
                    hob = ho[64:128, 1024 * i:1024 * (i + 1)].rearrange(
                        "p (n two) -> p two n", two=2)
                    nc.vector.tensor_mul(hob[:, 0:1, :], av[64:128, :],
                                         rbc[64:128, :])
                    nc.vector.tensor_mul(hob[:, 1:2, :], av[64:128, :],
                                         rbc[64:128, :])

            units = [lambda g0=g0: u_group(g0) for g0 in range(0, nst, grp)]
            units.append(u_norm)
            return units

        def outproj_units(qt):
            qs = slice(qt * QT, (qt + 1) * QT)

            def u_pmask():
                nc.vector.tensor_mul(
                    ho[64:128, qs], ho[64:128, qs],
                    mask_sb[64:128, 12 * QT:13 * QT])

            def u_op(dch):
                op = opp.tile([128, QT], F32, tag="av" if opp is avp else "op", name="op")
                nc.tensor.matmul(
                    op[:], wo_sb[:, dch * 128:(dch + 1) * 128],
                    ho[:, qs], start=True, stop=True, tile_position=(0, 0),
                )
                ot = osb.tile([128, QT], BF16, tag="ot", name="ot")
                if qt >= 6 or dch % 2 == 1:
                    nc.scalar.copy(ot[:], op[:])
                else:
                    nc.vector.tensor_copy(ot[:], op[:])
                nc.sync.dma_start(
                    out=outT_d[dch * 128:(dch + 1) * 128, qs], in_=ot[:])

            return [u_pmask] + [lambda d=d: u_op(d) for d in range(CCH)]

        def weave(main, filler):
            """Emit `main` units with `filler` units distributed evenly."""
            if not main:
                for f in filler:
                    f()
                return
            nf, nm = len(filler), len(main)
            fi = 0
            for k, u in enumerate(main):
                u()
                while fi * nm < (k + 1) * nf:
                    filler[fi]()
                    fi += 1
            while fi < nf:
                filler[fi]()
                fi += 1

        # ---- software-pipelined emission: proj(tt+1) + deferred input
        # loads woven into jobs(tt) --
        for u in proj_units(0):
            u()
        for tt in range(NQT):
            # A_tt woven with proj(tt+1); then (even tt) B_{tt//2} — safe
            # only after ALL proj(tt+1) units, since its diagonal-band
            # chunks read K/V of tile tt+1
            stream = job_units("A", tt, avd=2 if tt == NQT - 1 else None)
            filler = []
            if tt in (0, 2, 4):
                k = tt // 2 + 1
                filler.append(lambda k=k: load_xt(k))
                filler.append(lambda k=k: load_xtb(k))
            filler += proj_units(tt + 1) if tt + 1 < NQT else []
            weave(stream, filler)
            if tt % 2 == 0:
                for u in job_units("B", tt // 2,
                                   avd=2 if tt == NQT - 2 else None):
                    u()
                for u in outproj_units(tt):
                    u()
            else:
                for u in outproj_units(tt):
                    u()
    nc.finalize()
    return nc


def _host_inputs(x, wq, bq, wk, bk, wv, bv, wo):
    """Per-core input maps. Slot A of core c = head c; slot B = split head
    8 + c//2 with token parity c%2."""
    bf16 = ml_dtypes.bfloat16
    xT = np.ascontiguousarray(x[0].T).astype(bf16)
    xTB_by_par = [np.ascontiguousarray(x[0][p::2].T).astype(bf16) for p in (0, 1)]

    in_maps = []
    for c in range(N_CORES):
        ha, hb, par = c, 8 + c // 2, c % 2
        w = np.zeros((D_MODEL, 384), np.float32)
        w[:, 0:64] = wq[ha]
        w[:, 64:128] = wk[ha]
        w[:, 128:192] = wk[hb]
        w[:, 192:256] = wv[ha]
        w[:, 256:320] = wv[hb]
        w[:, 320:384] = wq[hb]
        b = np.zeros((128, 3), np.float32)
        b[0:64, 0] = bq[ha]
        b[0:64, 1] = bk[ha]
        b[64:128, 1] = bk[hb]
        b[64:128, 2] = bq[hb]
        wo2 = np.zeros((128, D_MODEL), np.float32)
        wo2[0:64] = wo[ha * 64:(ha + 1) * 64]
        wo2[64:128] = wo[hb * 64:(hb + 1) * 64]
        kl = np.arange(128)[:, None]
        qq = np.arange(QT)[None, :]
        masks = np.zeros((NMASK, 128, QT), np.float32)
        for pat in range(4):
            masks[pat] = (128 * pat + kl) <= qq
        for pat in range(8):
            masks[4 + pat] = (128 * pat + kl) <= (2 * qq + par)
        masks[12, :, :] = (qq % 2 == par)
        in_maps.append({
            "xT": xT,
            "xTB": xTB_by_par[par],
            "wproj": w.astype(bf16),
            "bqk": b.astype(np.float32),
            "wo2": wo2.astype(bf16),
            "masks": masks.astype(bf16),
        })
    return in_maps


def kernel(_trace=False, _tmpdir=None, **inputs):
    x = np.asarray(inputs["x"], np.float32)
    wq = np.asarray(inputs["wq"], np.float32)
    bq = np.asarray(inputs["bq"], np.float32)
    wk = np.asarray(inputs["wk"], np.float32)
    bk = np.asarray(inputs["bk"], np.float32)
    wv = np.asarray(inputs["wv"], np.float32)
    bv = np.asarray(inputs["bv"], np.float32)
    wo = np.asarray(inputs["wo"], np.float32)
    bo = np.asarray(inputs["bo"], np.float32)

    if "nc" not in _PROGRAM_CACHE:
        _PROGRAM_CACHE["nc"] = build_program()
    nc = _PROGRAM_CACHE["nc"]

    in_maps = _host_inputs(x, wq, bq, wk, bk, wv, bv, wo)
    res = run_bass_kernel_spmd(
        nc, in_maps, list(range(N_CORES)), trace=_trace, tmpdir=_tmpdir,
    )
    acc = np.zeros((D_MODEL, T), np.float32)
    for c in range(N_CORES):
        acc += res.results[c]["outT"]
    # V-bias folds to a constant through softmax: + bv_cat @ wo (+ bo)
    const = bv.reshape(-1) @ wo + bo
    out = acc.T + const[None, :]
    if _trace:
        return out[None].astype(np.float32), res
    return out[None].astype(np.float32)


# revision 72
# speedup vs baseline: 1.2510x; 1.0004x over previous
"""Multi-head causal attention (B=1, T=4096, D=768, H=12) on 8 trn2 cores.

Sharding: per core, slot A = one full head (heads 0-7 across the 8 cores);
slot B = half of a split head (heads 8-11, each split across 2 cores by
token PARITY: core 2k gets even tokens of head 8+k, core 2k+1 odd tokens).
Parity-splitting keeps the causal key extents identical across cores, so
every core runs the IDENTICAL program (SPMD); cores differ only in data
(weights, masks, parity).  Slot B's queries are packed (parity-strided
projection); its head outputs are written back to natural token columns
with stride-2 DVE writes, so one merged out-projection covers both slots
and the host just sums the 8 partial [768, 4096] outputs.

Per-core work: slot A = 144 key-chunk units, slot B = 80 units (vs 288 for
the old 2-full-slot scheme).  V-bias is folded into a host-side constant
(P@(V+1 bv^T) = P@V + denom bv^T, exact through softmax normalization).

On-device layout (per core):
  xT    [768, 4096] bf16   x transposed (host supplies); xTB = parity cols
  K_sb  [128, 4096] bf16   rows 0:64 head-A K dims, 64:128 head-B K dims
  Q_sb  [128, 4096] bf16   rows 0:64 head-A Q (natural); rows 64:128 cols
                           0:2048 head-B Q (parity-packed)
  V2    [128, 32*208] bf16 per key-chunk: [V_A 0:64 |1@64| 0 |1@97| 0 |
                           V_B 129:193] -> one matmul per slot yields AV
                           rows + a denominator row (A: row 64, B: row 32)
  scores chunks [128 keys, 512 q] in PSUM, exp'd on ACT -> PT bf16

Scheduling: emission is software-pipelined — projection of token tile
tt+1 and deferred xT loads are woven between the attention score groups
of stage tt, and each group's mask+AV matmuls are delayed 8 groups behind
its exp so PE never stalls on the exp latency.  Softmax normalization
broadcasts 1/denom across partitions with a K=1 ones-matmul on PE (no
DRAM bounce).  Out-projection per 512-query tile -> bf16 partials.
"""

import math
import numpy as np
import ml_dtypes
from contextlib import ExitStack

import concourse.bass as bass
import concourse.bacc as bacc
import concourse.mybir as mybir
import concourse.tile as tile
from concourse.bass_utils import run_bass_kernel_spmd

BF16 = mybir.dt.bfloat16
F32 = mybir.dt.float32
F8 = mybir.dt.float8e4
DR = mybir.MatmulPerfMode.DoubleRow
AF = mybir.ActivationFunctionType

T = 4096
D_MODEL = 768
HEAD_DIM = 64
N_HEADS = 12
N_CORES = 8
QT = 512                  # query tile width (A natural / B packed)
KC = 128                  # key chunk (psum partition dim)
GRP = 2                   # score chunks per exp group (psum banks)
NQT = T // QT             # 8 A-tiles
NPB = 4                   # B packed tiles (each covers 1024 natural tokens)
CCH = D_MODEL // 128      # 6 contraction chunks
VST = 208                 # V2 stride per key chunk
NMASK = 13                # 4 A diag patterns + 8 B patterns + parity col mask

_PROGRAM_CACHE = {}


def build_program():
    nc = bacc.Bacc(None)

    xT_d = nc.declare_dram_parameter("xT", [D_MODEL, T], BF16, isOutput=False)
    # x columns of this core's parity, packed: x[:, parity::2].T
    xTB_d = nc.declare_dram_parameter("xTB", [D_MODEL, T // 2], BF16, isOutput=False)
    # w cols: 0:64 wq_A | 64:192 wk_AB | 192:320 wv_AB | 320:384 wq_B
    w_d = nc.declare_dram_parameter("wproj", [D_MODEL, 384], BF16, isOutput=False)
    b_d = nc.declare_dram_parameter("bqk", [128, 3], F32, isOutput=False)
    wo_d = nc.declare_dram_parameter("wo2", [128, D_MODEL], BF16, isOutput=False)
    mk_d = nc.declare_dram_parameter("masks", [NMASK, 128, QT], BF16, isOutput=False)
    outT_d = nc.declare_dram_parameter("outT", [D_MODEL, T], BF16, isOutput=True)

    with tile.TileContext(nc) as tc, ExitStack() as ctx:
        consts = ctx.enter_context(tc.tile_pool(name="consts", bufs=1))
        big = ctx.enter_context(tc.tile_pool(name="big", bufs=1))
        ptp = ctx.enter_context(tc.tile_pool(name="ptp", bufs=int(__import__("os").environ.get("KPTP", "16"))))
        osb = ctx.enter_context(tc.tile_pool(name="osb", bufs=int(__import__("os").environ.get("KOSB", "6"))))
        rp = ctx.enter_context(tc.tile_pool(name="rp", bufs=int(__import__("os").environ.get("KRP", "2"))))
        dramp = ctx.enter_context(tc.tile_pool(name="dramp", bufs=2, space="DRAM"))
        # PSUM budget is 8 banks total; knobs for the split
        import os as _os
        fp8 = _os.environ.get("KFP8", "0") == "1"
        grp = int(_os.environ.get("KGRP", str(GRP)))
        _scb = int(_os.environ.get("KSCB", "2"))
        _avp = int(_os.environ.get("KAVP", "2"))
        _opp = int(_os.environ.get("KOPP", "2"))
        scp = ctx.enter_context(tc.tile_pool(name="scp", bufs=_scb, space="PSUM"))
        avp = ctx.enter_context(tc.tile_pool(name="avp", bufs=_avp, space="PSUM"))
        if _opp > 0:
            opp = ctx.enter_context(
                tc.tile_pool(name="opp", bufs=_opp, space="PSUM"))
        else:
            opp = avp

        # ---- inputs to SBUF: small consts first, then xT streamed in
        # token-tile slices so stage-0 projection starts within ~4us ----
        w_sb = consts.tile([128, CCH * 384], BF16, tag="w")
        _wap = w_d[:, :]
        nc.sync.dma_start(
            out=w_sb[:],
            in_=bass.AP(tensor=_wap.tensor, offset=_wap.offset,
                        ap=[[384, 128], [128 * 384, CCH], [1, 384]]))
        b_sb = consts.tile([128, 3], F32, tag="b")
        nc.sync.dma_start(out=b_sb[:], in_=b_d[:, :])
        wo_sb = consts.tile([128, D_MODEL], BF16, tag="wo")
        mask_sb = consts.tile([128, NMASK * QT], BF16, tag="mask")
        # xT sliced per token-tile pair so stage-0/1 projection starts early;
        # first xTB slice interleaved (stage 1 needs it)
        xT_sb = [big.tile([128, T], BF16, tag=f"xT{j}", name=f"xT{j}")
                 for j in range(CCH)]
        xTB_sb = [big.tile([128, T // 2], BF16, tag=f"xTB{j}", name=f"xTB{j}")
                  for j in range(CCH)]

        def xt(j, cs):
            return xT_sb[j][:, cs]

        def xtb(j, cs):
            return xTB_sb[j][:, cs]

        def load_xt1(tt):
            cs = slice(tt * QT, (tt + 1) * QT)
            for j in range(CCH):
                nc.sync.dma_start(out=xT_sb[j][:, cs],
                                  in_=xT_d[j * 128:(j + 1) * 128, cs])

        def load_xt(tp):
            cs = slice(tp * 2 * QT, (tp + 1) * 2 * QT)
            for j in range(CCH):
                nc.sync.dma_start(out=xT_sb[j][:, cs],
                                  in_=xT_d[j * 128:(j + 1) * 128, cs])

        def load_xtb(pb):
            cs = slice(pb * QT, (pb + 1) * QT)
            for j in range(CCH):
                nc.sync.dma_start(out=xTB_sb[j][:, cs],
                                  in_=xTB_d[j * 128:(j + 1) * 128, cs])

        # only the slices stage 0/1 need right away; masks deferred behind
        # them; the rest are woven in as filler
        load_xt1(0)
        load_xt1(1)
        load_xtb(0)
        nc.sync.dma_start(out=wo_sb[:], in_=wo_d[:, :])
        _map = mk_d[:, :, :]
        nc.sync.dma_start(
            out=mask_sb[:],
            in_=bass.AP(tensor=_map.tensor, offset=_map.offset,
                        ap=[[QT, 128], [128 * QT, NMASK], [1, QT]]))

        # ---- persistent tensors ----
        QKDT = F8 if fp8 else BF16
        K_sb = big.tile([128, T], QKDT, tag="K")
        Q_sb = big.tile([128, T], QKDT, tag="Q")
        if fp8:
            # plane layout for DoubleRow: rows 0:32 slot A (head-dim planes
            # 0:32 / 32:64 at byte offsets 0 / T), rows 32:64 slot B
            K8p = big.tile([128, 2 * T], F8, tag="K8p")
            Q8p = big.tile([128, 2 * T], F8, tag="Q8p")
            K8p3 = K8p[:].rearrange("p (two n) -> p two n", two=2)
            Q8p3 = Q8p[:].rearrange("p (two n) -> p two n", two=2)
        V2 = big.tile([128, (T // KC) * VST], BF16, tag="V2")
        ho = big.tile([128, T], BF16, tag="ho")
        nc.gpsimd.memset(ho[:], 0.0)
        nc.gpsimd.memset(V2[:], 0.0)
        v3 = V2[:].rearrange("p (t c) -> p t c", c=VST)
        nc.gpsimd.memset(v3[:, :, 64:65], 1.0)    # ones row for denom_A
        nc.gpsimd.memset(v3[:, :, 97:98], 1.0)    # ones row for denom_B
        ones64 = consts.tile([128, 64], BF16, tag="ones64")
        nc.gpsimd.memset(ones64[:], 1.0)          # lhsT for recip broadcast

        def proj_units(tt):
            """List of thunks emitting projection for token tile tt."""
            ts = slice(tt * QT, (tt + 1) * QT)
            odd = tt % 2 == 1
            pb = (tt - 1) // 2
            st_ = {}
            units = []

            # NOTE: only ONE pending psum accumulation group per 2KB bank —
            # K (bank0) + Q_A (bank1) may interleave, but Q_B (also bank1)
            # and each V sub-group (all in pv bank0) must run after the
            # previous same-bank group has stopped.
            def u_kq(j):
                if j == 0:
                    st_["pk"] = scp.tile([128, grp * QT], F32, tag="sc",
                                         name="pk")
                pk = st_["pk"]
                rhs = xt(j, ts)
                st, sp = j == 0, j == CCH - 1
                nc.tensor.matmul(
                    pk[:, 0:QT], w_sb[:, j * 384 + 64:j * 384 + 192], rhs,
                    start=st, stop=sp, tile_position=(0, 0),
                )
                nc.tensor.matmul(
                    pk[0:64, QT:2 * QT], w_sb[:, j * 384:j * 384 + 64], rhs,
                    start=st, stop=sp, tile_position=(0, 0),
                )

            def u_qb(j):
                pk = st_["pk"]
                nc.tensor.matmul(
                    pk[64:128, QT:2 * QT],
                    w_sb[:, j * 384 + 320:j * 384 + 384],
                    xtb(j, slice(pb * QT, (pb + 1) * QT)),
                    start=(j == 0), stop=(j == CCH - 1), tile_position=(0, 64),
                )

            def u_cast_kqa():
                pk = st_["pk"]
                nc.vector.tensor_scalar_add(K_sb[:, ts], pk[:, 0:QT], b_sb[:, 1:2])
                nc.vector.tensor_scalar_add(
                    Q_sb[0:64, ts], pk[0:64, QT:2 * QT], b_sb[0:64, 0:1])

            def u_regroup_kqa():
                # fp8 plane regroup: flat rows (4 groups of 32) -> plane
                # layout rows 0:32 (A) / 32:64 (B), byte offset 0 / T.
                # SWDGE (gpsimd) path keeps these off the busy HWDGE.
                for src0, dst0, pl in ((0, 0, 0), (32, 0, 1),
                                       (64, 32, 0), (96, 32, 1)):
                    nc.gpsimd.dma_start(
                        out=K8p3[dst0:dst0 + 32, pl:pl + 1, ts],
                        in_=K_sb[src0:src0 + 32, ts])
                for src0, pl in ((0, 0), (32, 1)):
                    nc.gpsimd.dma_start(
                        out=Q8p3[0:32, pl:pl + 1, ts],
                        in_=Q_sb[src0:src0 + 32, ts])

            def u_cast_qb():
                pk = st_["pk"]
                nc.vector.tensor_scalar_add(
                    Q_sb[64:128, pb * QT:(pb + 1) * QT],
                    pk[64:128, QT:2 * QT], b_sb[64:128, 2:3])

            def u_regroup_qb():
                pbs = slice(pb * QT, (pb + 1) * QT)
                for src0, pl in ((64, 0), (96, 1)):
                    nc.gpsimd.dma_start(
                        out=Q8p3[32:64, pl:pl + 1, pbs],
                        in_=Q_sb[src0:src0 + 32, pbs])

            def u_v(sub):
                if sub == 0:
                    st_["pv"] = scp.tile([128, grp * QT], F32, tag="sc",
                                         name="pv")
                pv = st_["pv"]
                kc = tt * 4 + sub
                for j in range(CCH):
                    nc.tensor.matmul(
                        pv[:, sub * 128:(sub + 1) * 128],
                        xt(j, slice(kc * KC, (kc + 1) * KC)),
                        w_sb[:, j * 384 + 192:j * 384 + 320],
                        start=(j == 0), stop=(j == CCH - 1), tile_position=(0, 0),
                    )

            def u_vcopy():
                pv = st_["pv"]
                for sub in range(4):
                    kc = tt * 4 + sub
                    blk = V2[:, kc * VST:kc * VST + 193]
                    out_ap = bass.AP(tensor=blk.tensor, offset=blk.offset,
                                     ap=[list(blk.ap[0]), [129, 2], [1, 64]])
                    nc.vector.tensor_copy(out_ap, pv[:, sub * 128:(sub + 1) * 128])

            for j in range(CCH):
                units.append(lambda j=j: u_kq(j))
            units.append(u_cast_kqa)
            if fp8:
                units.append(u_regroup_kqa)
            if odd:
                for j in range(CCH):
                    units.append(lambda j=j: u_qb(j))
                units.append(u_cast_qb)
                if fp8:
                    units.append(u_regroup_qb)
            for sub in range(4):
                units.append(lambda sub=sub: u_v(sub))
            units.append(u_vcopy)
            return units

        def job_units(slot, i, avd=None):
            """slot 'A': full head, query tile i (natural); slot 'B': split
            head, packed tile i.  Returns list of thunks (one per score
            group + a normalize tail)."""
            if slot == "A":
                nst, band0 = 4 * (i + 1), 4 * i
                krow, tp = 0, (0, 0)
            else:
                nst, band0 = 8 * (i + 1), 8 * i
                krow, tp = 64, (64, 0)
            qrow = slice(krow, krow + 64)
            st_ = {}

            def flush_av(force=False):
                # masks + AV for a group exp'd earlier; the delay keeps PE
                # from stalling on the exp latency
                import os as _os2
                depth = avd if avd is not None else int(_os2.environ.get("KAVD", "12"))
                pend = st_.setdefault("pendq", [])
                if not pend or (not force and len(pend) <= depth - 1):
                    return
                pt, g0, w = pend.pop(0)
                av = st_["av"]
                for gi in range(w):
                    kc = g0 + gi
                    # valid-query truncation: for diagonal-band chunks,
                    # queries below qoff are entirely masked-out, so the
                    # mask mul and AV matmul (incl. denom row) skip them —
                    # exact, since those queries don't attend these keys.
                    qoff = 0
                    if kc >= band0:
                        pat = kc - band0
                        mi = pat if slot == "A" else 4 + pat
                        qoff = (128 if slot == "A" else 64) * pat
                        nc.vector.tensor_mul(
                            pt[:, gi * QT + qoff:(gi + 1) * QT],
                            pt[:, gi * QT + qoff:(gi + 1) * QT],
                            mask_sb[:, mi * QT + qoff:(mi + 1) * QT])
                    ptj = pt[:, gi * QT + qoff:(gi + 1) * QT]
                    st, sp = kc == 0, kc == nst - 1
                    if slot == "A":
                        nc.tensor.matmul(
                            av[0:65, qoff:QT], V2[:, kc * VST:kc * VST + 65],
                            ptj, start=st, stop=sp, tile_position=(0, 0),
                        )
                    else:
                        nc.tensor.matmul(
                            av[0:128, qoff:QT],
                            V2[:, kc * VST + 65:kc * VST + 193],
                            ptj, start=st, stop=sp, tile_position=(0, 0),
                        )

            def _qoff(kc):
                # valid-query start for diagonal-band chunks: queries below
                # this are strictly non-causal for every key in the chunk
                if kc < band0:
                    return 0
                return (128 if slot == "A" else 64) * (kc - band0)

            def u_group(g0):
                if g0 == 0:
                    st_["av"] = avp.tile([128, QT], F32, tag="av", name="av")
                w = min(grp, nst - g0)
                sc = scp.tile([128, grp * QT], F32, tag="sc", name="sc")
                qoffs = [_qoff(g0 + gi) for gi in range(w)]
                for gi in range(w):
                    kc = g0 + gi
                    qo = qoffs[gi]
                    if fp8:
                        prow = 0 if slot == "A" else 32
                        nc.tensor.matmul(
                            sc[:, gi * QT + qo:(gi + 1) * QT],
                            K8p3[prow:prow + 32, :, kc * KC:(kc + 1) * KC],
                            Q8p3[prow:prow + 32, :, i * QT + qo:(i + 1) * QT],
                            start=True, stop=True, perf_mode=DR,
                            tile_position=(prow, 0),
                        )
                    else:
                        nc.tensor.matmul(
                            sc[:, gi * QT + qo:(gi + 1) * QT],
                            K_sb[krow:krow + 64, kc * KC:(kc + 1) * KC],
                            Q_sb[qrow, i * QT + qo:(i + 1) * QT],
                            start=True, stop=True, tile_position=tp,
                        )
                pt = ptp.tile([128, grp * QT], BF16, tag="pt", name="pt")
                if any(qoffs):
                    # ragged group: exp per chunk over its valid sub-range
                    # (reads only psum the truncated score matmul wrote)
                    for gi in range(w):
                        qo = qoffs[gi]
                        nc.scalar.activation(
                            pt[:, gi * QT + qo:(gi + 1) * QT],
                            sc[:, gi * QT + qo:(gi + 1) * QT], AF.Exp,
                            scale=1.0 / math.sqrt(HEAD_DIM))
                else:
                    nc.scalar.activation(
                        pt[:, 0:w * QT], sc[:, 0:w * QT], AF.Exp,
                        scale=1.0 / math.sqrt(HEAD_DIM))
                st_.setdefault("pendq", []).append((pt, g0, w))
                flush_av()

            def u_norm():
                while st_.get("pendq"):
                    flush_av(force=True)
                # normalize: bf16 reciprocal of the denom row, broadcast
                # across 64 partitions via a K=1 ones-matmul on PE (no DMA)
                av = st_["av"]
                drow = 64 if slot == "A" else 32
                rows = slice(0, 64) if slot == "A" else slice(64, 128)
                r = rp.tile([128, QT], BF16, tag="r", name="r")
                with nc.allow_low_precision(reason="softmax denom recip bf16"):
                    nc.vector.reciprocal(r[drow:drow + 1, :], av[drow:drow + 1, :])
                rbc_ps = avp.tile([128, QT], F32, tag="av", name="rbc_ps")
                nc.tensor.matmul(
                    rbc_ps[rows, :], ones64[drow:drow + 1, :], r[drow:drow + 1, :],
                    start=True, stop=True, tile_position=(drow, rows.start),
                )
                rbc = rp.tile([128, QT], F32, tag="rbc", name="rbc")
                nc.vector.tensor_copy(rbc[rows, :], rbc_ps[rows, :])
                if slot == "A":
                    nc.vector.tensor_mul(
                        ho[0:64, i * QT:(i + 1) * QT], av[0:64, :], rbc[0:64, :])
                else:
                    # write packed value j to BOTH natural columns 2j, 2j+1;
                    # the per-core parity column mask (data) zeroes the
                    # wrong one right before the out-projection.
                    hob = ho[64:128, 1024 * i:1024 * (i + 1)].rearrange(
                        "p (n two) -> p two n", two=2)
                    nc.vector.tensor_mul(hob[:, 0:1, :], av[64:128, :],
                                         rbc[64:128, :])
                    nc.vector.tensor_mul(hob[:, 1:2, :], av[64:128, :],
                                         rbc[64:128, :])

            units = [lambda g0=g0: u_group(g0) for g0 in range(0, nst, grp)]
            units.append(u_norm)
            return units

        def outproj_units(qt):
            qs = slice(qt * QT, (qt + 1) * QT)

            def u_pmask():
                nc.vector.tensor_mul(
                    ho[64:128, qs], ho[64:128, qs],
                    mask_sb[64:128, 12 * QT:13 * QT])

            def u_op(dch):
                op = opp.tile([128, QT], F32, tag="av" if opp is avp else "op", name="op")
                nc.tensor.matmul(
                    op[:], wo_sb[:, dch * 128:(dch + 1) * 128],
                    ho[:, qs], start=True, stop=True, tile_position=(0, 0),
                )
                ot = osb.tile([128, QT], BF16, tag="ot", name="ot")
                if qt >= 6 or dch % 2 == 1:
                    nc.scalar.copy(ot[:], op[:])
                else:
                    nc.vector.tensor_copy(ot[:], op[:])
                nc.sync.dma_start(
                    out=outT_d[dch * 128:(dch + 1) * 128, qs], in_=ot[:])

            return [u_pmask] + [lambda d=d: u_op(d) for d in range(CCH)]

        def weave(main, filler):
            """Emit `main` units with `filler` units distributed evenly."""
            if not main:
                for f in filler:
                    f()
                return
            nf, nm = len(filler), len(main)
            fi = 0
            for k, u in enumerate(main):
                u()
                while fi * nm < (k + 1) * nf:
                    filler[fi]()
                    fi += 1
            while fi < nf:
                filler[fi]()
                fi += 1

        # ---- software-pipelined emission: proj(tt+1) + deferred input
        # loads woven into jobs(tt) --
        for u in proj_units(0):
            u()
        for tt in range(NQT):
            # A_tt woven with proj(tt+1); then (even tt) B_{tt//2} — safe
            # only after ALL proj(tt+1) units, since its diagonal-band
            # chunks read K/V of tile tt+1
            stream = job_units("A", tt, avd=2 if tt == NQT - 1 else None)
            filler = []
            if tt in (0, 2, 4):
                k = tt // 2 + 1
                filler.append(lambda k=k: load_xt(k))
                filler.append(lambda k=k: load_xtb(k))
            filler += proj_units(tt + 1) if tt + 1 < NQT else []
            weave(stream, filler)
            if tt % 2 == 0:
                for u in job_units("B", tt // 2,
                                   avd=2 if tt == NQT - 2 else None):
                    u()
                for u in outproj_units(tt):
                    u()
            else:
                for u in outproj_units(tt):
                    u()
    nc.finalize()
    return nc


def _host_inputs(x, wq, bq, wk, bk, wv, bv, wo):
    """Per-core input maps. Slot A of core c = head c; slot B = split head
    8 + c//2 with token parity c%2."""
    bf16 = ml_dtypes.bfloat16
    xT = np.ascontiguousarray(x[0].T).astype(bf16)
    xTB_by_par = [np.ascontiguousarray(x[0][p::2].T).astype(bf16) for p in (0, 1)]

    in_maps = []
    for c in range(N_CORES):
        ha, hb, par = c, 8 + c // 2, c % 2
        w = np.zeros((D_MODEL, 384), np.float32)
        w[:, 0:64] = wq[ha]
        w[:, 64:128] = wk[ha]
        w[:, 128:192] = wk[hb]
        w[:, 192:256] = wv[ha]
        w[:, 256:320] = wv[hb]
        w[:, 320:384] = wq[hb]
        b = np.zeros((128, 3), np.float32)
        b[0:64, 0] = bq[ha]
        b[0:64, 1] = bk[ha]
        b[64:128, 1] = bk[hb]
        b[64:128, 2] = bq[hb]
        wo2 = np.zeros((128, D_MODEL), np.float32)
        wo2[0:64] = wo[ha * 64:(ha + 1) * 64]
        wo2[64:128] = wo[hb * 64:(hb + 1) * 64]
        kl = np.arange(128)[:, None]
        qq = np.arange(QT)[None, :]
        masks = np.zeros((NMASK, 128, QT), np.float32)
        for pat in range(4):
            masks[pat] = (128 * pat + kl) <= qq
        for pat in range(8):
            masks[4 + pat] = (128 * pat + kl) <= (2 * qq + par)
        masks[12, :, :] = (qq % 2 == par)
        in_maps.append({
            "xT": xT,
            "xTB": xTB_by_par[par],
            "wproj": w.astype(bf16),
            "bqk": b.astype(np.float32),
            "wo2": wo2.astype(bf16),
            "masks": masks.astype(bf16),
        })
    return in_maps


def kernel(_trace=False, _tmpdir=None, **inputs):
    x = np.asarray(inputs["x"], np.float32)
    wq = np.asarray(inputs["wq"], np.float32)
    bq = np.asarray(inputs["bq"], np.float32)
    wk = np.asarray(inputs["wk"], np.float32)
    bk = np.asarray(inputs["bk"], np.float32)
    wv = np.asarray(inputs["wv"], np.float32)
    bv = np.asarray(inputs["bv"], np.float32)
    wo = np.asarray(inputs["wo"], np.float32)
    bo = np.asarray(inputs["bo"], np.float32)

    if "nc" not in _PROGRAM_CACHE:
        _PROGRAM_CACHE["nc"] = build_program()
    nc = _PROGRAM_CACHE["nc"]

    in_maps = _host_inputs(x, wq, bq, wk, bk, wv, bv, wo)
    res = run_bass_kernel_spmd(
        nc, in_maps, list(range(N_CORES)), trace=_trace, tmpdir=_tmpdir,
    )
    acc = np.zeros((D_MODEL, T), np.float32)
    for c in range(N_CORES):
        acc += res.results[c]["outT"]
    # V-bias folds to a constant through softmax: + bv_cat @ wo (+ bo)
    const = bv.reshape(-1) @ wo + bo
    out = acc.T + const[None, :]
    if _trace:
        return out[None].astype(np.float32), res
    return out[None].astype(np.float32)
